# revision 1
# baseline (speedup 1.0000x reference)
"""Trainium2 Bass kernel for nn_AttentionSubLayer (dense transformer attention
sublayer with time-lerp K/V mixing, QK-norm, RoPE, GQA, per-head l2 output
norm, gating, out-proj + final RMS norm).

Sharding: 8 cores = 2 batch groups x 4-way sequence parallel with causal
load balancing.  Core c handles batch c//4 and query blocks {p, 7-p}
(256 tokens each, p = c%4).  K/V projections are computed on the owning
quarter of the sequence and AllGathered within each 4-core batch group.
No other communication; each core computes its out-proj rows and final
RMS norm locally.

Numerics: fp32 data; matmuls run in float32r (full PE rate for moving dim
>= 256).  float32r operands must be produced by a rounding instruction, so
every matmul input tile is either written by the scalar engine (copy / exp /
square) or DMA'd from an f32r-typed DRAM tensor.  Softmax skips the
max-subtraction (scores are bounded by sqrt(HD) after QK rms-norm) and the
denominator entirely (the subsequent per-head l2 norm cancels it).  Causal /
validity masking is additive pre-exp (host-supplied tiles).  All rsqrts are
exp(-0.5*ln(x)) so the scalar engine stays in one LUT table set.
"""

import math
import sys
import types
from contextlib import ExitStack

sys.path.insert(0, "/opt/trn_rl_repo")

import numpy as np

# ---------------------------------------------------------------- problem dims
B, T, D, H, KVH, HD = 2, 2048, 2048, 16, 4, 128
N_LAYER = 24
EPS = 1e-8
NCORE = 8
TB = 256          # token block for attention tiling
NBLK = T // TB    # 8 blocks per batch
QTOK = 2 * TB     # 512 q tokens per core
KVTOK = 2 * TB    # 512 kv tokens per core (contiguous quarter)
INV_SQRT_HD = 1.0 / math.sqrt(HD)
OUT_SCALE = 2 * N_LAYER  # final rms divided by sqrt(2*N_LAYER)
MASK_NEG = -60000.0


def _install_ntff_hook():
    try:
        import antenv
        if "antenv.axon_hooks" in sys.modules:
            return
        from trn_agent_boot.trn_boot import _ntff_profile_via_ctypes
        hook = _ntff_profile_via_ctypes("/opt/axon/libaxon_pjrt.so")
        mod = types.ModuleType("antenv.axon_hooks")
        mod.get_axon_ntff_profile_hook = lambda: hook
        antenv.axon_hooks = mod
        sys.modules["antenv.axon_hooks"] = mod
    except Exception:
        pass


_CACHE = {}


def _build():
    if "nc" in _CACHE:
        return _CACHE["nc"]
    import os
    phases = os.environ.get("KERN_PHASES", "1234")

    import concourse.bass as bass
    import concourse.mybir as mybir
    import concourse.tile as tile
    from concourse import bacc
    from concourse.masks import make_identity

    f32 = mybir.dt.float32
    f32r = mybir.dt.float32r
    bf16 = mybir.dt.bfloat16
    AF = mybir.ActivationFunctionType
    ALU = mybir.AluOpType

    def bc_free(ap, n, at):
        """Insert a broadcast (stride-0) free dim of size n at position `at`
        of the AP's dim list (position counted incl. partition dim 0)."""
        new = list(list(d) for d in ap.ap)
        new.insert(at, [0, n])
        return bass.AP(tensor=ap.tensor, offset=ap.offset, ap=new)

    nc = bacc.Bacc("TRN2", target_bir_lowering=False, debug=False,
                   num_devices=NCORE)

    # ------------------------------------------------------------- I/O tensors
    xq_sh = nc.dram_tensor("xq_sh", [QTOK, D], f32, kind="ExternalInput")
    xk_sh = nc.dram_tensor("xk_sh", [KVTOK + 128, D], f32, kind="ExternalInput")
    xv_sh = nc.dram_tensor("xv_sh", [KVTOK + 128, D], f32, kind="ExternalInput")
    Wq = nc.dram_tensor("Wq", [D, H * HD], f32r, kind="ExternalInput")
    Wg = nc.dram_tensor("Wg", [D, H * HD], f32r, kind="ExternalInput")
    Wo = nc.dram_tensor("Wo", [H * HD, D], f32r, kind="ExternalInput")
    Wk1 = nc.dram_tensor("Wk1", [D, KVH * HD], f32r, kind="ExternalInput")
    Wk2 = nc.dram_tensor("Wk2", [D, KVH * HD], f32r, kind="ExternalInput")
    Wv1 = nc.dram_tensor("Wv1", [D, KVH * HD], f32r, kind="ExternalInput")
    Wv2 = nc.dram_tensor("Wv2", [D, KVH * HD], f32r, kind="ExternalInput")
    cos_q = nc.dram_tensor("cos_q", [QTOK, HD], f32, kind="ExternalInput")
    sin_q = nc.dram_tensor("sin_q", [QTOK, HD], f32, kind="ExternalInput")
    cos_k = nc.dram_tensor("cos_k", [KVTOK, HD], f32, kind="ExternalInput")
    sin_k = nc.dram_tensor("sin_k", [KVTOK, HD], f32, kind="ExternalInput")
    mask_all = nc.dram_tensor("mask_all", [12, 128, 2 * TB], bf16,
                              kind="ExternalInput")
    out_y = nc.dram_tensor("out_y", [QTOK, D], f32, kind="ExternalOutput")

    # staging for K/V allgather (within 4-core batch group)
    SHARD = KVH * HD * KVTOK
    kv_loc = nc.dram_tensor("kv_loc", [2, SHARD], f32r)
    kv_gath = nc.dram_tensor("kv_gath", [4, 2, SHARD], f32r)
    k_loc_v = kv_loc[0].rearrange("(kv hd t) -> kv hd t", kv=KVH, hd=HD)
    v_loc_v = kv_loc[1].rearrange("(t kv hd) -> t kv hd", kv=KVH, hd=HD)

    with tile.TileContext(nc) as tc, ExitStack() as es:
        # ------------------------------------------------------------ constants
        cpool = es.enter_context(tc.tile_pool(name="consts", bufs=1))
        ident = cpool.tile([128, 128], f32)
        make_identity(nc, ident[:])
        ones_f = cpool.tile([128, 1], f32)
        nc.vector.memset(ones_f[:], 1.0)
        ones_rf = cpool.tile([1, 128], f32)
        nc.vector.memset(ones_rf[:], 1.0)
        eps_t = cpool.tile([128, 1], f32)
        nc.vector.memset(eps_t[:], EPS)
        oeps_t = cpool.tile([128, 1], f32)
        nc.vector.memset(oeps_t[:], float(OUT_SCALE) * EPS)
        cosq_sb = cpool.tile([128, 4, HD], f32)
        sinq_sb = cpool.tile([128, 4, HD], f32)
        cosk_sb = cpool.tile([128, 4, HD], f32)
        sink_sb = cpool.tile([128, 4, HD], f32)
        for m in range(4):
            nc.sync.dma_start(out=cosq_sb[:, m, :], in_=cos_q[128 * m:128 * m + 128, :])
            nc.sync.dma_start(out=sinq_sb[:, m, :], in_=sin_q[128 * m:128 * m + 128, :])
            nc.sync.dma_start(out=cosk_sb[:, m, :], in_=cos_k[128 * m:128 * m + 128, :])
            nc.sync.dma_start(out=sink_sb[:, m, :], in_=sin_k[128 * m:128 * m + 128, :])


        # ============================================================ helpers
        def transpose_in(x_dram, xT, nrows, natp, ptp):
            """Load natural [nrows, D] DRAM -> xT [128, 16, ncols] transposed
            (scalar-engine evacuation rounds to xT's dtype)."""
            nfull = nrows // 128
            for m in range(nfull):
                nat = natp.tile([128, D], f32, tag="nat")
                nc.sync.dma_start(out=nat[:], in_=x_dram[128 * m:128 * m + 128, :])
                for k in range(16):
                    pst = ptp.tile([128, 128], f32, tag="pst")
                    nc.tensor.transpose(pst[:], nat[:, 128 * k:128 * k + 128], ident[:])
                    nc.scalar.copy(out=xT[:, k, 128 * m:128 * m + 128], in_=pst[:])

        def rms_scale(x_t, nh, smp):
            """In-place x *= rsqrt(mean(x^2 over HD) + EPS); x_t [128, nh*HD]."""
            s2 = smp.tile([128, nh], f32, tag="rs2")
            scrap = smp.tile([128, HD], f32, tag="rscrap")
            for h in range(nh):
                sl = x_t[:, 128 * h:128 * h + 128]
                nc.vector.tensor_tensor(out=scrap[:], in0=sl, in1=sl, op=ALU.mult)
                nc.vector.tensor_reduce(out=s2[:, h:h + 1], in_=scrap[:],
                                        axis=mybir.AxisListType.X, op=ALU.add)
            ln = smp.tile([128, nh], f32, tag="rln")
            nc.scalar.activation(out=ln[:], in_=s2[:], func=AF.Ln,
                                 bias=eps_t[:], scale=1.0 / HD)
            ri = smp.tile([128, nh], f32, tag="rri")
            nc.scalar.activation(out=ri[:], in_=ln[:], func=AF.Exp, scale=-0.5)
            for h in range(nh):
                sl = x_t[:, 128 * h:128 * h + 128]
                nc.vector.tensor_scalar_mul(sl, sl, ri[:, h:h + 1])

        def rope(dst_t, src_t, nh, cos_sb, sin_sb, m, smp):
            """dst = rope(src), per-head standard ops; cos/sin tiles [128,4,HD]."""
            half = HD // 2
            cos_t = cos_sb[:, m, :]
            sin_lo = sin_sb[:, m, 0:half]
            sin_hi = sin_sb[:, m, half:HD]
            t1 = smp.tile([128, half], f32, tag="ro1")
            for h in range(nh):
                d = dst_t[:, 128 * h:128 * h + 128]
                s = src_t[:, 128 * h:128 * h + 128]
                d_lo = dst_t[:, 128 * h:128 * h + half]
                d_hi = dst_t[:, 128 * h + half:128 * h + 128]
                s_lo = src_t[:, 128 * h:128 * h + half]
                s_hi = src_t[:, 128 * h + half:128 * h + 128]
                nc.vector.tensor_tensor(out=d, in0=s, in1=cos_t, op=ALU.mult)
                nc.vector.tensor_tensor(out=t1[:], in0=s_hi, in1=sin_lo, op=ALU.mult)
                nc.vector.tensor_tensor(out=d_lo, in0=d_lo, in1=t1[:], op=ALU.subtract)
                nc.vector.tensor_tensor(out=t1[:], in0=s_lo, in1=sin_hi, op=ALU.mult)
                nc.vector.tensor_tensor(out=d_hi, in0=d_hi, in1=t1[:], op=ALU.add)

        # ===================================================== phase 1: K / V
        stage_dmas = []
        with tc.tile_pool(name="p1nat", bufs=2) as natp, \
             tc.tile_pool(name="p1pst", bufs=2, space="PSUM") as ptp, \
             tc.tile_pool(name="p1xt", bufs=1) as xtp, \
             tc.tile_pool(name="p1w", bufs=3) as wp, \
             tc.tile_pool(name="p1kv", bufs=3) as kvp, \
             tc.tile_pool(name="p1ps", bufs=1, space="PSUM") as pskv, \
             tc.tile_pool(name="p1sm", bufs=3) as smp:
            for (x_dram, W1, W2, is_k) in ((xk_sh, Wk1, Wk2, True),
                                           (xv_sh, Wv1, Wv2, False)):
                xT = xtp.tile([128, 16, KVTOK + 128], f32r, tag="xT",
                              name="xkT" if is_k else "xvT")
                transpose_in(x_dram, xT, KVTOK + 128, natp, ptp)
                ps = [pskv.tile([128, KVH * HD], f32, tag=f"pkv{m}", name=f"pkv{m}")
                      for m in range(4)]
                for k in range(16):
                    w1t = wp.tile([128, KVH * HD], f32r, tag="w1")
                    nc.sync.dma_start(out=w1t[:], in_=W1[128 * k:128 * k + 128, :])
                    w2t = wp.tile([128, KVH * HD], f32r, tag="w2")
                    nc.sync.dma_start(out=w2t[:], in_=W2[128 * k:128 * k + 128, :])
                    for m in range(4):
                        nc.tensor.matmul(ps[m][:],
                                         xT[:, k, 128 + 128 * m:256 + 128 * m],
                                         w1t[:], start=(k == 0), stop=False)
                        nc.tensor.matmul(ps[m][:],
                                         xT[:, k, 127 + 128 * m:255 + 128 * m],
                                         w2t[:], start=False, stop=(k == 15))
                for m in range(4):
                    nat = kvp.tile([128, KVH * HD], f32, tag="kvnat")
                    nc.scalar.copy(out=nat[:], in_=ps[m][:])
                    rms_scale(nat, KVH, smp)
                    if is_k:
                        rot = kvp.tile([128, KVH * HD], f32, tag="krot")
                        rope(rot, nat, KVH, cosk_sb, sink_sb, m, smp)
                        for kv in range(KVH):
                            pst = ptp.tile([128, 128], f32, tag="pst")
                            nc.tensor.transpose(pst[:], rot[:, 128 * kv:128 * kv + 128],
                                                ident[:])
                            kst = kvp.tile([128, 128], f32r, tag="kst")
                            nc.scalar.copy(out=kst[:], in_=pst[:])
                            d = nc.sync.dma_start(
                                out=k_loc_v[kv, :, 128 * m:128 * m + 128], in_=kst[:])
                            stage_dmas.append(d)
                    else:
                        vr = kvp.tile([128, KVH * HD], f32r, tag="vr")
                        nc.scalar.copy(out=vr[:], in_=nat[:])
                        d = nc.sync.dma_start(
                            out=v_loc_v[128 * m:128 * m + 128, :, :],
                            in_=vr[:].rearrange("p (h d) -> p h d", h=KVH))
                        stage_dmas.append(d)

        ag_k = nc.gpsimd.collective_compute(
            "AllGather", ALU.bypass,
            replica_groups=[[0, 1, 2, 3], [4, 5, 6, 7]],
            ins=[kv_loc[:]], outs=[kv_gath[:]])
        for d in stage_dmas:
            tile.add_dep_helper(ag_k.ins, d.ins, reason="stage before allgather")

        # ===================================================== phase 2: Q / G
        p_gT = es.enter_context(tc.tile_pool(name="ppgT", bufs=1))
        gT_sb = p_gT.tile([128, H, QTOK], f32, tag="gT", name="gT_sb")
        p_qT = es.enter_context(tc.tile_pool(name="ppqT", bufs=1))
        qT_sb = p_qT.tile([128, H, QTOK], f32r, tag="qT", name="qT_sb")
        with tc.tile_pool(name="p2nat", bufs=2) as natp, \
             tc.tile_pool(name="p2pst", bufs=2, space="PSUM") as ptp, \
             tc.tile_pool(name="p2xt", bufs=1) as xtp, \
             tc.tile_pool(name="p2w", bufs=3) as wp, \
             tc.tile_pool(name="p2q", bufs=1) as qp, \
             tc.tile_pool(name="p2ps", bufs=1, space="PSUM") as psq, \
             tc.tile_pool(name="p2sm", bufs=2) as smp:
            xqT = xtp.tile([128, 16, QTOK], f32r, tag="xqT")
            transpose_in(xq_sh, xqT, QTOK, natp, ptp)

            # G projection -> transposed [gcol, tok] directly
            for gq in range(4):
                psg = [psq.tile([128, 512], f32, tag=f"pp{i}", name=f"pg{i}") for i in range(4)]
                for k in range(16):
                    wgt = wp.tile([128, 512], f32r, tag="wg")
                    nc.sync.dma_start(out=wgt[:],
                                      in_=Wg[128 * k:128 * k + 128, 512 * gq:512 * gq + 512])
                    for gi in range(4):
                        nc.tensor.matmul(
                            psg[gi][:],
                            wgt[:, 128 * gi:128 * gi + 128],
                            xqT[:, k, :],
                            start=(k == 0), stop=(k == 15))
                for gi in range(4):
                    nc.scalar.copy(out=gT_sb[:, 4 * gq + gi, :], in_=psg[gi][:])

            # Q projection -> natural [tok, H*HD]
            q_sb = [qp.tile([128, H * HD], f32, tag=f"q{m}", name=f"q{m}") for m in range(4)]
            for n in range(4):
                ps = [psq.tile([128, 512], f32, tag=f"pp{m}", name=f"pq{m}") for m in range(4)]
                for k in range(16):
                    wqt = wp.tile([128, 512], f32r, tag="wq")
                    nc.sync.dma_start(out=wqt[:],
                                      in_=Wq[128 * k:128 * k + 128, 512 * n:512 * n + 512])
                    for m in range(4):
                        nc.tensor.matmul(ps[m][:],
                                         xqT[:, k, 128 * m:128 * m + 128],
                                         wqt[:], start=(k == 0), stop=(k == 15))
                for m in range(4):
                    nc.scalar.copy(out=q_sb[m][:, 512 * n:512 * n + 512], in_=ps[m][:])

            # rms + rope + transpose q
            for m in range(4):
                rms_scale(q_sb[m], H, smp)
                rot = smp.tile([128, H * HD], f32, tag="qrot")
                rope(rot, q_sb[m], H, cosq_sb, sinq_sb, m, smp)
                for h in range(H):
                    pst = ptp.tile([128, 128], f32, tag="pst")
                    nc.tensor.transpose(pst[:], rot[:, 128 * h:128 * h + 128], ident[:])
                    nc.scalar.copy(out=qT_sb[:, h, 128 * m:128 * m + 128], in_=pst[:])

        if "3" not in phases:
            # debug: write g instead of attention output
            with tc.tile_pool(name="dbg", bufs=2) as dbp:
                for m in range(4):
                    t = dbp.tile([128, D], f32, tag="dbg")
                    nc.vector.tensor_copy(out=t[:], in_=gT_sb[:, 4 * m:4 * m + 4, :].rearrange("p a b -> p (a b)"))
                    nc.sync.dma_start(out=out_y[128 * m:128 * m + 128, :], in_=t[:])

        # ==================================================== phase 3: attention
        p_gTr = es.enter_context(tc.tile_pool(name="ppgTr", bufs=1))
        gTr_sb = p_gTr.tile([128, H, QTOK], f32r, tag="gTr", name="gTr_sb")
        if "3" in phases:
          with tc.tile_pool(name="p3m", bufs=1) as mp, \
               tc.tile_pool(name="p3kv", bufs=2) as kvp, \
               tc.tile_pool(name="p3pt", bufs=3) as ptq, \
               tc.tile_pool(name="p3ps", bufs=2, space="PSUM") as pss_p, \
               tc.tile_pool(name="p3py", bufs=2, space="PSUM") as psy_p, \
               tc.tile_pool(name="p3pn", bufs=1, space="PSUM") as psn_p, \
               tc.tile_pool(name="p3sm", bufs=4) as smp:
              masks_sb = mp.tile([128, 12, 2 * TB], bf16, tag="masks")
              for s in range(12):
                  nc.sync.dma_start(out=masks_sb[:, s, :], in_=mask_all[s])

              kload = []
              for kv in range(KVH):
                  K_sb = kvp.tile([128, NBLK, TB], f32r, tag="K")
                  V_sb = kvp.tile([128, 2 * NBLK, 128], f32r, tag="V")
                  for j in range(NBLK):
                      kg = kv_gath[j // 2, 0].rearrange(
                          "(kv hd t) -> kv hd t", kv=KVH, hd=HD)
                      vg = kv_gath[j // 2, 1].rearrange(
                          "(t kv hd) -> t kv hd", kv=KVH, hd=HD)
                      d = nc.sync.dma_start(
                          out=K_sb[:, j, :],
                          in_=kg[kv, :, TB * (j % 2):TB * (j % 2) + TB])
                      kload.append(d)
                      for ss in range(2):
                          base = TB * (j % 2) + 128 * ss
                          d = nc.sync.dma_start(
                              out=V_sb[:, 2 * j + ss, :],
                              in_=vg[base:base + 128, kv, :])
                          kload.append(d)
                  for hi in range(4):
                      h = 4 * kv + hi
                      for s01, nblk in ((0, 4), (1, NBLK)):
                          psy = psy_p.tile([128, TB], f32, tag="psy")
                          for i in range(nblk):
                              pss = pss_p.tile([128, 2 * TB], f32, tag="pss")
                              for ss in range(2):
                                  nc.tensor.matmul(
                                      pss[:, TB * ss:TB * ss + TB],
                                      K_sb[:, i, 128 * ss:128 * ss + 128],
                                      qT_sb[:, h, TB * s01:TB * s01 + TB],
                                      start=True, stop=True)
                              sidx = i if s01 == 0 else 4 + i
                              sm_t = smp.tile([128, 2 * TB], f32, tag="smt")
                              nc.vector.scalar_tensor_tensor(
                                  out=sm_t[:], in0=pss[:], scalar=INV_SQRT_HD,
                                  in1=masks_sb[:, sidx, :],
                                  op0=ALU.mult, op1=ALU.add)
                              pt = ptq.tile([128, 2 * TB], f32r, tag="pt")
                              nc.scalar.activation(out=pt[:], in_=sm_t[:], func=AF.Exp)
                              for ss in range(2):
                                  nc.tensor.matmul(
                                      psy[:], V_sb[:, 2 * i + ss, :],
                                      pt[:, TB * ss:TB * ss + TB],
                                      start=(i == 0 and ss == 0),
                                      stop=(i == nblk - 1 and ss == 1))
                          # l2 norm (cancels softmax denominator) + gate
                          ysq = smp.tile([128, TB], f32, tag="ysq")
                          nc.scalar.activation(out=ysq[:], in_=psy[:], func=AF.Square)
                          psn = psn_p.tile([1, TB], f32, tag="psn")
                          nc.tensor.matmul(psn[:], ones_f[:], ysq[:],
                                           start=True, stop=True)
                          nln = smp.tile([1, TB], f32, tag="nln")
                          nc.scalar.activation(out=nln[:], in_=psn[:], func=AF.Ln)
                          ri2 = smp.tile([1, TB], f32, tag="ri2")
                          nc.scalar.activation(out=ri2[:], in_=nln[:], func=AF.Exp,
                                               scale=-0.5)
                          psb = psn_p.tile([128, TB], f32, tag="psb")
                          nc.tensor.matmul(psb[:], ones_rf[:], ri2[:],
                                           start=True, stop=True)
                          gsl = gT_sb[:, h, TB * s01:TB * s01 + TB]
                          tmp = smp.tile([128, TB], f32, tag="ytmp")
                          nc.vector.tensor_tensor(out=tmp[:], in0=psy[:], in1=gsl,
                                                  op=ALU.mult)
                          nc.vector.tensor_tensor(out=gsl, in0=tmp[:], in1=psb[:],
                                                  op=ALU.mult)
                          # round the gated output for the PE (out-proj lhsT)
                          nc.scalar.copy(out=gTr_sb[:, h, TB * s01:TB * s01 + TB],
                                         in_=gsl)
              for d in kload:
                  tile.add_dep_helper(d.ins, ag_k.ins, reason="allgather before load")

        # ==================================================== phase 4: out proj
        if "4" in phases:
          with tc.tile_pool(name="p4w", bufs=3) as wp, \
               tc.tile_pool(name="p4o", bufs=1) as op_, \
               tc.tile_pool(name="p4ps", bufs=1, space="PSUM") as pso_p, \
               tc.tile_pool(name="p4sm", bufs=2) as smp:
              out_sb = [op_.tile([128, D], f32, tag=f"o{m}", name=f"o{m}") for m in range(4)]
              for n in range(4):
                  pso = [pso_p.tile([128, 512], f32, tag=f"po{m}", name=f"po{m}") for m in range(4)]
                  for k in range(16):
                      wot = wp.tile([128, 512], f32r, tag="wo")
                      nc.sync.dma_start(out=wot[:],
                                        in_=Wo[128 * k:128 * k + 128, 512 * n:512 * n + 512])
                      for m in range(4):
                          nc.tensor.matmul(pso[m][:],
                                           gTr_sb[:, k, 128 * m:128 * m + 128],
                                           wot[:], start=(k == 0), stop=(k == 15))
                  for m in range(4):
                      nc.scalar.copy(out=out_sb[m][:, 512 * n:512 * n + 512],
                                     in_=pso[m][:])
              for m in range(4):
                  sq2 = smp.tile([128, D], f32, tag="osq")
                  nc.vector.tensor_tensor(out=sq2[:], in0=out_sb[m][:],
                                          in1=out_sb[m][:], op=ALU.mult)
                  s2 = smp.tile([128, 1], f32, tag="os2")
                  nc.vector.tensor_reduce(out=s2[:], in_=sq2[:],
                                          axis=mybir.AxisListType.X, op=ALU.add)
                  l2 = smp.tile([128, 1], f32, tag="oln")
                  nc.scalar.activation(out=l2[:], in_=s2[:], func=AF.Ln,
                                       bias=oeps_t[:],
                                       scale=float(OUT_SCALE) / D)
                  r2 = smp.tile([128, 1], f32, tag="ori")
                  nc.scalar.activation(out=r2[:], in_=l2[:], func=AF.Exp, scale=-0.5)
                  nc.vector.tensor_scalar_mul(out_sb[m][:], out_sb[m][:], r2[:])
                  nc.sync.dma_start(out=out_y[128 * m:128 * m + 128, :],
                                    in_=out_sb[m][:])

    nc.compile()
    _CACHE["nc"] = nc
    return nc


def _host_inputs(xq, xk, xv, Wq, Wk, Wv, Wg, Wo, mix_k, mix_v):
    """Build the 8 per-core input maps."""
    import ml_dtypes
    f = np.float32
    bf = ml_dtypes.bfloat16
    xq = np.asarray(xq, f)
    xk = np.asarray(xk, f)
    xv = np.asarray(xv, f)
    Wq = np.ascontiguousarray(np.asarray(Wq, f))
    Wk = np.asarray(Wk, f)
    Wv = np.asarray(Wv, f)
    Wg = np.ascontiguousarray(np.asarray(Wg, f))
    Wo = np.ascontiguousarray(np.asarray(Wo, f))
    mix_k = np.asarray(mix_k, f)
    mix_v = np.asarray(mix_v, f)

    Wk1 = np.ascontiguousarray((1.0 - mix_k)[:, None] * Wk)
    Wk2 = np.ascontiguousarray(mix_k[:, None] * Wk)
    Wv1 = np.ascontiguousarray((1.0 - mix_v)[:, None] * Wv)
    Wv2 = np.ascontiguousarray(mix_v[:, None] * Wv)

    half = HD // 2
    inv_freq = 1.0 / (10000.0 ** (np.arange(half, dtype=np.float64) / half))
    ang = np.arange(T, dtype=np.float64)[:, None] * inv_freq[None, :]
    cos_t = np.concatenate([np.cos(ang), np.cos(ang)], axis=-1).astype(f)
    sin_t = np.concatenate([np.sin(ang), np.sin(ang)], axis=-1).astype(f)

    # additive pre-exp masks, layout [tk_within_subtile, (ss, tq)]:
    # pt subtile ss holds tk rows 128*ss..128*ss+127; valid iff tk <= tq.
    ii = np.arange(128)[:, None]
    jj = np.arange(TB)[None, :]
    diag_mask = np.zeros((128, 2, TB), f)
    for ss in range(2):
        diag_mask[:, ss, :] = np.where(128 * ss + ii <= jj, 0.0, MASK_NEG)
    diag_mask = diag_mask.reshape(128, 2 * TB)
    ones_m = np.zeros((128, 2 * TB), f)           # additive: 0 = pass
    zeros_m = np.full((128, 2 * TB), MASK_NEG, f)  # additive: -inf = drop

    in_maps = []
    for c in range(NCORE):
        b, p = divmod(c, 4)
        jq0, jq1 = p, NBLK - 1 - p
        rows_q = np.concatenate([np.arange(TB * jq0, TB * jq0 + TB),
                                 np.arange(TB * jq1, TB * jq1 + TB)])
        t0 = KVTOK * p
        rows_kv = np.arange(t0, t0 + KVTOK)

        xq_s = np.ascontiguousarray(xq[b, rows_q, :])
        xk_s = np.zeros((KVTOK + 128, D), f)
        xv_s = np.zeros((KVTOK + 128, D), f)
        xk_s[128:] = xk[b, t0:t0 + KVTOK, :]
        xv_s[128:] = xv[b, t0:t0 + KVTOK, :]
        if p > 0:
            xk_s[127] = xk[b, t0 - 1, :]
            xv_s[127] = xv[b, t0 - 1, :]

        mask = np.empty((12, 128, 2 * TB), f)
        for i in range(4):
            mask[i] = diag_mask if i == jq0 else (ones_m if i < jq0 else zeros_m)
        for i in range(NBLK):
            mask[4 + i] = diag_mask if i == jq1 else (ones_m if i < jq1 else zeros_m)

        in_maps.append({
            "xq_sh": xq_s, "xk_sh": xk_s, "xv_sh": xv_s,
            "Wq": Wq, "Wg": Wg, "Wo": Wo,
            "Wk1": Wk1, "Wk2": Wk2, "Wv1": Wv1, "Wv2": Wv2,
            "cos_q": np.ascontiguousarray(cos_t[rows_q]),
            "sin_q": np.ascontiguousarray(sin_t[rows_q]),
            "cos_k": np.ascontiguousarray(cos_t[rows_kv]),
            "sin_k": np.ascontiguousarray(sin_t[rows_kv]),
            "mask_all": mask.astype(bf),
        })
    return in_maps


def _run(in_maps, trace=False, tmpdir=None):
    _install_ntff_hook()
    from concourse.bass_utils import run_bass_kernel_spmd
    nc = _build()
    return run_bass_kernel_spmd(nc, in_maps, list(range(NCORE)),
                                trace=trace, tmpdir=tmpdir)


def kernel(xq, xk, xv, Wq, Wk, Wv, Wg, Wo, mix_k, mix_v,
           _trace=False, _tmpdir=None):
    in_maps = _host_inputs(xq, xk, xv, Wq, Wk, Wv, Wg, Wo, mix_k, mix_v)
    res = _run(in_maps, trace=_trace, tmpdir=_tmpdir)
    out = np.empty((B, T, D), np.float32)
    for c in range(NCORE):
        b, p = divmod(c, 4)
        jq0, jq1 = p, NBLK - 1 - p
        y = res.results[c]["out_y"]
        out[b, TB * jq0:TB * jq0 + TB, :] = y[:TB]
        out[b, TB * jq1:TB * jq1 + TB, :] = y[TB:]
    kernel._last_exec_ns = res.exec_time_ns
    return out



# revision 13
# speedup vs baseline: 1.3598x; 1.3598x over previous
"""Trainium2 Bass kernel for nn_AttentionSubLayer (dense transformer attention
sublayer with time-lerp K/V mixing, QK-norm, RoPE, GQA, per-head l2 output
norm, gating, out-proj + final RMS norm).

Sharding: 8 cores = 2 batch groups x 4-way sequence parallel.  Core c
handles batch c//4; within the group (p = c%4) it owns q slots
slot0 = tokens [256p, 256p+256) and slot1 = [1024+256p, 1024+256p+256),
so slot0 only ever attends to kv tokens < 1024 and slot1 to all 2048.
K/V projections are computed on the owning quarter [512p, 512p+512) and
AllGathered (bf16) within each 4-core batch group.  Out-proj and final
RMS norm are local.

Numerics: bf16 matmul operands everywhere (fp32 PSUM), fp32 vector math
for the norms/rope.  Weights are pre-tiled on the host into contiguous
[128, n] k-chunk blocks so every weight DMA is one fat transfer.
Softmax skips max-subtraction (scores bounded by sqrt(HD) after QK
rms-norm) and the denominator (cancelled by the per-head l2 norm).
Causal masking is a 0/1 bf16 multiply on the exp output; the scalar
engine runs Exp only in attention (the l2-norm rsqrt is one batched
Ln+Exp at the end).
"""

import math
import sys
import types
from contextlib import ExitStack

sys.path.insert(0, "/opt/trn_rl_repo")

import numpy as np

# ---------------------------------------------------------------- problem dims
B, T, D, H, KVH, HD = 2, 2048, 2048, 16, 4, 128
N_LAYER = 24
EPS = 1e-8
NCORE = 8
QTOK = 512        # q tokens per core (2 slots x 256)
KVTOK = 512       # kv tokens per core (contiguous quarter)
NCH = 16          # kv chunks of 128 tokens (full 2048)
INV_SQRT_HD = 1.0 / math.sqrt(HD)
OUT_SCALE = 2 * N_LAYER  # final rms divided by sqrt(2*N_LAYER)


def _install_ntff_hook():
    try:
        import antenv
        if "antenv.axon_hooks" in sys.modules:
            return
        from trn_agent_boot.trn_boot import _ntff_profile_via_ctypes
        hook = _ntff_profile_via_ctypes("/opt/axon/libaxon_pjrt.so")
        mod = types.ModuleType("antenv.axon_hooks")
        mod.get_axon_ntff_profile_hook = lambda: hook
        antenv.axon_hooks = mod
        sys.modules["antenv.axon_hooks"] = mod
    except Exception:
        pass


_CACHE = {}


def _build():
    if "nc" in _CACHE:
        return _CACHE["nc"]

    import concourse.bass as bass
    import concourse.mybir as mybir
    import concourse.tile as tile
    from concourse import bacc
    from concourse.masks import make_identity

    f32 = mybir.dt.float32
    bf16 = mybir.dt.bfloat16
    AF = mybir.ActivationFunctionType
    ALU = mybir.AluOpType

    def bc_free(ap, n, at):
        """Insert a broadcast (stride-0) free dim of size n at position `at`
        of the AP's dim list (position counted incl. partition dim 0)."""
        new = list(list(d) for d in ap.ap)
        new.insert(at, [0, n])
        return bass.AP(tensor=ap.tensor, offset=ap.offset, ap=new)

    nc = bacc.Bacc("TRN2", target_bir_lowering=False, debug=False,
                   num_devices=NCORE)

    # ------------------------------------------------------------- I/O tensors
    xq_sh = nc.dram_tensor("xq_sh", [QTOK, D], bf16, kind="ExternalInput")
    xk_sh = nc.dram_tensor("xk_sh", [KVTOK, D], bf16, kind="ExternalInput")
    xv_sh = nc.dram_tensor("xv_sh", [KVTOK, D], bf16, kind="ExternalInput")
    xkb = nc.dram_tensor("xkb", [1, D], bf16, kind="ExternalInput")
    xvb = nc.dram_tensor("xvb", [1, D], bf16, kind="ExternalInput")
    # pre-tiled weights (host layout, all bf16):
    #  Wkv_t[k] = [128, 512 Wk1 | 512 Wk2 | 512 Wv1 | 512 Wv2]
    Wkv_t = nc.dram_tensor("Wkv_t", [16, 128, 2048], bf16, kind="ExternalInput")
    #  Wqg_t[n2][k] = [128, 1024] cols n2*1024.. of [Wq | Wg]
    Wqg_t = nc.dram_tensor("Wqg_t", [4, 16, 128, 1024], bf16,
                           kind="ExternalInput")
    #  Wo_t[n][k] = [128, 512] rows 128k.., cols 512n.. of Wo
    Wo_t = nc.dram_tensor("Wo_t", [4, 16, 128, 512], bf16,
                          kind="ExternalInput")
    cos_q = nc.dram_tensor("cos_q", [QTOK, HD], f32, kind="ExternalInput")
    sin_q = nc.dram_tensor("sin_q", [QTOK, HD], f32, kind="ExternalInput")
    cos_k = nc.dram_tensor("cos_k", [KVTOK, HD], f32, kind="ExternalInput")
    sin_k = nc.dram_tensor("sin_k", [KVTOK, HD], f32, kind="ExternalInput")
    # maskS[c] = 0/1 validity for kv chunk rows vs 256 in-slot q cols
    maskS = nc.dram_tensor("maskS", [8, 128, 256], bf16, kind="ExternalInput")
    out_y = nc.dram_tensor("out_y", [QTOK, D], f32, kind="ExternalOutput")

    # staging for K/V allgather (within 4-core batch group)
    SHARD = KVH * HD * KVTOK
    kv_loc = nc.dram_tensor("kv_loc", [2, SHARD], bf16)
    kv_gath = nc.dram_tensor("kv_gath", [4, 2, SHARD], bf16)
    k_loc_v = kv_loc[0].rearrange("(kv hd t) -> kv hd t", kv=KVH, hd=HD)
    v_loc_v = kv_loc[1].rearrange("(kv t hd) -> kv t hd", kv=KVH, hd=HD)

    with tile.TileContext(nc) as tc, ExitStack() as es:
        # ------------------------------------------------------------ constants
        cpool = es.enter_context(tc.tile_pool(name="consts", bufs=1))
        ident = cpool.tile([128, 128], bf16)
        make_identity(nc, ident[:])
        ones_mat = cpool.tile([128, 128], bf16)
        nc.vector.memset(ones_mat[:], 1.0)
        eps_t = cpool.tile([128, 1], f32)
        nc.vector.memset(eps_t[:], EPS)
        oeps_t = cpool.tile([128, 1], f32)
        nc.vector.memset(oeps_t[:], float(OUT_SCALE) * EPS)
        cosq_sb = cpool.tile([128, 4, HD], f32)
        sinq_sb = cpool.tile([128, 4, HD], f32)
        cosk_sb = cpool.tile([128, 4, HD], f32)
        sink_sb = cpool.tile([128, 4, HD], f32)
        for m in range(4):
            nc.sync.dma_start(out=cosq_sb[:, m, :], in_=cos_q[128 * m:128 * m + 128, :])
            nc.sync.dma_start(out=sinq_sb[:, m, :], in_=sin_q[128 * m:128 * m + 128, :])
            nc.sync.dma_start(out=cosk_sb[:, m, :], in_=cos_k[128 * m:128 * m + 128, :])
            nc.sync.dma_start(out=sink_sb[:, m, :], in_=sin_k[128 * m:128 * m + 128, :])

        # ============================================================ helpers
        def ev(i):
            return nc.scalar if i % 2 == 0 else nc.vector

        def evac(engine, out, in_):
            if engine is nc.scalar:
                engine.copy(out=out, in_=in_)
            else:
                engine.tensor_copy(out=out, in_=in_)

        def transpose_in(x_dram, xT, col0, natp, ptp):
            """Load natural [512, D] bf16 DRAM -> xT[:, k, col0+...] transposed."""
            for m in range(4):
                nat = natp.tile([128, D], bf16, tag="nat")
                nc.sync.dma_start(out=nat[:], in_=x_dram[128 * m:128 * m + 128, :])
                for k in range(16):
                    pst = ptp.tile([128, 128], bf16, tag="pst")
                    nc.tensor.transpose(pst[:], nat[:, 128 * k:128 * k + 128], ident[:])
                    evac(ev(k), xT[:, k, col0 + 128 * m:col0 + 128 * m + 128], pst[:])

        def rms_batch(x_ap, nh, smp, out_bf=None):
            """x *= rsqrt(mean(x^2 over HD) + EPS), batched over nh heads.
            x_ap [128, nh*HD] f32 AP; optionally write result to out_bf (bf16)."""
            x3 = x_ap.rearrange("p (h d) -> p h d", h=nh)
            sq = smp.tile([128, nh * HD], f32, tag="rsq")
            nc.vector.tensor_tensor(out=sq[:], in0=x_ap, in1=x_ap, op=ALU.mult)
            s2 = smp.tile([128, nh], f32, tag="rs2")
            nc.vector.tensor_reduce(out=s2[:],
                                    in_=sq[:].rearrange("p (h d) -> p h d", h=nh),
                                    axis=mybir.AxisListType.X, op=ALU.add)
            ln = smp.tile([128, nh], f32, tag="rln")
            nc.scalar.activation(out=ln[:], in_=s2[:], func=AF.Ln,
                                 bias=eps_t[:], scale=1.0 / HD)
            ri = smp.tile([128, nh], f32, tag="rri")
            nc.scalar.activation(out=ri[:], in_=ln[:], func=AF.Exp, scale=-0.5)
            dst = (out_bf.rearrange("p (h d) -> p h d", h=nh)
                   if out_bf is not None else x3)
            nc.vector.tensor_tensor(out=dst, in0=x3, in1=bc_free(ri[:], HD, 2),
                                    op=ALU.mult)

        def rope_batch(dst_bf, src, nh, cos_sb, sin_sb, m, smp):
            """dst = rope(src) for nh heads at once; dst bf16 AP, src f32 AP."""
            half = HD // 2
            cos_bc = bc_free(cos_sb[:, m, :], nh, 1)          # [128, nh, HD]
            sinlo_bc = bc_free(sin_sb[:, m, 0:half], nh, 1)   # [128, nh, half]
            sinhi_bc = bc_free(sin_sb[:, m, half:HD], nh, 1)
            s3 = src.rearrange("p (h d) -> p h d", h=nh)
            d3 = dst_bf.rearrange("p (h d) -> p h d", h=nh)
            t0 = smp.tile([128, nh * HD], f32, tag="ro0")
            t03 = t0[:].rearrange("p (h d) -> p h d", h=nh)
            nc.vector.tensor_tensor(out=t03, in0=s3, in1=cos_bc, op=ALU.mult)
            t1 = smp.tile([128, nh * half], f32, tag="ro1")
            t13 = t1[:].rearrange("p (h d) -> p h d", h=nh)
            nc.vector.tensor_tensor(out=t13, in0=s3[:, :, half:HD], in1=sinlo_bc,
                                    op=ALU.mult)
            nc.vector.tensor_tensor(out=d3[:, :, 0:half], in0=t03[:, :, 0:half],
                                    in1=t13, op=ALU.subtract)
            nc.vector.tensor_tensor(out=t13, in0=s3[:, :, 0:half], in1=sinhi_bc,
                                    op=ALU.mult)
            nc.vector.tensor_tensor(out=d3[:, :, half:HD], in0=t03[:, :, half:HD],
                                    in1=t13, op=ALU.add)

        # ===================================================== phase 1: K / V
        stage_dmas = []
        with tc.tile_pool(name="p1kvn", bufs=1) as kvnat:
            knat = [kvnat.tile([128, KVH * HD], f32, name=f"kn{m}")
                     for m in range(4)]
            vnat = [kvnat.tile([128, KVH * HD], f32, name=f"vn{m}") for m in range(4)]
            with tc.tile_pool(name="p1x", bufs=1) as p1x:
                xkT = p1x.tile([128, 16, KVTOK + 1], bf16, name="xkT")
                xvT = p1x.tile([128, 16, KVTOK + 1], bf16, name="xvT")
                with tc.tile_pool(name="p1nat", bufs=2) as natp, \
                     tc.tile_pool(name="p1pst", bufs=4, space="PSUM") as ptp:
                    # boundary token -> free position 0 of each k-chunk
                    nc.sync.dma_start(out=xkT[:, :, 0],
                                      in_=xkb[0].rearrange("(k p) -> p k", p=128))
                    nc.sync.dma_start(out=xvT[:, :, 0],
                                      in_=xvb[0].rearrange("(k p) -> p k", p=128))
                    transpose_in(xk_sh, xkT, 1, natp, ptp)
                    transpose_in(xv_sh, xvT, 1, natp, ptp)

                with tc.tile_pool(name="p1w", bufs=3) as wp, \
                     tc.tile_pool(name="p1ps", bufs=1, space="PSUM") as pskv:
                    psK = [pskv.tile([128, 512], f32, tag=f"pK{m}", name=f"pK{m}") for m in range(4)]
                    psV = [pskv.tile([128, 512], f32, tag=f"pV{m}", name=f"pV{m}") for m in range(4)]
                    for k in range(16):
                        wt = wp.tile([128, 2048], bf16, tag="wkv")
                        nc.sync.dma_start(out=wt[:], in_=Wkv_t[k])
                        for m in range(4):
                            n0, n1 = 1 + 128 * m, 129 + 128 * m   # normal tokens
                            s0, s1 = 128 * m, 128 * m + 128       # shifted (t-1)
                            nc.tensor.matmul(psK[m][:], xkT[:, k, n0:n1],
                                             wt[:, 0:512], start=(k == 0), stop=False)
                            nc.tensor.matmul(psK[m][:], xkT[:, k, s0:s1],
                                             wt[:, 512:1024], start=False,
                                             stop=(k == 15))
                            nc.tensor.matmul(psV[m][:], xvT[:, k, n0:n1],
                                             wt[:, 1024:1536], start=(k == 0),
                                             stop=False)
                            nc.tensor.matmul(psV[m][:], xvT[:, k, s0:s1],
                                             wt[:, 1536:2048], start=False,
                                             stop=(k == 15))
                    for m in range(4):
                        evac(ev(m), knat[m][:], psK[m][:])
                        evac(ev(m + 1), vnat[m][:], psV[m][:])

            with tc.tile_pool(name="p1pst2", bufs=2, space="PSUM") as ptp2, \
                 tc.tile_pool(name="p1sm", bufs=2) as smp, \
                 tc.tile_pool(name="p1st", bufs=3) as stp:
                for m in range(4):
                    # V: rms -> bf16, stage natural [kv, t, hd]
                    vout = stp.tile([128, KVH * HD], bf16, tag="vout")
                    rms_batch(vnat[m][:], KVH, smp, out_bf=vout[:])
                    for kv in range(KVH):
                        d = nc.sync.dma_start(
                            out=v_loc_v[kv, 128 * m:128 * m + 128, :],
                            in_=vout[:, 128 * kv:128 * kv + 128])
                        stage_dmas.append(d)
                    # K: rms, rope -> bf16, transpose, stage [kv, hd, t]
                    rms_batch(knat[m][:], KVH, smp)
                    krot = stp.tile([128, KVH * HD], bf16, tag="krot")
                    rope_batch(krot[:], knat[m][:], KVH, cosk_sb, sink_sb, m, smp)
                    for kv in range(KVH):
                        pst = ptp2.tile([128, 128], bf16, tag="pst")
                        nc.tensor.transpose(pst[:], krot[:, 128 * kv:128 * kv + 128],
                                            ident[:])
                        kst = stp.tile([128, 128], bf16, tag="kst")
                        evac(ev(kv), kst[:], pst[:])
                        d = nc.sync.dma_start(
                            out=k_loc_v[kv, :, 128 * m:128 * m + 128], in_=kst[:])
                        stage_dmas.append(d)

        ag_k = nc.gpsimd.collective_compute(
            "AllGather", ALU.bypass,
            replica_groups=[[0, 1, 2, 3], [4, 5, 6, 7]],
            ins=[kv_loc[:]], outs=[kv_gath[:]])
        for d in stage_dmas:
            tile.add_dep_helper(ag_k.ins, d.ins, reason="stage before allgather")

        # ===================================================== phase 2: Q / G
        p_qT = es.enter_context(tc.tile_pool(name="ppqT", bufs=1))
        qT_sb = p_qT.tile([128, H, QTOK], bf16, name="qT_sb")
        p_gT = es.enter_context(tc.tile_pool(name="ppgT", bufs=1))
        gT_sb = p_gT.tile([128, H, QTOK], bf16, name="gT_sb")
        with tc.tile_pool(name="p2qgn", bufs=1) as qgnat:
            q_sb = [qgnat.tile([128, H * HD], bf16, name=f"q{m}") for m in range(4)]
            g_sb = [qgnat.tile([128, H * HD], bf16, name=f"g{m}") for m in range(4)]
            with tc.tile_pool(name="p2x", bufs=1) as p2x:
                xqT = p2x.tile([128, 16, QTOK], bf16, name="xqT")
                with tc.tile_pool(name="p2nat", bufs=2) as natp, \
                     tc.tile_pool(name="p2pst", bufs=4, space="PSUM") as ptp:
                    transpose_in(xq_sh, xqT, 0, natp, ptp)

                with tc.tile_pool(name="p2w", bufs=3) as wp, \
                     tc.tile_pool(name="p2ps", bufs=1, space="PSUM") as psqg:
                    for n2 in range(4):
                        ps = [psqg.tile([128, 512], f32, tag=f"pqg{i}",
                                     name=f"pqg{i}") for i in range(8)]
                        for k in range(16):
                            wt = wp.tile([128, 1024], bf16, tag="wqg")
                            nc.sync.dma_start(out=wt[:], in_=Wqg_t[n2, k])
                            for m in range(4):
                                nc.tensor.matmul(ps[2 * m][:],
                                                 xqT[:, k, 128 * m:128 * m + 128],
                                                 wt[:, 0:512], start=(k == 0),
                                                 stop=(k == 15))
                                nc.tensor.matmul(ps[2 * m + 1][:],
                                                 xqT[:, k, 128 * m:128 * m + 128],
                                                 wt[:, 512:1024], start=(k == 0),
                                                 stop=(k == 15))
                        for m in range(4):
                            t = q_sb[m] if n2 < 2 else g_sb[m]
                            c0 = 1024 * (n2 % 2)
                            evac(ev(m), t[:, c0:c0 + 512], ps[2 * m][:])
                            evac(ev(m + 1), t[:, c0 + 512:c0 + 1024], ps[2 * m + 1][:])

            # rms + rope + transpose q; transpose g
            with tc.tile_pool(name="p2pst2", bufs=4, space="PSUM") as ptp2, \
                 tc.tile_pool(name="p2sm", bufs=2) as smp:
                for m in range(4):
                    qf = smp.tile([128, H * HD], f32, tag="qf")
                    nc.vector.tensor_copy(out=qf[:], in_=q_sb[m][:])
                    rms_batch(qf[:], H, smp)
                    qrot = smp.tile([128, H * HD], bf16, tag="qrot")
                    rope_batch(qrot[:], qf[:], H, cosq_sb, sinq_sb, m, smp)
                    for h in range(H):
                        pst = ptp2.tile([128, 128], bf16, tag="pst")
                        nc.tensor.transpose(pst[:], qrot[:, 128 * h:128 * h + 128],
                                            ident[:])
                        evac(ev(h), qT_sb[:, h, 128 * m:128 * m + 128], pst[:])
                    for h in range(H):
                        pst = ptp2.tile([128, 128], bf16, tag="pst")
                        nc.tensor.transpose(pst[:], g_sb[m][:, 128 * h:128 * h + 128],
                                            ident[:])
                        evac(ev(h + 1), gT_sb[:, h, 128 * m:128 * m + 128], pst[:])

        # ==================================================== phase 3: attention
        p_y = es.enter_context(tc.tile_pool(name="ppy", bufs=1))
        y_sb = p_y.tile([128, H, QTOK], bf16, name="y_sb")
        p_n = es.enter_context(tc.tile_pool(name="ppn", bufs=1))
        rbf_all = p_n.tile([128, H, QTOK], bf16, name="rbf_all")
        masks_sb = p_n.tile([128, 8, 256], bf16, name="masks_sb")
        for c in range(8):
            nc.sync.dma_start(out=masks_sb[:, c, :], in_=maskS[c])

        kload = []
        with tc.tile_pool(name="p3kv", bufs=2) as kvp, \
             tc.tile_pool(name="p3pt", bufs=6) as ptq, \
             tc.tile_pool(name="p3ps", bufs=1, space="PSUM") as pss_p, \
             tc.tile_pool(name="p3py", bufs=1, space="PSUM") as psy_p, \
             tc.tile_pool(name="p3sm", bufs=4) as smp, \
             tc.tile_pool(name="p3nf", bufs=1) as nfp:
            norms_full = nfp.tile([128, H, QTOK], f32, name="norms_full")
            for kv in range(KVH):
                K_sb = kvp.tile([128, 4, 512], bf16, tag="K")
                V_sb = kvp.tile([128, NCH, 128], bf16, tag="V")
                for g in range(4):
                    kg = kv_gath[g, 0].rearrange("(kv hd t) -> kv hd t",
                                                 kv=KVH, hd=HD)
                    vg = kv_gath[g, 1].rearrange("(kv t hd) -> kv t hd",
                                                 kv=KVH, hd=HD)
                    d = nc.sync.dma_start(out=K_sb[:, g, :], in_=kg[kv])
                    kload.append(d)
                    d = nc.sync.dma_start(
                        out=V_sb[:, 4 * g:4 * g + 4, :],
                        in_=vg[kv].rearrange("(c p) hd -> p c hd", p=128))
                    kload.append(d)
                psy = [psy_p.tile([128, 512], f32, tag=f"psy{hi}",
                                   name=f"psy{hi}") for hi in range(4)]
                for c in range(NCH):
                    q0, n = (0, 512) if c < 8 else (256, 256)
                    Kc = K_sb[:, c // 4, 128 * (c % 4):128 * (c % 4) + 128]
                    pts = []
                    for hi in range(4):
                        h = 4 * kv + hi
                        pss = pss_p.tile([128, 512], f32, tag=f"pss{hi}")
                        nc.tensor.matmul(pss[:, q0:q0 + n], Kc,
                                         qT_sb[:, h, q0:q0 + n],
                                         start=True, stop=True)
                        pt = ptq.tile([128, 512], bf16, tag="pt")
                        nc.scalar.activation(out=pt[:, q0:q0 + n],
                                             in_=pss[:, q0:q0 + n],
                                             func=AF.Exp, scale=INV_SQRT_HD)
                        mcol = 0 if c < 8 else 256
                        eng = nc.vector if hi % 2 == 0 else nc.gpsimd
                        eng.tensor_tensor(out=pt[:, mcol:mcol + 256],
                                          in0=pt[:, mcol:mcol + 256],
                                          in1=masks_sb[:, c % 8, :],
                                          op=ALU.mult)
                        pts.append(pt)
                    for hi in range(4):
                        nc.tensor.matmul(psy[hi][:, q0:q0 + n], V_sb[:, c, :],
                                         pts[hi][:, q0:q0 + n],
                                         start=(c == 0), stop=(c == NCH - 1),
                                         skip_group_check=True)
                # evacuate y, collect squared norms
                for hi in range(4):
                    h = 4 * kv + hi
                    nc.vector.tensor_copy(out=y_sb[:, h, :], in_=psy[hi][:])
                    ysq = smp.tile([128, 512], bf16, tag="ysq")
                    nc.gpsimd.tensor_tensor(out=ysq[:], in0=y_sb[:, h, :],
                                            in1=y_sb[:, h, :], op=ALU.mult)
                    psn = pss_p.tile([128, 512], f32, tag=f"pss{hi}")
                    nc.tensor.matmul(psn[:], ones_mat[:], ysq[:],
                                     start=True, stop=True)
                    evac(ev(hi), norms_full[:, h, :], psn[:])
            # batched rsqrt of all norms (one Ln + one Exp, 128 lanes)
            nf_flat = norms_full[:].rearrange("p h q -> p (h q)")
            nc.scalar.activation(out=nf_flat, in_=nf_flat, func=AF.Ln)
            nc.scalar.activation(out=rbf_all[:].rearrange("p h q -> p (h q)"),
                                 in_=nf_flat, func=AF.Exp, scale=-0.5)

        # gating: gTr = y * g * rsqrt(norm)  (bf16 for out-proj lhsT)
        p_gTr = es.enter_context(tc.tile_pool(name="ppgTr", bufs=1))
        gTr_sb = p_gTr.tile([128, H, QTOK], bf16, name="gTr_sb")
        with tc.tile_pool(name="p3gs", bufs=4) as gsp:
            for h in range(H):
                tmp = gsp.tile([128, 512], bf16, tag="gtmp")
                nc.vector.tensor_tensor(out=tmp[:], in0=y_sb[:, h, :],
                                        in1=gT_sb[:, h, :], op=ALU.mult)
                nc.gpsimd.tensor_tensor(out=gTr_sb[:, h, :], in0=tmp[:],
                                        in1=rbf_all[:, h, :], op=ALU.mult)
        for d in kload:
            tile.add_dep_helper(d.ins, ag_k.ins, reason="allgather before load")

        # ==================================================== phase 4: out proj
        with tc.tile_pool(name="p4w", bufs=12) as wp, \
             tc.tile_pool(name="p4o", bufs=1) as op_, \
             tc.tile_pool(name="p4ps", bufs=1, space="PSUM") as pso_p, \
             tc.tile_pool(name="p4sm", bufs=2) as smp:
            out_sb = [op_.tile([128, D], f32, name=f"o{m}") for m in range(4)]
            for n in range(4):
                pso = [pso_p.tile([128, 512], f32, tag=f"po{m}", name=f"po{m}") for m in range(4)]
                for k in range(16):
                    wot = wp.tile([128, 512], bf16, tag="wo")
                    nc.sync.dma_start(out=wot[:], in_=Wo_t[n, k])
                    for m in range(4):
                        nc.tensor.matmul(pso[m][:],
                                         gTr_sb[:, k, 128 * m:128 * m + 128],
                                         wot[:], start=(k == 0), stop=(k == 15))
                for m in range(4):
                    evac(ev(m + n), out_sb[m][:, 512 * n:512 * n + 512], pso[m][:])
            for m in range(4):
                sq2 = smp.tile([128, D], f32, tag="osq")
                nc.vector.tensor_tensor(out=sq2[:], in0=out_sb[m][:],
                                        in1=out_sb[m][:], op=ALU.mult)
                s2 = smp.tile([128, 1], f32, tag="os2")
                nc.vector.tensor_reduce(out=s2[:], in_=sq2[:],
                                        axis=mybir.AxisListType.X, op=ALU.add)
                l2 = smp.tile([128, 1], f32, tag="oln")
                nc.scalar.activation(out=l2[:], in_=s2[:], func=AF.Ln,
                                     bias=oeps_t[:],
                                     scale=float(OUT_SCALE) / D)
                r2 = smp.tile([128, 1], f32, tag="ori")
                nc.scalar.activation(out=r2[:], in_=l2[:], func=AF.Exp, scale=-0.5)
                nc.vector.tensor_scalar_mul(out_sb[m][:], out_sb[m][:], r2[:])
                nc.sync.dma_start(out=out_y[128 * m:128 * m + 128, :],
                                  in_=out_sb[m][:])

    nc.compile()
    _CACHE["nc"] = nc
    return nc


def _host_inputs(xq, xk, xv, Wq, Wk, Wv, Wg, Wo, mix_k, mix_v):
    """Build the 8 per-core input maps."""
    import ml_dtypes
    f = np.float32
    bf = ml_dtypes.bfloat16
    xq = np.asarray(xq, f)
    xk = np.asarray(xk, f)
    xv = np.asarray(xv, f)
    Wq = np.asarray(Wq, f)
    Wk = np.asarray(Wk, f)
    Wv = np.asarray(Wv, f)
    Wg = np.asarray(Wg, f)
    Wo = np.asarray(Wo, f)
    mix_k = np.asarray(mix_k, f)
    mix_v = np.asarray(mix_v, f)

    Wk1 = (1.0 - mix_k)[:, None] * Wk
    Wk2 = mix_k[:, None] * Wk
    Wv1 = (1.0 - mix_v)[:, None] * Wv
    Wv2 = mix_v[:, None] * Wv

    # Wkv_t[k] = [128, Wk1|Wk2|Wv1|Wv2]
    Wkv_t = np.empty((16, 128, 2048), f)
    for k in range(16):
        r = slice(128 * k, 128 * k + 128)
        Wkv_t[k, :, 0:512] = Wk1[r]
        Wkv_t[k, :, 512:1024] = Wk2[r]
        Wkv_t[k, :, 1024:1536] = Wv1[r]
        Wkv_t[k, :, 1536:2048] = Wv2[r]
    Wkv_t = np.ascontiguousarray(Wkv_t.astype(bf))

    Wqg = np.concatenate([Wq, Wg], axis=1)  # [2048, 4096]
    Wqg_t = np.empty((4, 16, 128, 1024), f)
    for n2 in range(4):
        for k in range(16):
            Wqg_t[n2, k] = Wqg[128 * k:128 * k + 128, 1024 * n2:1024 * n2 + 1024]
    Wqg_t = np.ascontiguousarray(Wqg_t.astype(bf))

    Wo_t = np.empty((4, 16, 128, 512), f)
    for n in range(4):
        for k in range(16):
            Wo_t[n, k] = Wo[128 * k:128 * k + 128, 512 * n:512 * n + 512]
    Wo_t = np.ascontiguousarray(Wo_t.astype(bf))

    half = HD // 2
    inv_freq = 1.0 / (10000.0 ** (np.arange(half, dtype=np.float64) / half))
    ang = np.arange(T, dtype=np.float64)[:, None] * inv_freq[None, :]
    cos_t = np.concatenate([np.cos(ang), np.cos(ang)], axis=-1).astype(f)
    sin_t = np.concatenate([np.sin(ang), np.sin(ang)], axis=-1).astype(f)

    in_maps = []
    for c in range(NCORE):
        b, p = divmod(c, 4)
        rows_q = np.concatenate([np.arange(256 * p, 256 * p + 256),
                                 np.arange(1024 + 256 * p, 1024 + 256 * p + 256)])
        t0 = KVTOK * p
        rows_kv = np.arange(t0, t0 + KVTOK)

        xq_s = np.ascontiguousarray(xq[b, rows_q, :].astype(bf))
        xk_s = np.ascontiguousarray(xk[b, t0:t0 + KVTOK, :].astype(bf))
        xv_s = np.ascontiguousarray(xv[b, t0:t0 + KVTOK, :].astype(bf))
        xkb = np.zeros((1, D), f)
        xvb = np.zeros((1, D), f)
        if p > 0:
            xkb[0] = xk[b, t0 - 1, :]
            xvb[0] = xv[b, t0 - 1, :]

        # maskS[cc][i][j] = 1 iff kv token 128cc+i <= in-slot q token 256p+j
        ii = np.arange(128)[:, None]
        jj = np.arange(256)[None, :]
        mask = np.empty((8, 128, 256), f)
        for cc in range(8):
            mask[cc] = (128 * cc + ii <= 256 * p + jj).astype(f)

        in_maps.append({
            "xq_sh": xq_s, "xk_sh": xk_s, "xv_sh": xv_s,
            "xkb": xkb.astype(bf), "xvb": xvb.astype(bf),
            "Wkv_t": Wkv_t, "Wqg_t": Wqg_t, "Wo_t": Wo_t,
            "cos_q": np.ascontiguousarray(cos_t[rows_q]),
            "sin_q": np.ascontiguousarray(sin_t[rows_q]),
            "cos_k": np.ascontiguousarray(cos_t[rows_kv]),
            "sin_k": np.ascontiguousarray(sin_t[rows_kv]),
            "maskS": np.ascontiguousarray(mask.astype(bf)),
        })
    return in_maps


def _run(in_maps, trace=False, tmpdir=None):
    _install_ntff_hook()
    from concourse.bass_utils import run_bass_kernel_spmd
    nc = _build()
    return run_bass_kernel_spmd(nc, in_maps, list(range(NCORE)),
                                trace=trace, tmpdir=tmpdir)


def kernel(xq, xk, xv, Wq, Wk, Wv, Wg, Wo, mix_k, mix_v,
           _trace=False, _tmpdir=None):
    in_maps = _host_inputs(xq, xk, xv, Wq, Wk, Wv, Wg, Wo, mix_k, mix_v)
    res = _run(in_maps, trace=_trace, tmpdir=_tmpdir)
    out = np.empty((B, T, D), np.float32)
    for c in range(NCORE):
        b, p = divmod(c, 4)
        y = res.results[c]["out_y"]
        out[b, 256 * p:256 * p + 256, :] = y[:256]
        out[b, 1024 + 256 * p:1024 + 256 * p + 256, :] = y[256:]
    kernel._last_exec_ns = res.exec_time_ns
    return out


# revision 19
# speedup vs baseline: 1.4338x; 1.0544x over previous
"""Trainium2 Bass kernel for nn_AttentionSubLayer (dense transformer attention
sublayer with time-lerp K/V mixing, QK-norm, RoPE, GQA, per-head l2 output
norm, gating, out-proj + final RMS norm).

Sharding: 8 cores = 2 batch groups x 4-way sequence parallel.  Core c
handles batch c//4; within the group (p = c%4) it owns q slots
slot0 = tokens [256p, 256p+256) and slot1 = [1024+256p, 1024+256p+256),
so slot0 only ever attends to kv tokens < 1024 and slot1 to all 2048.
K/V projections are computed on the owning quarter [512p, 512p+512) and
AllGathered (bf16) within each 4-core batch group.  Out-proj and final
RMS norm are local.

Numerics: bf16 matmul operands everywhere (fp32 PSUM), fp32 vector math
for the norms/rope.  Weights are pre-tiled on the host into contiguous
[128, n] k-chunk blocks so every weight DMA is one fat transfer.
Softmax skips max-subtraction (scores bounded by sqrt(HD) after QK
rms-norm) and the denominator (cancelled by the per-head l2 norm).
Causal masking is a 0/1 bf16 multiply on the exp output; the scalar
engine runs Exp only in attention (the l2-norm rsqrt is one batched
Ln+Exp at the end).
"""

import math
import sys
import types
from contextlib import ExitStack

sys.path.insert(0, "/opt/trn_rl_repo")

import numpy as np

# ---------------------------------------------------------------- problem dims
B, T, D, H, KVH, HD = 2, 2048, 2048, 16, 4, 128
N_LAYER = 24
EPS = 1e-8
NCORE = 8
QTOK = 512        # q tokens per core (2 slots x 256)
KVTOK = 512       # kv tokens per core (contiguous quarter)
NCH = 16          # kv chunks of 128 tokens (full 2048)
INV_SQRT_HD = 1.0 / math.sqrt(HD)
OUT_SCALE = 2 * N_LAYER  # final rms divided by sqrt(2*N_LAYER)


def _install_ntff_hook():
    try:
        import antenv
        if "antenv.axon_hooks" in sys.modules:
            return
        from trn_agent_boot.trn_boot import _ntff_profile_via_ctypes
        hook = _ntff_profile_via_ctypes("/opt/axon/libaxon_pjrt.so")
        mod = types.ModuleType("antenv.axon_hooks")
        mod.get_axon_ntff_profile_hook = lambda: hook
        antenv.axon_hooks = mod
        sys.modules["antenv.axon_hooks"] = mod
    except Exception:
        pass


_CACHE = {}


def _build():
    if "nc" in _CACHE:
        return _CACHE["nc"]

    import concourse.bass as bass
    import concourse.mybir as mybir
    import concourse.tile as tile
    from concourse import bacc
    from concourse.masks import make_identity

    f32 = mybir.dt.float32
    bf16 = mybir.dt.bfloat16
    AF = mybir.ActivationFunctionType
    ALU = mybir.AluOpType

    def bc_free(ap, n, at):
        """Insert a broadcast (stride-0) free dim of size n at position `at`
        of the AP's dim list (position counted incl. partition dim 0)."""
        new = list(list(d) for d in ap.ap)
        new.insert(at, [0, n])
        return bass.AP(tensor=ap.tensor, offset=ap.offset, ap=new)

    nc = bacc.Bacc("TRN2", target_bir_lowering=False, debug=False,
                   num_devices=NCORE)

    # ------------------------------------------------------------- I/O tensors
    xq_sh = nc.dram_tensor("xq_sh", [QTOK, D], bf16, kind="ExternalInput")
    xk_sh = nc.dram_tensor("xk_sh", [KVTOK, D], bf16, kind="ExternalInput")
    xv_sh = nc.dram_tensor("xv_sh", [KVTOK, D], bf16, kind="ExternalInput")
    xkb = nc.dram_tensor("xkb", [1, D], bf16, kind="ExternalInput")
    xvb = nc.dram_tensor("xvb", [1, D], bf16, kind="ExternalInput")
    # pre-tiled weights (host layout, all bf16):
    #  Wkv_t[k] = [128, 512 Wk1 | 512 Wk2 | 512 Wv1 | 512 Wv2]
    Wkv_t = nc.dram_tensor("Wkv_t", [16, 128, 2048], bf16, kind="ExternalInput")
    #  Wqg_t[n2][k] = [128, 1024] cols n2*1024.. of [Wq | Wg]
    Wqg_t = nc.dram_tensor("Wqg_t", [4, 16, 128, 1024], bf16,
                           kind="ExternalInput")
    #  Wo_t[n][k] = [128, 512] rows 128k.., cols 512n.. of Wo
    Wo_t = nc.dram_tensor("Wo_t", [4, 16, 128, 512], bf16,
                          kind="ExternalInput")
    cos_q = nc.dram_tensor("cos_q", [QTOK, HD], f32, kind="ExternalInput")
    sin_q = nc.dram_tensor("sin_q", [QTOK, HD], f32, kind="ExternalInput")
    cos_k = nc.dram_tensor("cos_k", [KVTOK, HD], f32, kind="ExternalInput")
    sin_k = nc.dram_tensor("sin_k", [KVTOK, HD], f32, kind="ExternalInput")
    # maskS[c] = 0/1 validity for kv chunk rows vs 256 in-slot q cols
    maskS = nc.dram_tensor("maskS", [8, 128, 256], bf16, kind="ExternalInput")
    out_y = nc.dram_tensor("out_y", [QTOK, D], f32, kind="ExternalOutput")

    # staging for K/V allgather (within 4-core batch group)
    SHARD = KVH * HD * KVTOK
    kv_loc = nc.dram_tensor("kv_loc", [2, SHARD], bf16)
    kv_gath = nc.dram_tensor("kv_gath", [4, 2, SHARD], bf16)
    k_loc_v = kv_loc[0].rearrange("(kv hd t) -> kv hd t", kv=KVH, hd=HD)
    v_loc_v = kv_loc[1].rearrange("(kv t hd) -> kv t hd", kv=KVH, hd=HD)

    with tile.TileContext(nc) as tc, ExitStack() as es:
        # ------------------------------------------------------------ constants
        cpool = es.enter_context(tc.tile_pool(name="consts", bufs=1))
        ident = cpool.tile([128, 128], bf16)
        make_identity(nc, ident[:])
        ones_mat = cpool.tile([128, 128], bf16)
        nc.vector.memset(ones_mat[:], 1.0)
        eps_t = cpool.tile([128, 1], f32)
        nc.vector.memset(eps_t[:], EPS)
        oeps_t = cpool.tile([128, 1], f32)
        nc.vector.memset(oeps_t[:], float(OUT_SCALE) * EPS)
        cosq_sb = cpool.tile([128, 4, HD], f32)
        sinq_sb = cpool.tile([128, 4, HD], f32)
        cosk_sb = cpool.tile([128, 4, HD], f32)
        sink_sb = cpool.tile([128, 4, HD], f32)
        for m in range(4):
            nc.sync.dma_start(out=cosq_sb[:, m, :], in_=cos_q[128 * m:128 * m + 128, :])
            nc.sync.dma_start(out=sinq_sb[:, m, :], in_=sin_q[128 * m:128 * m + 128, :])
            nc.sync.dma_start(out=cosk_sb[:, m, :], in_=cos_k[128 * m:128 * m + 128, :])
            nc.sync.dma_start(out=sink_sb[:, m, :], in_=sin_k[128 * m:128 * m + 128, :])

        # ============================================================ helpers
        def ev(i):
            return nc.scalar if i % 2 == 0 else nc.vector

        def evac(engine, out, in_):
            if engine is nc.scalar:
                engine.copy(out=out, in_=in_)
            else:
                engine.tensor_copy(out=out, in_=in_)

        def transpose_in(x_dram, xT, col0, natp, ptp, eng=None):
            """Load natural [512, D] bf16 DRAM -> xT[:, k, col0+...] transposed."""
            for m in range(4):
                nat = natp.tile([128, D], bf16, tag="nat")
                nc.sync.dma_start(out=nat[:], in_=x_dram[128 * m:128 * m + 128, :])
                for k in range(16):
                    pst = ptp.tile([128, 128], bf16, tag="pst")
                    nc.tensor.transpose(pst[:], nat[:, 128 * k:128 * k + 128], ident[:])
                    evac(eng or ev(k), xT[:, k, col0 + 128 * m:col0 + 128 * m + 128],
                         pst[:])

        def rms_batch(x_ap, nh, smp, out_bf=None):
            """x *= rsqrt(mean(x^2 over HD) + EPS), batched over nh heads.
            x_ap [128, nh*HD] f32 AP; optionally write result to out_bf (bf16)."""
            x3 = x_ap.rearrange("p (h d) -> p h d", h=nh)
            sq = smp.tile([128, nh * HD], f32, tag="rsq")
            nc.vector.tensor_tensor(out=sq[:], in0=x_ap, in1=x_ap, op=ALU.mult)
            s2 = smp.tile([128, nh], f32, tag="rs2")
            nc.vector.tensor_reduce(out=s2[:],
                                    in_=sq[:].rearrange("p (h d) -> p h d", h=nh),
                                    axis=mybir.AxisListType.X, op=ALU.add)
            ln = smp.tile([128, nh], f32, tag="rln")
            nc.scalar.activation(out=ln[:], in_=s2[:], func=AF.Ln,
                                 bias=eps_t[:], scale=1.0 / HD)
            ri = smp.tile([128, nh], f32, tag="rri")
            nc.scalar.activation(out=ri[:], in_=ln[:], func=AF.Exp, scale=-0.5)
            dst = (out_bf.rearrange("p (h d) -> p h d", h=nh)
                   if out_bf is not None else x3)
            nc.vector.tensor_tensor(out=dst, in0=x3, in1=bc_free(ri[:], HD, 2),
                                    op=ALU.mult)

        def rope_batch(dst_bf, src, nh, cos_sb, sin_sb, m, smp):
            """dst = rope(src) for nh heads at once; dst bf16 AP, src f32 AP."""
            half = HD // 2
            cos_bc = bc_free(cos_sb[:, m, :], nh, 1)          # [128, nh, HD]
            sinlo_bc = bc_free(sin_sb[:, m, 0:half], nh, 1)   # [128, nh, half]
            sinhi_bc = bc_free(sin_sb[:, m, half:HD], nh, 1)
            s3 = src.rearrange("p (h d) -> p h d", h=nh)
            d3 = dst_bf.rearrange("p (h d) -> p h d", h=nh)
            t0 = smp.tile([128, nh * HD], f32, tag="ro0")
            t03 = t0[:].rearrange("p (h d) -> p h d", h=nh)
            nc.vector.tensor_tensor(out=t03, in0=s3, in1=cos_bc, op=ALU.mult)
            t1 = smp.tile([128, nh * half], f32, tag="ro1")
            t13 = t1[:].rearrange("p (h d) -> p h d", h=nh)
            nc.vector.tensor_tensor(out=t13, in0=s3[:, :, half:HD], in1=sinlo_bc,
                                    op=ALU.mult)
            nc.vector.tensor_tensor(out=d3[:, :, 0:half], in0=t03[:, :, 0:half],
                                    in1=t13, op=ALU.subtract)
            nc.vector.tensor_tensor(out=t13, in0=s3[:, :, 0:half], in1=sinhi_bc,
                                    op=ALU.mult)
            nc.vector.tensor_tensor(out=d3[:, :, half:HD], in0=t03[:, :, half:HD],
                                    in1=t13, op=ALU.add)

        # ===================================================== phase 1: K / V
        stage_dmas = []
        p2x = es.enter_context(tc.tile_pool(name="p2x", bufs=1))
        xqT = p2x.tile([128, 16, QTOK], bf16, name="xqT")
        with tc.tile_pool(name="p1kvn", bufs=1) as kvnat:
            knat = [kvnat.tile([128, KVH * HD], f32, name=f"kn{m}")
                     for m in range(4)]
            vnat = [kvnat.tile([128, KVH * HD], f32, name=f"vn{m}") for m in range(4)]
            with tc.tile_pool(name="p1x", bufs=1) as p1x:
                xkT = p1x.tile([128, 16, KVTOK + 1], bf16, name="xkT")
                xvT = p1x.tile([128, 16, KVTOK + 1], bf16, name="xvT")
                with tc.tile_pool(name="p1nat", bufs=2) as natp, \
                     tc.tile_pool(name="p1pst", bufs=4, space="PSUM") as ptp:
                    # boundary token -> free position 0 of each k-chunk
                    nc.sync.dma_start(out=xkT[:, :, 0],
                                      in_=xkb[0].rearrange("(k p) -> p k", p=128))
                    nc.sync.dma_start(out=xvT[:, :, 0],
                                      in_=xvb[0].rearrange("(k p) -> p k", p=128))
                    transpose_in(xk_sh, xkT, 1, natp, ptp)
                    transpose_in(xv_sh, xvT, 1, natp, ptp)

                with tc.tile_pool(name="p1w", bufs=3) as wp, \
                     tc.tile_pool(name="p1ps", bufs=1, space="PSUM") as pskv:
                    psK = [pskv.tile([128, 512], f32, tag=f"pK{m}", name=f"pK{m}") for m in range(4)]
                    psV = [pskv.tile([128, 512], f32, tag=f"pV{m}", name=f"pV{m}") for m in range(4)]
                    for k in range(16):
                        wt = wp.tile([128, 2048], bf16, tag="wkv")
                        nc.sync.dma_start(out=wt[:], in_=Wkv_t[k])
                        for m in range(4):
                            n0, n1 = 1 + 128 * m, 129 + 128 * m   # normal tokens
                            s0, s1 = 128 * m, 128 * m + 128       # shifted (t-1)
                            nc.tensor.matmul(psK[m][:], xkT[:, k, n0:n1],
                                             wt[:, 0:512], start=(k == 0), stop=False)
                            nc.tensor.matmul(psK[m][:], xkT[:, k, s0:s1],
                                             wt[:, 512:1024], start=False,
                                             stop=(k == 15))
                            nc.tensor.matmul(psV[m][:], xvT[:, k, n0:n1],
                                             wt[:, 1024:1536], start=(k == 0),
                                             stop=False)
                            nc.tensor.matmul(psV[m][:], xvT[:, k, s0:s1],
                                             wt[:, 1536:2048], start=False,
                                             stop=(k == 15))
                    for m in range(4):
                        evac(ev(m), knat[m][:], psK[m][:])
                        evac(ev(m + 1), vnat[m][:], psV[m][:])

            # xq transposes here: tensor fills the p1-tail gap while the
            # vector engine does the K/V rms/rope below (evacs on scalar)
            with tc.tile_pool(name="p2nat", bufs=2) as natp, \
                 tc.tile_pool(name="p2pst", bufs=4, space="PSUM") as ptp:
                transpose_in(xq_sh, xqT, 0, natp, ptp, eng=nc.scalar)

            with tc.tile_pool(name="p1pst2", bufs=2, space="PSUM") as ptp2, \
                 tc.tile_pool(name="p1sm", bufs=2) as smp, \
                 tc.tile_pool(name="p1st", bufs=3) as stp:
                for m in range(4):
                    # V: rms -> bf16, stage natural [kv, t, hd]
                    vout = stp.tile([128, KVH * HD], bf16, tag="vout")
                    rms_batch(vnat[m][:], KVH, smp, out_bf=vout[:])
                    for kv in range(KVH):
                        d = nc.sync.dma_start(
                            out=v_loc_v[kv, 128 * m:128 * m + 128, :],
                            in_=vout[:, 128 * kv:128 * kv + 128])
                        stage_dmas.append(d)
                    # K: rms, rope -> bf16, transpose, stage [kv, hd, t]
                    rms_batch(knat[m][:], KVH, smp)
                    krot = stp.tile([128, KVH * HD], bf16, tag="krot")
                    rope_batch(krot[:], knat[m][:], KVH, cosk_sb, sink_sb, m, smp)
                    for kv in range(KVH):
                        pst = ptp2.tile([128, 128], bf16, tag="pst")
                        nc.tensor.transpose(pst[:], krot[:, 128 * kv:128 * kv + 128],
                                            ident[:])
                        kst = stp.tile([128, 128], bf16, tag="kst")
                        evac(ev(kv), kst[:], pst[:])
                        d = nc.sync.dma_start(
                            out=k_loc_v[kv, :, 128 * m:128 * m + 128], in_=kst[:])
                        stage_dmas.append(d)

        ag_k = nc.gpsimd.collective_compute(
            "AllGather", ALU.bypass,
            replica_groups=[[0, 1, 2, 3], [4, 5, 6, 7]],
            ins=[kv_loc[:]], outs=[kv_gath[:]])
        for d in stage_dmas:
            tile.add_dep_helper(ag_k.ins, d.ins, reason="stage before allgather")

        # ===================================================== phase 2: Q / G
        p_qT = es.enter_context(tc.tile_pool(name="ppqT", bufs=1))
        qT_sb = p_qT.tile([128, H, QTOK], bf16, name="qT_sb")
        p_gT = es.enter_context(tc.tile_pool(name="ppgT", bufs=1))
        gT_sb = p_gT.tile([128, H, QTOK], bf16, name="gT_sb")
        p_gn = es.enter_context(tc.tile_pool(name="ppgn", bufs=1))
        g_sb = [p_gn.tile([128, H * HD], bf16, name=f"g{m}") for m in range(4)]
        with tc.tile_pool(name="p2qn", bufs=1) as qnat, \
             tc.tile_pool(name="p2qr", bufs=1) as qrp:
            q_sb = [qnat.tile([128, H * HD], bf16, name=f"q{m}") for m in range(4)]
            qrot = [qrp.tile([128, H * HD], bf16, name=f"qr{m}") for m in range(4)]
            with tc.tile_pool(name="p2w", bufs=3) as wp, \
                 tc.tile_pool(name="p2ps", bufs=1, space="PSUM") as psqg, \
                 tc.tile_pool(name="p2sm", bufs=2) as smp:
                def qg_pass(n2, dsts):
                    ps = [psqg.tile([128, 512], f32, tag=f"pqg{i}",
                                    name=f"pqg{i}") for i in range(8)]
                    for k in range(16):
                        wt = wp.tile([128, 1024], bf16, tag="wqg")
                        nc.sync.dma_start(out=wt[:], in_=Wqg_t[n2, k])
                        for m in range(4):
                            nc.tensor.matmul(ps[2 * m][:],
                                             xqT[:, k, 128 * m:128 * m + 128],
                                             wt[:, 0:512], start=(k == 0),
                                             stop=(k == 15))
                            nc.tensor.matmul(ps[2 * m + 1][:],
                                             xqT[:, k, 128 * m:128 * m + 128],
                                             wt[:, 512:1024], start=(k == 0),
                                             stop=(k == 15))
                    for m in range(4):
                        c0 = 1024 * (n2 % 2)
                        t = dsts[m]
                        evac(ev(m), t[:, c0:c0 + 512], ps[2 * m][:])
                        evac(ev(m + 1), t[:, c0 + 512:c0 + 1024], ps[2 * m + 1][:])

                qg_pass(0, q_sb)
                qg_pass(1, q_sb)
                # q rms + rope on vector/scalar; hidden under the G matmuls
                for m in range(4):
                    qf = smp.tile([128, H * HD], f32, tag="qf")
                    nc.vector.tensor_copy(out=qf[:], in_=q_sb[m][:])
                    rms_batch(qf[:], H, smp)
                    rope_batch(qrot[m][:], qf[:], H, cosq_sb, sinq_sb, m, smp)
                qg_pass(2, g_sb)
                qg_pass(3, g_sb)

            # transpose q (g is transposed later, during the attention tail)
            with tc.tile_pool(name="p2pst2", bufs=4, space="PSUM") as ptp2:
                for m in range(4):
                    for h in range(H):
                        pst = ptp2.tile([128, 128], bf16, tag="pst")
                        nc.tensor.transpose(pst[:],
                                            qrot[m][:, 128 * h:128 * h + 128],
                                            ident[:])
                        evac(ev(h), qT_sb[:, h, 128 * m:128 * m + 128], pst[:])

        # ==================================================== phase 3: attention
        p_y = es.enter_context(tc.tile_pool(name="ppy", bufs=1))
        y_sb = p_y.tile([128, H, QTOK], bf16, name="y_sb")
        p_n = es.enter_context(tc.tile_pool(name="ppn", bufs=1))
        rbf_all = p_n.tile([128, H, QTOK], bf16, name="rbf_all")
        masks_sb = p_n.tile([128, 8, 256], bf16, name="masks_sb")
        for c in range(8):
            nc.sync.dma_start(out=masks_sb[:, c, :], in_=maskS[c])

        kload = []
        with tc.tile_pool(name="p3kv", bufs=2) as kvp, \
             tc.tile_pool(name="p3pt", bufs=6) as ptq, \
             tc.tile_pool(name="p3ps", bufs=1, space="PSUM") as pss_p, \
             tc.tile_pool(name="p3py", bufs=1, space="PSUM") as psy_p, \
             tc.tile_pool(name="p3sm", bufs=4) as smp, \
             tc.tile_pool(name="p3nf", bufs=1) as nfp:
            norms_full = nfp.tile([128, H, QTOK], f32, name="norms_full")
            for kv in range(KVH):
                K_sb = kvp.tile([128, 4, 512], bf16, tag="K")
                V_sb = kvp.tile([128, NCH, 128], bf16, tag="V")
                for g in range(4):
                    kg = kv_gath[g, 0].rearrange("(kv hd t) -> kv hd t",
                                                 kv=KVH, hd=HD)
                    vg = kv_gath[g, 1].rearrange("(kv t hd) -> kv t hd",
                                                 kv=KVH, hd=HD)
                    d = nc.sync.dma_start(out=K_sb[:, g, :], in_=kg[kv])
                    kload.append(d)
                    d = nc.sync.dma_start(
                        out=V_sb[:, 4 * g:4 * g + 4, :],
                        in_=vg[kv].rearrange("(c p) hd -> p c hd", p=128))
                    kload.append(d)
                psy = [psy_p.tile([128, 512], f32, tag=f"psy{hi}",
                                   name=f"psy{hi}") for hi in range(4)]
                for c in range(NCH):
                    q0, n = (0, 512) if c < 8 else (256, 256)
                    Kc = K_sb[:, c // 4, 128 * (c % 4):128 * (c % 4) + 128]
                    pts = []
                    for hi in range(4):
                        h = 4 * kv + hi
                        pss = pss_p.tile([128, 512], f32, tag=f"pss{hi}")
                        nc.tensor.matmul(pss[:, q0:q0 + n], Kc,
                                         qT_sb[:, h, q0:q0 + n],
                                         start=True, stop=True)
                        pt = ptq.tile([128, 512], bf16, tag="pt")
                        nc.scalar.activation(out=pt[:, q0:q0 + n],
                                             in_=pss[:, q0:q0 + n],
                                             func=AF.Exp, scale=INV_SQRT_HD)
                        mcol = 0 if c < 8 else 256
                        eng = nc.vector if hi % 2 == 0 else nc.gpsimd
                        eng.tensor_tensor(out=pt[:, mcol:mcol + 256],
                                          in0=pt[:, mcol:mcol + 256],
                                          in1=masks_sb[:, c % 8, :],
                                          op=ALU.mult)
                        pts.append(pt)
                    for hi in range(4):
                        nc.tensor.matmul(psy[hi][:, q0:q0 + n], V_sb[:, c, :],
                                         pts[hi][:, q0:q0 + n],
                                         start=(c == 0), stop=(c == NCH - 1),
                                         skip_group_check=True)
                # evacuate y, collect squared norms
                for hi in range(4):
                    h = 4 * kv + hi
                    nc.vector.tensor_copy(out=y_sb[:, h, :], in_=psy[hi][:])
                    ysq = smp.tile([128, 512], bf16, tag="ysq")
                    nc.gpsimd.tensor_tensor(out=ysq[:], in0=y_sb[:, h, :],
                                            in1=y_sb[:, h, :], op=ALU.mult)
                    psn = pss_p.tile([128, 512], f32, tag=f"pss{hi}")
                    nc.tensor.matmul(psn[:], ones_mat[:], ysq[:],
                                     start=True, stop=True)
                    evac(ev(hi), norms_full[:, h, :], psn[:])
            # batched rsqrt of all norms (one Ln + one Exp, 128 lanes)
            nf_flat = norms_full[:].rearrange("p h q -> p (h q)")
            nc.scalar.activation(out=nf_flat, in_=nf_flat, func=AF.Ln)
            nc.scalar.activation(out=rbf_all[:].rearrange("p h q -> p (h q)"),
                                 in_=nf_flat, func=AF.Exp, scale=-0.5)

        # transpose g (overlaps the attention tail; evacs off the scalar engine)
        with tc.tile_pool(name="p3gt", bufs=4, space="PSUM") as gtp:
            for m in range(4):
                for h in range(H):
                    pst = gtp.tile([128, 128], bf16, tag="pst")
                    nc.tensor.transpose(pst[:], g_sb[m][:, 128 * h:128 * h + 128],
                                        ident[:])
                    nc.vector.tensor_copy(out=gT_sb[:, h, 128 * m:128 * m + 128],
                                          in_=pst[:])

        # gating: gTr = y * g * rsqrt(norm)  (bf16 for out-proj lhsT)
        p_gTr = es.enter_context(tc.tile_pool(name="ppgTr", bufs=1))
        gTr_sb = p_gTr.tile([128, H, QTOK], bf16, name="gTr_sb")
        with tc.tile_pool(name="p3gs", bufs=4) as gsp:
            for h in range(H):
                tmp = gsp.tile([128, 512], bf16, tag="gtmp")
                nc.vector.tensor_tensor(out=tmp[:], in0=y_sb[:, h, :],
                                        in1=gT_sb[:, h, :], op=ALU.mult)
                nc.gpsimd.tensor_tensor(out=gTr_sb[:, h, :], in0=tmp[:],
                                        in1=rbf_all[:, h, :], op=ALU.mult)
        for d in kload:
            tile.add_dep_helper(d.ins, ag_k.ins, reason="allgather before load")

        # ==================================================== phase 4: out proj
        with tc.tile_pool(name="p4w", bufs=12) as wp, \
             tc.tile_pool(name="p4o", bufs=1) as op_, \
             tc.tile_pool(name="p4ps", bufs=1, space="PSUM") as pso_p, \
             tc.tile_pool(name="p4sm", bufs=2) as smp:
            out_sb = [op_.tile([128, D], f32, name=f"o{m}") for m in range(4)]
            for n in range(4):
                pso = [pso_p.tile([128, 512], f32, tag=f"po{m}", name=f"po{m}") for m in range(4)]
                for k in range(16):
                    wot = wp.tile([128, 512], bf16, tag="wo")
                    nc.sync.dma_start(out=wot[:], in_=Wo_t[n, k])
                    for m in range(4):
                        nc.tensor.matmul(pso[m][:],
                                         gTr_sb[:, k, 128 * m:128 * m + 128],
                                         wot[:], start=(k == 0), stop=(k == 15))
                for m in range(4):
                    evac(ev(m + n), out_sb[m][:, 512 * n:512 * n + 512], pso[m][:])
            for m in range(4):
                sq2 = smp.tile([128, D], f32, tag="osq")
                nc.vector.tensor_tensor(out=sq2[:], in0=out_sb[m][:],
                                        in1=out_sb[m][:], op=ALU.mult)
                s2 = smp.tile([128, 1], f32, tag="os2")
                nc.vector.tensor_reduce(out=s2[:], in_=sq2[:],
                                        axis=mybir.AxisListType.X, op=ALU.add)
                l2 = smp.tile([128, 1], f32, tag="oln")
                nc.scalar.activation(out=l2[:], in_=s2[:], func=AF.Ln,
                                     bias=oeps_t[:],
                                     scale=float(OUT_SCALE) / D)
                r2 = smp.tile([128, 1], f32, tag="ori")
                nc.scalar.activation(out=r2[:], in_=l2[:], func=AF.Exp, scale=-0.5)
                nc.vector.tensor_scalar_mul(out_sb[m][:], out_sb[m][:], r2[:])
                nc.sync.dma_start(out=out_y[128 * m:128 * m + 128, :],
                                  in_=out_sb[m][:])

    nc.compile()
    _CACHE["nc"] = nc
    return nc


def _host_inputs(xq, xk, xv, Wq, Wk, Wv, Wg, Wo, mix_k, mix_v):
    """Build the 8 per-core input maps."""
    import ml_dtypes
    f = np.float32
    bf = ml_dtypes.bfloat16
    xq = np.asarray(xq, f)
    xk = np.asarray(xk, f)
    xv = np.asarray(xv, f)
    Wq = np.asarray(Wq, f)
    Wk = np.asarray(Wk, f)
    Wv = np.asarray(Wv, f)
    Wg = np.asarray(Wg, f)
    Wo = np.asarray(Wo, f)
    mix_k = np.asarray(mix_k, f)
    mix_v = np.asarray(mix_v, f)

    Wk1 = (1.0 - mix_k)[:, None] * Wk
    Wk2 = mix_k[:, None] * Wk
    Wv1 = (1.0 - mix_v)[:, None] * Wv
    Wv2 = mix_v[:, None] * Wv

    # Wkv_t[k] = [128, Wk1|Wk2|Wv1|Wv2]
    Wkv_t = np.empty((16, 128, 2048), f)
    for k in range(16):
        r = slice(128 * k, 128 * k + 128)
        Wkv_t[k, :, 0:512] = Wk1[r]
        Wkv_t[k, :, 512:1024] = Wk2[r]
        Wkv_t[k, :, 1024:1536] = Wv1[r]
        Wkv_t[k, :, 1536:2048] = Wv2[r]
    Wkv_t = np.ascontiguousarray(Wkv_t.astype(bf))

    Wqg = np.concatenate([Wq, Wg], axis=1)  # [2048, 4096]
    Wqg_t = np.empty((4, 16, 128, 1024), f)
    for n2 in range(4):
        for k in range(16):
            Wqg_t[n2, k] = Wqg[128 * k:128 * k + 128, 1024 * n2:1024 * n2 + 1024]
    Wqg_t = np.ascontiguousarray(Wqg_t.astype(bf))

    Wo_t = np.empty((4, 16, 128, 512), f)
    for n in range(4):
        for k in range(16):
            Wo_t[n, k] = Wo[128 * k:128 * k + 128, 512 * n:512 * n + 512]
    Wo_t = np.ascontiguousarray(Wo_t.astype(bf))

    half = HD // 2
    inv_freq = 1.0 / (10000.0 ** (np.arange(half, dtype=np.float64) / half))
    ang = np.arange(T, dtype=np.float64)[:, None] * inv_freq[None, :]
    cos_t = np.concatenate([np.cos(ang), np.cos(ang)], axis=-1).astype(f)
    sin_t = np.concatenate([np.sin(ang), np.sin(ang)], axis=-1).astype(f)

    in_maps = []
    for c in range(NCORE):
        b, p = divmod(c, 4)
        rows_q = np.concatenate([np.arange(256 * p, 256 * p + 256),
                                 np.arange(1024 + 256 * p, 1024 + 256 * p + 256)])
        t0 = KVTOK * p
        rows_kv = np.arange(t0, t0 + KVTOK)

        xq_s = np.ascontiguousarray(xq[b, rows_q, :].astype(bf))
        xk_s = np.ascontiguousarray(xk[b, t0:t0 + KVTOK, :].astype(bf))
        xv_s = np.ascontiguousarray(xv[b, t0:t0 + KVTOK, :].astype(bf))
        xkb = np.zeros((1, D), f)
        xvb = np.zeros((1, D), f)
        if p > 0:
            xkb[0] = xk[b, t0 - 1, :]
            xvb[0] = xv[b, t0 - 1, :]

        # maskS[cc][i][j] = 1 iff kv token 128cc+i <= in-slot q token 256p+j
        ii = np.arange(128)[:, None]
        jj = np.arange(256)[None, :]
        mask = np.empty((8, 128, 256), f)
        for cc in range(8):
            mask[cc] = (128 * cc + ii <= 256 * p + jj).astype(f)

        in_maps.append({
            "xq_sh": xq_s, "xk_sh": xk_s, "xv_sh": xv_s,
            "xkb": xkb.astype(bf), "xvb": xvb.astype(bf),
            "Wkv_t": Wkv_t, "Wqg_t": Wqg_t, "Wo_t": Wo_t,
            "cos_q": np.ascontiguousarray(cos_t[rows_q]),
            "sin_q": np.ascontiguousarray(sin_t[rows_q]),
            "cos_k": np.ascontiguousarray(cos_t[rows_kv]),
            "sin_k": np.ascontiguousarray(sin_t[rows_kv]),
            "maskS": np.ascontiguousarray(mask.astype(bf)),
        })
    return in_maps


def _run(in_maps, trace=False, tmpdir=None):
    _install_ntff_hook()
    from concourse.bass_utils import run_bass_kernel_spmd
    nc = _build()
    return run_bass_kernel_spmd(nc, in_maps, list(range(NCORE)),
                                trace=trace, tmpdir=tmpdir)


def kernel(xq, xk, xv, Wq, Wk, Wv, Wg, Wo, mix_k, mix_v,
           _trace=False, _tmpdir=None):
    in_maps = _host_inputs(xq, xk, xv, Wq, Wk, Wv, Wg, Wo, mix_k, mix_v)
    res = _run(in_maps, trace=_trace, tmpdir=_tmpdir)
    out = np.empty((B, T, D), np.float32)
    for c in range(NCORE):
        b, p = divmod(c, 4)
        y = res.results[c]["out_y"]
        out[b, 256 * p:256 * p + 256, :] = y[:256]
        out[b, 1024 + 256 * p:1024 + 256 * p + 256, :] = y[256:]
    kernel._last_exec_ns = res.exec_time_ns
    return out


# revision 31
# speedup vs baseline: 1.4352x; 1.0010x over previous
"""Trainium2 Bass kernel for nn_AttentionSubLayer (dense transformer attention
sublayer with time-lerp K/V mixing, QK-norm, RoPE, GQA, per-head l2 output
norm, gating, out-proj + final RMS norm).

Sharding: 8 cores = 2 batch groups x 4-way sequence parallel.  Core c
handles batch c//4; within the group (p = c%4) it owns q slots
slot0 = tokens [256p, 256p+256) and slot1 = [1024+256p, 1024+256p+256),
so slot0 only ever attends to kv tokens < 1024 and slot1 to all 2048.
K/V projections are computed on the owning quarter [512p, 512p+512) and
AllGathered (bf16) within each 4-core batch group.  Out-proj and final
RMS norm are local.

Numerics: bf16 matmul operands everywhere (fp32 PSUM), fp32 vector math
for the norms/rope.  Weights are pre-tiled on the host into contiguous
[128, n] k-chunk blocks so every weight DMA is one fat transfer.
Softmax skips max-subtraction (scores bounded by sqrt(HD) after QK
rms-norm) and the denominator (cancelled by the per-head l2 norm).
Causal masking is a 0/1 bf16 multiply on the exp output; the scalar
engine runs Exp only in attention (the l2-norm rsqrt is one batched
Ln+Exp at the end).
"""

import math
import sys
import types
from contextlib import ExitStack

sys.path.insert(0, "/opt/trn_rl_repo")

import numpy as np

# ---------------------------------------------------------------- problem dims
B, T, D, H, KVH, HD = 2, 2048, 2048, 16, 4, 128
N_LAYER = 24
EPS = 1e-8
NCORE = 8
QTOK = 512        # q tokens per core (2 slots x 256)
KVTOK = 512       # kv tokens per core (contiguous quarter)
NCH = 16          # kv chunks of 128 tokens (full 2048)
INV_SQRT_HD = 1.0 / math.sqrt(HD)
OUT_SCALE = 2 * N_LAYER  # final rms divided by sqrt(2*N_LAYER)


def _install_ntff_hook():
    try:
        import antenv
        if "antenv.axon_hooks" in sys.modules:
            return
        from trn_agent_boot.trn_boot import _ntff_profile_via_ctypes
        hook = _ntff_profile_via_ctypes("/opt/axon/libaxon_pjrt.so")
        mod = types.ModuleType("antenv.axon_hooks")
        mod.get_axon_ntff_profile_hook = lambda: hook
        antenv.axon_hooks = mod
        sys.modules["antenv.axon_hooks"] = mod
    except Exception:
        pass


_CACHE = {}


def _build():
    if "nc" in _CACHE:
        return _CACHE["nc"]

    import concourse.bass as bass
    import concourse.mybir as mybir
    import concourse.tile as tile
    from concourse import bacc
    from concourse.masks import make_identity

    f32 = mybir.dt.float32
    bf16 = mybir.dt.bfloat16
    AF = mybir.ActivationFunctionType
    ALU = mybir.AluOpType

    def bc_free(ap, n, at):
        """Insert a broadcast (stride-0) free dim of size n at position `at`
        of the AP's dim list (position counted incl. partition dim 0)."""
        new = list(list(d) for d in ap.ap)
        new.insert(at, [0, n])
        return bass.AP(tensor=ap.tensor, offset=ap.offset, ap=new)

    nc = bacc.Bacc("TRN2", target_bir_lowering=False, debug=False,
                   num_devices=NCORE)

    # ------------------------------------------------------------- I/O tensors
    xq_sh = nc.dram_tensor("xq_sh", [QTOK, D], bf16, kind="ExternalInput")
    xk_sh = nc.dram_tensor("xk_sh", [KVTOK, D], bf16, kind="ExternalInput")
    xv_sh = nc.dram_tensor("xv_sh", [KVTOK, D], bf16, kind="ExternalInput")
    xkb = nc.dram_tensor("xkb", [1, D], bf16, kind="ExternalInput")
    xvb = nc.dram_tensor("xvb", [1, D], bf16, kind="ExternalInput")
    # pre-tiled weights (host layout, all bf16, >=4KB per partition per DMA):
    #  Wkv_t[k2][p][j] = row 256*k2+128*j+p of [Wk1 | Wk2 | Wv1 | Wv2]
    Wkv_t = nc.dram_tensor("Wkv_t", [8, 128, 2, 2048], bf16, kind="ExternalInput")
    #  Wqg_t[n2][k2][p][j] = row 256*k2+128*j+p, cols 1024*n2.. of [Wq | Wg]
    Wqg_t = nc.dram_tensor("Wqg_t", [4, 8, 128, 2, 1024], bf16,
                           kind="ExternalInput")
    #  Wo_t[n][k4][p][j] = row 512*k4+128*j+p, cols 512*n.. of Wo
    Wo_t = nc.dram_tensor("Wo_t", [4, 4, 128, 4, 512], bf16,
                          kind="ExternalInput")
    # partition-major rope tables: [p][m][hd] = table[rows[128*m+p]][hd]
    cos_q = nc.dram_tensor("cos_q", [128, 4, HD], f32, kind="ExternalInput")
    sin_q = nc.dram_tensor("sin_q", [128, 4, HD], f32, kind="ExternalInput")
    cos_k = nc.dram_tensor("cos_k", [128, 4, HD], f32, kind="ExternalInput")
    sin_k = nc.dram_tensor("sin_k", [128, 4, HD], f32, kind="ExternalInput")
    # maskS[p][c] = 0/1 validity row p of kv chunk c vs 256 in-slot q cols
    maskS = nc.dram_tensor("maskS", [128, 8, 256], bf16, kind="ExternalInput")
    out_y = nc.dram_tensor("out_y", [QTOK, D], f32, kind="ExternalOutput")

    # staging for K/V allgather (within 4-core batch group)
    SHARD = KVH * HD * KVTOK
    kv_loc = nc.dram_tensor("kv_loc", [2, SHARD], bf16)
    kv_gath = nc.dram_tensor("kv_gath", [4, 2, SHARD], bf16)
    k_loc_v = kv_loc[0].rearrange("(kv hd t) -> kv hd t", kv=KVH, hd=HD)
    v_loc_v = kv_loc[1].rearrange("(t kv hd) -> t kv hd", kv=KVH, hd=HD)

    with tile.TileContext(nc) as tc, ExitStack() as es:
        # ------------------------------------------------------------ constants
        cpool = es.enter_context(tc.tile_pool(name="consts", bufs=1))
        ident = cpool.tile([128, 128], bf16)
        make_identity(nc, ident[:])
        ones_mat = cpool.tile([128, 128], bf16)
        nc.vector.memset(ones_mat[:], 1.0)
        eps_t = cpool.tile([128, 1], f32)
        nc.vector.memset(eps_t[:], EPS)
        oeps_t = cpool.tile([128, 1], f32)
        nc.vector.memset(oeps_t[:], float(OUT_SCALE) * EPS)
        cosq_sb = cpool.tile([128, 4, HD], f32)
        sinq_sb = cpool.tile([128, 4, HD], f32)
        cosk_sb = cpool.tile([128, 4, HD], f32)
        sink_sb = cpool.tile([128, 4, HD], f32)
        nc.sync.dma_start(out=cosq_sb[:], in_=cos_q[:, :, :])
        nc.sync.dma_start(out=sinq_sb[:], in_=sin_q[:, :, :])
        nc.sync.dma_start(out=cosk_sb[:], in_=cos_k[:, :, :])
        nc.sync.dma_start(out=sink_sb[:], in_=sin_k[:, :, :])
        masks_sb = cpool.tile([128, 8, 256], bf16, name="masks_sb")
        nc.sync.dma_start(out=masks_sb[:], in_=maskS[:, :, :])

        # ============================================================ helpers
        def ev(i):
            return nc.scalar if i % 2 == 0 else nc.vector

        def evac(engine, out, in_):
            if engine is nc.scalar:
                engine.copy(out=out, in_=in_)
            else:
                engine.tensor_copy(out=out, in_=in_)

        def transpose_in(x_dram, xT, col0, natp, ptp, eng=None):
            """Load natural [512, D] bf16 DRAM -> xT[:, k, col0+...] transposed."""
            for m in range(4):
                nat = natp.tile([128, D], bf16, tag="nat")
                nc.sync.dma_start(out=nat[:], in_=x_dram[128 * m:128 * m + 128, :])
                for k in range(16):
                    pst = ptp.tile([128, 128], bf16, tag="pst")
                    nc.tensor.transpose(pst[:], nat[:, 128 * k:128 * k + 128], ident[:])
                    evac(eng or ev(k), xT[:, k, col0 + 128 * m:col0 + 128 * m + 128],
                         pst[:])

        def rms_batch(x_ap, nh, smp, out_bf=None):
            """x *= rsqrt(mean(x^2 over HD) + EPS), batched over nh heads.
            x_ap [128, nh*HD] f32 AP; optionally write result to out_bf (bf16)."""
            x3 = x_ap.rearrange("p (h d) -> p h d", h=nh)
            sq = smp.tile([128, nh * HD], f32, tag="rsq")
            nc.vector.tensor_tensor(out=sq[:], in0=x_ap, in1=x_ap, op=ALU.mult)
            s2 = smp.tile([128, nh], f32, tag="rs2")
            nc.vector.tensor_reduce(out=s2[:],
                                    in_=sq[:].rearrange("p (h d) -> p h d", h=nh),
                                    axis=mybir.AxisListType.X, op=ALU.add)
            ln = smp.tile([128, nh], f32, tag="rln")
            nc.scalar.activation(out=ln[:], in_=s2[:], func=AF.Ln,
                                 bias=eps_t[:], scale=1.0 / HD)
            ri = smp.tile([128, nh], f32, tag="rri")
            nc.scalar.activation(out=ri[:], in_=ln[:], func=AF.Exp, scale=-0.5)
            dst = (out_bf.rearrange("p (h d) -> p h d", h=nh)
                   if out_bf is not None else x3)
            nc.vector.tensor_tensor(out=dst, in0=x3, in1=bc_free(ri[:], HD, 2),
                                    op=ALU.mult)

        def rope_batch(dst_bf, src, nh, cos_sb, sin_sb, m, smp, eng=None):
            """dst = rope(src) for nh heads at once; dst bf16 AP, src f32 AP."""
            eng = eng or nc.vector
            half = HD // 2
            cos_bc = bc_free(cos_sb[:, m, :], nh, 1)          # [128, nh, HD]
            sinlo_bc = bc_free(sin_sb[:, m, 0:half], nh, 1)   # [128, nh, half]
            sinhi_bc = bc_free(sin_sb[:, m, half:HD], nh, 1)
            s3 = src.rearrange("p (h d) -> p h d", h=nh)
            d3 = dst_bf.rearrange("p (h d) -> p h d", h=nh)
            t0 = smp.tile([128, nh * HD], f32, tag="ro0")
            t03 = t0[:].rearrange("p (h d) -> p h d", h=nh)
            eng.tensor_tensor(out=t03, in0=s3, in1=cos_bc, op=ALU.mult)
            t1 = smp.tile([128, nh * half], f32, tag="ro1")
            t13 = t1[:].rearrange("p (h d) -> p h d", h=nh)
            eng.tensor_tensor(out=t13, in0=s3[:, :, half:HD], in1=sinlo_bc,
                              op=ALU.mult)
            eng.tensor_tensor(out=d3[:, :, 0:half], in0=t03[:, :, 0:half],
                              in1=t13, op=ALU.subtract)
            eng.tensor_tensor(out=t13, in0=s3[:, :, 0:half], in1=sinhi_bc,
                              op=ALU.mult)
            eng.tensor_tensor(out=d3[:, :, half:HD], in0=t03[:, :, half:HD],
                              in1=t13, op=ALU.add)

        # ===================================================== phase 1: K / V
        stage_dmas = []
        p2x = es.enter_context(tc.tile_pool(name="p2x", bufs=1))
        xqT = p2x.tile([128, 16, QTOK], bf16, name="xqT")
        with tc.tile_pool(name="p1kvn", bufs=1) as kvnat:
            knat = [kvnat.tile([128, KVH * HD], f32, name=f"kn{m}")
                     for m in range(4)]
            vnat = [kvnat.tile([128, KVH * HD], f32, name=f"vn{m}") for m in range(4)]
            with tc.tile_pool(name="p1x", bufs=1) as p1x:
                xkT = p1x.tile([128, 16, KVTOK + 1], bf16, name="xkT")
                xvT = p1x.tile([128, 16, KVTOK + 1], bf16, name="xvT")
                with tc.tile_pool(name="p1nat", bufs=2) as natp, \
                     tc.tile_pool(name="p1pst", bufs=4, space="PSUM") as ptp:
                    # boundary token -> free position 0 of each k-chunk
                    nc.sync.dma_start(out=xkT[:, :, 0],
                                      in_=xkb[0].rearrange("(k p) -> p k", p=128))
                    nc.sync.dma_start(out=xvT[:, :, 0],
                                      in_=xvb[0].rearrange("(k p) -> p k", p=128))
                    transpose_in(xk_sh, xkT, 1, natp, ptp)
                    transpose_in(xv_sh, xvT, 1, natp, ptp)

                with tc.tile_pool(name="p1w", bufs=3) as wp, \
                     tc.tile_pool(name="p1ps", bufs=1, space="PSUM") as pskv:
                    psK = [pskv.tile([128, 512], f32, tag=f"pK{m}", name=f"pK{m}") for m in range(4)]
                    psV = [pskv.tile([128, 512], f32, tag=f"pV{m}", name=f"pV{m}") for m in range(4)]
                    for k2 in range(8):
                        wt = wp.tile([128, 2, 2048], bf16, tag="wkv")
                        nc.sync.dma_start(out=wt[:], in_=Wkv_t[k2])
                        for j in range(2):
                            k = 2 * k2 + j
                            w = wt[:, j, :]
                            for m in range(4):
                                n0, n1 = 1 + 128 * m, 129 + 128 * m   # normal
                                s0, s1 = 128 * m, 128 * m + 128       # shifted
                                nc.tensor.matmul(psK[m][:], xkT[:, k, n0:n1],
                                                 w[:, 0:512], start=(k == 0),
                                                 stop=False)
                                nc.tensor.matmul(psK[m][:], xkT[:, k, s0:s1],
                                                 w[:, 512:1024], start=False,
                                                 stop=(k == 15))
                                nc.tensor.matmul(psV[m][:], xvT[:, k, n0:n1],
                                                 w[:, 1024:1536], start=(k == 0),
                                                 stop=False)
                                nc.tensor.matmul(psV[m][:], xvT[:, k, s0:s1],
                                                 w[:, 1536:2048], start=False,
                                                 stop=(k == 15))
                    for m in range(4):
                        evac(ev(m), knat[m][:], psK[m][:])
                        evac(ev(m + 1), vnat[m][:], psV[m][:])

            # xq transposes here: tensor fills the p1-tail gap while the
            # vector engine does the K/V rms/rope below (evacs on scalar)
            with tc.tile_pool(name="p2nat", bufs=2) as natp, \
                 tc.tile_pool(name="p2pst", bufs=4, space="PSUM") as ptp:
                transpose_in(xq_sh, xqT, 0, natp, ptp, eng=nc.scalar)

            with tc.tile_pool(name="p1pst2", bufs=2, space="PSUM") as ptp2, \
                 tc.tile_pool(name="p1sm", bufs=2) as smp, \
                 tc.tile_pool(name="p1st", bufs=3) as stp, \
                 tc.tile_pool(name="p1kt", bufs=1) as ktp:
                kT_full = ktp.tile([128, KVH, KVTOK], bf16, name="kT_full")
                for m in range(4):
                    # V: rms -> bf16, stage [t, kv, hd] (one fat DMA per m)
                    vout = stp.tile([128, KVH * HD], bf16, tag="vout")
                    rms_batch(vnat[m][:], KVH, smp, out_bf=vout[:])
                    d = nc.sync.dma_start(
                        out=v_loc_v[128 * m:128 * m + 128, :, :],
                        in_=vout[:].rearrange("p (kv hd) -> p kv hd", kv=KVH))
                    stage_dmas.append(d)
                    # K: rms (vector), rope (gpsimd) -> bf16, transpose
                    rms_batch(knat[m][:], KVH, smp)
                    krot = stp.tile([128, KVH * HD], bf16, tag="krot")
                    rope_batch(krot[:], knat[m][:], KVH, cosk_sb, sink_sb, m, smp,
                               eng=nc.gpsimd)
                    for kv in range(KVH):
                        pst = ptp2.tile([128, 128], bf16, tag="pst")
                        nc.tensor.transpose(pst[:], krot[:, 128 * kv:128 * kv + 128],
                                            ident[:])
                        evac(ev(kv), kT_full[:, kv, 128 * m:128 * m + 128], pst[:])
                for kv in range(KVH):
                    d = nc.sync.dma_start(out=k_loc_v[kv], in_=kT_full[:, kv, :])
                    stage_dmas.append(d)

        ag_k = nc.gpsimd.collective_compute(
            "AllGather", ALU.bypass,
            replica_groups=[[0, 1, 2, 3], [4, 5, 6, 7]],
            ins=[kv_loc[:]], outs=[kv_gath[:]])
        for d in stage_dmas:
            tile.add_dep_helper(ag_k.ins, d.ins, reason="stage before allgather")

        # ===================================================== phase 2: Q / G
        p_qT = es.enter_context(tc.tile_pool(name="ppqT", bufs=1))
        qT_sb = p_qT.tile([128, H, QTOK], bf16, name="qT_sb")
        p_gT = es.enter_context(tc.tile_pool(name="ppgT", bufs=1))
        gT_sb = p_gT.tile([128, H, QTOK], bf16, name="gT_sb")
        p_gn = es.enter_context(tc.tile_pool(name="ppgn", bufs=1))
        g_sb = [p_gn.tile([128, H * HD], bf16, name=f"g{m}") for m in range(4)]
        with tc.tile_pool(name="p2qn", bufs=1) as qnat, \
             tc.tile_pool(name="p2qr", bufs=1) as qrp:
            q_sb = [qnat.tile([128, H * HD], bf16, name=f"q{m}") for m in range(4)]
            qrot = [qrp.tile([128, H * HD], bf16, name=f"qr{m}") for m in range(4)]
            with tc.tile_pool(name="p2w", bufs=3) as wp, \
                 tc.tile_pool(name="p2ps", bufs=1, space="PSUM") as psqg, \
                 tc.tile_pool(name="p2sm", bufs=2) as smp:
                def qg_pass(n2, dsts):
                    ps = [psqg.tile([128, 512], f32, tag=f"pqg{i}",
                                    name=f"pqg{i}") for i in range(8)]
                    for k2 in range(8):
                        wt = wp.tile([128, 2, 1024], bf16, tag="wqg")
                        nc.sync.dma_start(out=wt[:], in_=Wqg_t[n2, k2])
                        for j in range(2):
                            k = 2 * k2 + j
                            for m in range(4):
                                nc.tensor.matmul(ps[2 * m][:],
                                                 xqT[:, k, 128 * m:128 * m + 128],
                                                 wt[:, j, 0:512], start=(k == 0),
                                                 stop=(k == 15))
                                nc.tensor.matmul(ps[2 * m + 1][:],
                                                 xqT[:, k, 128 * m:128 * m + 128],
                                                 wt[:, j, 512:1024], start=(k == 0),
                                                 stop=(k == 15))
                    for m in range(4):
                        c0 = 1024 * (n2 % 2)
                        t = dsts[m]
                        evac(ev(m), t[:, c0:c0 + 512], ps[2 * m][:])
                        evac(ev(m + 1), t[:, c0 + 512:c0 + 1024], ps[2 * m + 1][:])

                qg_pass(0, q_sb)
                qg_pass(1, q_sb)
                # q rms + rope on vector/scalar; hidden under the G matmuls
                for m in range(4):
                    qf = smp.tile([128, H * HD], f32, tag="qf")
                    nc.vector.tensor_copy(out=qf[:], in_=q_sb[m][:])
                    rms_batch(qf[:], H, smp)
                    rope_batch(qrot[m][:], qf[:], H, cosq_sb, sinq_sb, m, smp)
                qg_pass(2, g_sb)
                qg_pass(3, g_sb)

            # transpose q (g is transposed later, during the attention tail)
            with tc.tile_pool(name="p2pst2", bufs=4, space="PSUM") as ptp2:
                for m in range(4):
                    for h in range(H):
                        pst = ptp2.tile([128, 128], bf16, tag="pst")
                        nc.tensor.transpose(pst[:],
                                            qrot[m][:, 128 * h:128 * h + 128],
                                            ident[:])
                        evac(ev(h), qT_sb[:, h, 128 * m:128 * m + 128], pst[:])

        # ==================================================== phase 3: attention
        p_y = es.enter_context(tc.tile_pool(name="ppy", bufs=1))
        y_sb = p_y.tile([128, H, QTOK], bf16, name="y_sb")
        p_n = es.enter_context(tc.tile_pool(name="ppn", bufs=1))
        rbf_all = p_n.tile([128, H, QTOK], bf16, name="rbf_all")

        kload = []
        with tc.tile_pool(name="p3kv", bufs=2) as kvp, \
             tc.tile_pool(name="p3pt", bufs=6) as ptq, \
             tc.tile_pool(name="p3ps", bufs=1, space="PSUM") as pss_p, \
             tc.tile_pool(name="p3py", bufs=1, space="PSUM") as psy_p, \
             tc.tile_pool(name="p3sm", bufs=4) as smp, \
             tc.tile_pool(name="p3nf", bufs=1) as nfp:
            norms_full = nfp.tile([128, H, QTOK], f32, name="norms_full")
            for kv in range(KVH):
                K_sb = kvp.tile([128, 4, 512], bf16, tag="K")
                V_sb = kvp.tile([128, NCH, 128], bf16, tag="V")
                for g in range(4):
                    kg = kv_gath[g, 0].rearrange("(kv hd t) -> kv hd t",
                                                 kv=KVH, hd=HD)
                    vg = kv_gath[g, 1].rearrange("(t kv hd) -> t kv hd",
                                                 kv=KVH, hd=HD)
                    d = nc.sync.dma_start(out=K_sb[:, g, :], in_=kg[kv])
                    kload.append(d)
                    d = nc.sync.dma_start(
                        out=V_sb[:, 4 * g:4 * g + 4, :],
                        in_=vg[:, kv, :].rearrange("(c p) hd -> p c hd", p=128))
                    kload.append(d)
                psy = [psy_p.tile([128, 512], f32, tag=f"psy{hi}",
                                   name=f"psy{hi}") for hi in range(4)]
                for c in range(NCH):
                    q0, n = (0, 512) if c < 8 else (256, 256)
                    Kc = K_sb[:, c // 4, 128 * (c % 4):128 * (c % 4) + 128]
                    pts = []
                    for hi in range(4):
                        h = 4 * kv + hi
                        pss = pss_p.tile([128, 512], f32, tag=f"pss{hi}")
                        nc.tensor.matmul(pss[:, q0:q0 + n], Kc,
                                         qT_sb[:, h, q0:q0 + n],
                                         start=True, stop=True)
                        pt = ptq.tile([128, 512], bf16, tag="pt")
                        nc.scalar.activation(out=pt[:, q0:q0 + n],
                                             in_=pss[:, q0:q0 + n],
                                             func=AF.Exp, scale=INV_SQRT_HD)
                        mcol = 0 if c < 8 else 256
                        eng = nc.vector if hi % 2 == 0 else nc.gpsimd
                        eng.tensor_tensor(out=pt[:, mcol:mcol + 256],
                                          in0=pt[:, mcol:mcol + 256],
                                          in1=masks_sb[:, c % 8, :],
                                          op=ALU.mult)
                        pts.append(pt)
                    for hi in range(4):
                        nc.tensor.matmul(psy[hi][:, q0:q0 + n], V_sb[:, c, :],
                                         pts[hi][:, q0:q0 + n],
                                         start=(c == 0), stop=(c == NCH - 1),
                                         skip_group_check=True)
                # evacuate y, collect squared norms
                for hi in range(4):
                    h = 4 * kv + hi
                    nc.vector.tensor_copy(out=y_sb[:, h, :], in_=psy[hi][:])
                    ysq = smp.tile([128, 512], bf16, tag="ysq")
                    nc.gpsimd.tensor_tensor(out=ysq[:], in0=y_sb[:, h, :],
                                            in1=y_sb[:, h, :], op=ALU.mult)
                    psn = pss_p.tile([128, 512], f32, tag=f"pss{hi}")
                    nc.tensor.matmul(psn[:], ones_mat[:], ysq[:],
                                     start=True, stop=True)
                    evac(ev(hi), norms_full[:, h, :], psn[:])
            # batched rsqrt of all norms (one Ln + one Exp, 128 lanes)
            nf_flat = norms_full[:].rearrange("p h q -> p (h q)")
            nc.scalar.activation(out=nf_flat, in_=nf_flat, func=AF.Ln)
            nc.scalar.activation(out=rbf_all[:].rearrange("p h q -> p (h q)"),
                                 in_=nf_flat, func=AF.Exp, scale=-0.5)

        # transpose g (overlaps the attention tail; evacs off the scalar engine)
        with tc.tile_pool(name="p3gt", bufs=4, space="PSUM") as gtp:
            for m in range(4):
                for h in range(H):
                    pst = gtp.tile([128, 128], bf16, tag="pst")
                    nc.tensor.transpose(pst[:], g_sb[m][:, 128 * h:128 * h + 128],
                                        ident[:])
                    nc.vector.tensor_copy(out=gT_sb[:, h, 128 * m:128 * m + 128],
                                          in_=pst[:])

        # gating: gTr = y * g * rsqrt(norm)  (bf16 for out-proj lhsT)
        p_gTr = es.enter_context(tc.tile_pool(name="ppgTr", bufs=1))
        gTr_sb = p_gTr.tile([128, H, QTOK], bf16, name="gTr_sb")
        with tc.tile_pool(name="p3gs", bufs=4) as gsp:
            for h in range(H):
                tmp = gsp.tile([128, 512], bf16, tag="gtmp")
                nc.vector.tensor_tensor(out=tmp[:], in0=y_sb[:, h, :],
                                        in1=gT_sb[:, h, :], op=ALU.mult)
                nc.gpsimd.tensor_tensor(out=gTr_sb[:, h, :], in0=tmp[:],
                                        in1=rbf_all[:, h, :], op=ALU.mult)
        for d in kload:
            tile.add_dep_helper(d.ins, ag_k.ins, reason="allgather before load")

        # ==================================================== phase 4: out proj
        with tc.tile_pool(name="p4w", bufs=5) as wp, \
             tc.tile_pool(name="p4o", bufs=1) as op_, \
             tc.tile_pool(name="p4ps", bufs=1, space="PSUM") as pso_p, \
             tc.tile_pool(name="p4sm", bufs=2) as smp:
            out_sb = [op_.tile([128, D], f32, name=f"o{m}") for m in range(4)]
            for n in range(4):
                pso = [pso_p.tile([128, 512], f32, tag=f"po{m}", name=f"po{m}") for m in range(4)]
                for k4 in range(4):
                    wot = wp.tile([128, 4, 512], bf16, tag="wo")
                    nc.sync.dma_start(out=wot[:], in_=Wo_t[n, k4])
                    for j in range(4):
                        k = 4 * k4 + j
                        for m in range(4):
                            nc.tensor.matmul(pso[m][:],
                                             gTr_sb[:, k, 128 * m:128 * m + 128],
                                             wot[:, j, :], start=(k == 0),
                                             stop=(k == 15))
                for m in range(4):
                    evac(ev(m + n), out_sb[m][:, 512 * n:512 * n + 512], pso[m][:])
            for m in range(4):
                sq2 = smp.tile([128, D], f32, tag="osq")
                nc.vector.tensor_tensor(out=sq2[:], in0=out_sb[m][:],
                                        in1=out_sb[m][:], op=ALU.mult)
                s2 = smp.tile([128, 1], f32, tag="os2")
                nc.vector.tensor_reduce(out=s2[:], in_=sq2[:],
                                        axis=mybir.AxisListType.X, op=ALU.add)
                l2 = smp.tile([128, 1], f32, tag="oln")
                nc.scalar.activation(out=l2[:], in_=s2[:], func=AF.Ln,
                                     bias=oeps_t[:],
                                     scale=float(OUT_SCALE) / D)
                r2 = smp.tile([128, 1], f32, tag="ori")
                nc.scalar.activation(out=r2[:], in_=l2[:], func=AF.Exp, scale=-0.5)
                nc.vector.tensor_scalar_mul(out_sb[m][:], out_sb[m][:], r2[:])
                nc.sync.dma_start(out=out_y[128 * m:128 * m + 128, :],
                                  in_=out_sb[m][:])

    nc.compile()
    _CACHE["nc"] = nc
    return nc


def _host_inputs(xq, xk, xv, Wq, Wk, Wv, Wg, Wo, mix_k, mix_v):
    """Build the 8 per-core input maps."""
    import ml_dtypes
    f = np.float32
    bf = ml_dtypes.bfloat16
    xq = np.asarray(xq, f)
    xk = np.asarray(xk, f)
    xv = np.asarray(xv, f)
    Wq = np.asarray(Wq, f)
    Wk = np.asarray(Wk, f)
    Wv = np.asarray(Wv, f)
    Wg = np.asarray(Wg, f)
    Wo = np.asarray(Wo, f)
    mix_k = np.asarray(mix_k, f)
    mix_v = np.asarray(mix_v, f)

    Wk1 = (1.0 - mix_k)[:, None] * Wk
    Wk2 = mix_k[:, None] * Wk
    Wv1 = (1.0 - mix_v)[:, None] * Wv
    Wv2 = mix_v[:, None] * Wv

    # Wkv_t[k2][p][j] = row 256*k2+128*j+p of [Wk1|Wk2|Wv1|Wv2]
    Wkv = np.concatenate([Wk1, Wk2, Wv1, Wv2], axis=1)  # [2048, 2048]
    Wkv_t = np.ascontiguousarray(
        Wkv.reshape(8, 2, 128, 2048).transpose(0, 2, 1, 3).astype(bf))

    Wqg = np.concatenate([Wq, Wg], axis=1)  # [2048, 4096]
    # Wqg_t[n2][k2][p][j] = row 256*k2+128*j+p, cols 1024*n2..
    Wqg_t = np.ascontiguousarray(
        Wqg.reshape(8, 2, 128, 4, 1024).transpose(3, 0, 2, 1, 4).astype(bf))

    # Wo_t[n][k4][p][j] = row 512*k4+128*j+p, cols 512*n..
    Wo_t = np.ascontiguousarray(
        Wo.reshape(4, 4, 128, 4, 512).transpose(3, 0, 2, 1, 4).astype(bf))

    half = HD // 2
    inv_freq = 1.0 / (10000.0 ** (np.arange(half, dtype=np.float64) / half))
    ang = np.arange(T, dtype=np.float64)[:, None] * inv_freq[None, :]
    cos_t = np.concatenate([np.cos(ang), np.cos(ang)], axis=-1).astype(f)
    sin_t = np.concatenate([np.sin(ang), np.sin(ang)], axis=-1).astype(f)

    in_maps = []
    for c in range(NCORE):
        b, p = divmod(c, 4)
        rows_q = np.concatenate([np.arange(256 * p, 256 * p + 256),
                                 np.arange(1024 + 256 * p, 1024 + 256 * p + 256)])
        t0 = KVTOK * p
        rows_kv = np.arange(t0, t0 + KVTOK)

        xq_s = np.ascontiguousarray(xq[b, rows_q, :].astype(bf))
        xk_s = np.ascontiguousarray(xk[b, t0:t0 + KVTOK, :].astype(bf))
        xv_s = np.ascontiguousarray(xv[b, t0:t0 + KVTOK, :].astype(bf))
        xkb = np.zeros((1, D), f)
        xvb = np.zeros((1, D), f)
        if p > 0:
            xkb[0] = xk[b, t0 - 1, :]
            xvb[0] = xv[b, t0 - 1, :]

        # maskS[i][cc][j] = 1 iff kv token 128cc+i <= in-slot q token 256p+j
        ii = np.arange(128)[:, None]
        jj = np.arange(256)[None, :]
        mask = np.empty((8, 128, 256), f)
        for cc in range(8):
            mask[cc] = (128 * cc + ii <= 256 * p + jj).astype(f)
        mask = mask.transpose(1, 0, 2)  # partition-major [128, 8, 256]

        def pm(tab, rows):  # partition-major rope table [128, 4, HD]
            return np.ascontiguousarray(
                tab[rows].reshape(4, 128, HD).transpose(1, 0, 2))

        in_maps.append({
            "xq_sh": xq_s, "xk_sh": xk_s, "xv_sh": xv_s,
            "xkb": xkb.astype(bf), "xvb": xvb.astype(bf),
            "Wkv_t": Wkv_t, "Wqg_t": Wqg_t, "Wo_t": Wo_t,
            "cos_q": pm(cos_t, rows_q), "sin_q": pm(sin_t, rows_q),
            "cos_k": pm(cos_t, rows_kv), "sin_k": pm(sin_t, rows_kv),
            "maskS": np.ascontiguousarray(mask.astype(bf)),
        })
    return in_maps


def _run(in_maps, trace=False, tmpdir=None):
    _install_ntff_hook()
    from concourse.bass_utils import run_bass_kernel_spmd
    nc = _build()
    return run_bass_kernel_spmd(nc, in_maps, list(range(NCORE)),
                                trace=trace, tmpdir=tmpdir)


def kernel(xq, xk, xv, Wq, Wk, Wv, Wg, Wo, mix_k, mix_v,
           _trace=False, _tmpdir=None):
    in_maps = _host_inputs(xq, xk, xv, Wq, Wk, Wv, Wg, Wo, mix_k, mix_v)
    res = _run(in_maps, trace=_trace, tmpdir=_tmpdir)
    out = np.empty((B, T, D), np.float32)
    for c in range(NCORE):
        b, p = divmod(c, 4)
        y = res.results[c]["out_y"]
        out[b, 256 * p:256 * p + 256, :] = y[:256]
        out[b, 1024 + 256 * p:1024 + 256 * p + 256, :] = y[256:]
    kernel._last_exec_ns = res.exec_time_ns
    return out


# revision 41
# speedup vs baseline: 1.4414x; 1.0044x over previous
"""Trainium2 Bass kernel for nn_AttentionSubLayer (dense transformer attention
sublayer with time-lerp K/V mixing, QK-norm, RoPE, GQA, per-head l2 output
norm, gating, out-proj + final RMS norm).

Sharding: 8 cores = 2 batch groups x 4-way sequence parallel.  Core c
handles batch c//4; within the group (p = c%4) it owns q slots
slot0 = tokens [256p, 256p+256) and slot1 = [1024+256p, 1024+256p+256),
so slot0 only ever attends to kv tokens < 1024 and slot1 to all 2048.
K/V projections are computed on the owning quarter [512p, 512p+512) and
AllGathered (bf16) within each 4-core batch group.  Out-proj and final
RMS norm are local.

Numerics: bf16 matmul operands everywhere (fp32 PSUM), fp32 vector math
for the norms/rope.  Weights are pre-tiled on the host into contiguous
[128, n] k-chunk blocks so every weight DMA is one fat transfer.
Softmax skips max-subtraction (scores bounded by sqrt(HD) after QK
rms-norm) and the denominator (cancelled by the per-head l2 norm).
Causal masking is a 0/1 bf16 multiply on the exp output; the scalar
engine runs Exp only in attention (the l2-norm rsqrt is one batched
Ln+Exp at the end).
"""

import math
import sys
import types
from contextlib import ExitStack

sys.path.insert(0, "/opt/trn_rl_repo")

import numpy as np

# ---------------------------------------------------------------- problem dims
B, T, D, H, KVH, HD = 2, 2048, 2048, 16, 4, 128
N_LAYER = 24
EPS = 1e-8
NCORE = 8
QTOK = 512        # q tokens per core (2 slots x 256)
KVTOK = 512       # kv tokens per core (contiguous quarter)
NCH = 16          # kv chunks of 128 tokens (full 2048)
INV_SQRT_HD = 1.0 / math.sqrt(HD)
OUT_SCALE = 2 * N_LAYER  # final rms divided by sqrt(2*N_LAYER)


def _install_ntff_hook():
    try:
        import antenv
        if "antenv.axon_hooks" in sys.modules:
            return
        from trn_agent_boot.trn_boot import _ntff_profile_via_ctypes
        hook = _ntff_profile_via_ctypes("/opt/axon/libaxon_pjrt.so")
        mod = types.ModuleType("antenv.axon_hooks")
        mod.get_axon_ntff_profile_hook = lambda: hook
        antenv.axon_hooks = mod
        sys.modules["antenv.axon_hooks"] = mod
    except Exception:
        pass


_CACHE = {}


def _build():
    if "nc" in _CACHE:
        return _CACHE["nc"]

    import concourse.bass as bass
    import concourse.mybir as mybir
    import concourse.tile as tile
    from concourse import bacc
    from concourse.masks import make_identity

    f32 = mybir.dt.float32
    bf16 = mybir.dt.bfloat16
    AF = mybir.ActivationFunctionType
    ALU = mybir.AluOpType

    def bc_free(ap, n, at):
        """Insert a broadcast (stride-0) free dim of size n at position `at`
        of the AP's dim list (position counted incl. partition dim 0)."""
        new = list(list(d) for d in ap.ap)
        new.insert(at, [0, n])
        return bass.AP(tensor=ap.tensor, offset=ap.offset, ap=new)

    nc = bacc.Bacc("TRN2", target_bir_lowering=False, debug=False,
                   num_devices=NCORE)

    # ------------------------------------------------------------- I/O tensors
    xq_sh = nc.dram_tensor("xq_sh", [QTOK, D], bf16, kind="ExternalInput")
    xk_sh = nc.dram_tensor("xk_sh", [KVTOK, D], bf16, kind="ExternalInput")
    xv_sh = nc.dram_tensor("xv_sh", [KVTOK, D], bf16, kind="ExternalInput")
    xkb = nc.dram_tensor("xkb", [1, D], bf16, kind="ExternalInput")
    xvb = nc.dram_tensor("xvb", [1, D], bf16, kind="ExternalInput")
    # pre-tiled weights (host layout, all bf16, >=4KB per partition per DMA):
    #  Wkv_t[k2][p][j] = row 256*k2+128*j+p of [Wk1 | Wk2 | Wv1 | Wv2]
    Wkv_t = nc.dram_tensor("Wkv_t", [8, 128, 2, 2048], bf16, kind="ExternalInput")
    #  Wqg_t[n2][k2][p][j] = row 256*k2+128*j+p, cols 1024*n2.. of [Wq | Wg]
    Wqg_t = nc.dram_tensor("Wqg_t", [4, 8, 128, 2, 1024], bf16,
                           kind="ExternalInput")
    #  Wo_t[n][k4][p][j] = row 512*k4+128*j+p, cols 512*n.. of Wo
    Wo_t = nc.dram_tensor("Wo_t", [4, 4, 128, 4, 512], bf16,
                          kind="ExternalInput")
    # partition-major rope tables: [p][m][hd] = table[rows[128*m+p]][hd]
    cos_q = nc.dram_tensor("cos_q", [128, 4, HD], f32, kind="ExternalInput")
    sin_q = nc.dram_tensor("sin_q", [128, 4, HD], f32, kind="ExternalInput")
    cos_k = nc.dram_tensor("cos_k", [128, 4, HD], f32, kind="ExternalInput")
    sin_k = nc.dram_tensor("sin_k", [128, 4, HD], f32, kind="ExternalInput")
    # maskS[p][c] = 0/1 validity row p of kv chunk c vs 256 in-slot q cols
    maskS = nc.dram_tensor("maskS", [128, 8, 256], bf16, kind="ExternalInput")
    out_y = nc.dram_tensor("out_y", [QTOK, D], f32, kind="ExternalOutput")

    # staging for K/V allgather (within 4-core batch group)
    SHARD = KVH * HD * KVTOK
    kv_loc = nc.dram_tensor("kv_loc", [2, SHARD], bf16)
    kv_gath = nc.dram_tensor("kv_gath", [4, 2, SHARD], bf16)
    k_loc_v = kv_loc[0].rearrange("(kv hd t) -> kv hd t", kv=KVH, hd=HD)
    v_loc_v = kv_loc[1].rearrange("(t kv hd) -> t kv hd", kv=KVH, hd=HD)

    with tile.TileContext(nc) as tc, ExitStack() as es:
        # ------------------------------------------------------------ constants
        cpool = es.enter_context(tc.tile_pool(name="consts", bufs=1))
        ident = cpool.tile([128, 128], bf16)
        make_identity(nc, ident[:])
        ones_mat = cpool.tile([128, 128], bf16)
        nc.vector.memset(ones_mat[:], 1.0)
        eps_t = cpool.tile([128, 1], f32)
        nc.vector.memset(eps_t[:], EPS)
        oeps_t = cpool.tile([128, 1], f32)
        nc.vector.memset(oeps_t[:], float(OUT_SCALE) * EPS)
        cosq_sb = cpool.tile([128, 4, HD], f32)
        sinq_sb = cpool.tile([128, 4, HD], f32)
        cosk_sb = cpool.tile([128, 4, HD], f32)
        sink_sb = cpool.tile([128, 4, HD], f32)
        nc.sync.dma_start(out=cosq_sb[:], in_=cos_q[:, :, :])
        nc.sync.dma_start(out=sinq_sb[:], in_=sin_q[:, :, :])
        nc.sync.dma_start(out=cosk_sb[:], in_=cos_k[:, :, :])
        nc.sync.dma_start(out=sink_sb[:], in_=sin_k[:, :, :])
        masks_sb = cpool.tile([128, 8, 256], bf16, name="masks_sb")
        nc.sync.dma_start(out=masks_sb[:], in_=maskS[:, :, :])

        # ============================================================ helpers
        def ev(i):
            return nc.scalar if i % 2 == 0 else nc.vector

        def evac(engine, out, in_):
            if engine is nc.scalar:
                engine.copy(out=out, in_=in_)
            else:
                engine.tensor_copy(out=out, in_=in_)

        def transpose_in(x_dram, xT, col0, natp, ptp, eng=None):
            """Load natural [512, D] bf16 DRAM -> xT[:, k, col0+...] transposed."""
            for m in range(4):
                nat = natp.tile([128, D], bf16, tag="nat")
                nc.sync.dma_start(out=nat[:], in_=x_dram[128 * m:128 * m + 128, :])
                for k in range(16):
                    pst = ptp.tile([128, 128], bf16, tag="pst")
                    nc.tensor.transpose(pst[:], nat[:, 128 * k:128 * k + 128], ident[:])
                    evac(eng or ev(k), xT[:, k, col0 + 128 * m:col0 + 128 * m + 128],
                         pst[:])

        def rms_factors(x_ap, nh, smp):
            """Per-head rsqrt(mean(x^2 over HD) + EPS); returns ri [128, nh]."""
            sq = smp.tile([128, nh * HD], f32, tag="rsq")
            nc.vector.tensor_tensor(out=sq[:], in0=x_ap, in1=x_ap, op=ALU.mult)
            s2 = smp.tile([128, nh], f32, tag="rs2")
            nc.vector.tensor_reduce(out=s2[:],
                                    in_=sq[:].rearrange("p (h d) -> p h d", h=nh),
                                    axis=mybir.AxisListType.X, op=ALU.add)
            ln = smp.tile([128, nh], f32, tag="rln")
            nc.scalar.activation(out=ln[:], in_=s2[:], func=AF.Ln,
                                 bias=eps_t[:], scale=1.0 / HD)
            ri = smp.tile([128, nh], f32, tag="rri")
            nc.scalar.activation(out=ri[:], in_=ln[:], func=AF.Exp, scale=-0.5)
            return ri

        def rms_batch(x_ap, nh, smp, out_bf=None):
            """x *= rsqrt(mean(x^2 over HD) + EPS), batched over nh heads.
            x_ap [128, nh*HD] f32 AP; optionally write result to out_bf (bf16)."""
            x3 = x_ap.rearrange("p (h d) -> p h d", h=nh)
            ri = rms_factors(x_ap, nh, smp)
            dst = (out_bf.rearrange("p (h d) -> p h d", h=nh)
                   if out_bf is not None else x3)
            nc.vector.tensor_tensor(out=dst, in0=x3, in1=bc_free(ri[:], HD, 2),
                                    op=ALU.mult)

        def rope_batch(dst_bf, src, nh, cos_sb, sin_sb, m, smp, eng=None):
            """dst = rope(src) for nh heads at once; dst bf16 AP, src f32 AP."""
            eng = eng or nc.vector
            half = HD // 2
            cos_bc = bc_free(cos_sb[:, m, :], nh, 1)          # [128, nh, HD]
            sinlo_bc = bc_free(sin_sb[:, m, 0:half], nh, 1)   # [128, nh, half]
            sinhi_bc = bc_free(sin_sb[:, m, half:HD], nh, 1)
            s3 = src.rearrange("p (h d) -> p h d", h=nh)
            d3 = dst_bf.rearrange("p (h d) -> p h d", h=nh)
            t0 = smp.tile([128, nh * HD], f32, tag="ro0")
            t03 = t0[:].rearrange("p (h d) -> p h d", h=nh)
            eng.tensor_tensor(out=t03, in0=s3, in1=cos_bc, op=ALU.mult)
            t1 = smp.tile([128, nh * half], f32, tag="ro1")
            t13 = t1[:].rearrange("p (h d) -> p h d", h=nh)
            eng.tensor_tensor(out=t13, in0=s3[:, :, half:HD], in1=sinlo_bc,
                              op=ALU.mult)
            eng.tensor_tensor(out=d3[:, :, 0:half], in0=t03[:, :, 0:half],
                              in1=t13, op=ALU.subtract)
            eng.tensor_tensor(out=t13, in0=s3[:, :, 0:half], in1=sinhi_bc,
                              op=ALU.mult)
            eng.tensor_tensor(out=d3[:, :, half:HD], in0=t03[:, :, half:HD],
                              in1=t13, op=ALU.add)

        # ===================================================== phase 1: K / V
        stage_dmas = []
        p2x = es.enter_context(tc.tile_pool(name="p2x", bufs=1))
        xqT = p2x.tile([128, 16, QTOK], bf16, name="xqT")
        wq0p = es.enter_context(tc.tile_pool(name="wq0", bufs=1))
        wq0_t = [wq0p.tile([128, 2, 1024], bf16, name=f"wq0_{i}")
                 for i in range(4)]
        with tc.tile_pool(name="p1kvn", bufs=1) as kvnat:
            knat = [kvnat.tile([128, KVH * HD], f32, name=f"kn{m}")
                     for m in range(4)]
            vnat = [kvnat.tile([128, KVH * HD], f32, name=f"vn{m}") for m in range(4)]
            with tc.tile_pool(name="p1x", bufs=1) as p1x:
                xkT = p1x.tile([128, 16, KVTOK + 1], bf16, name="xkT")
                xvT = p1x.tile([128, 16, KVTOK + 1], bf16, name="xvT")
                with tc.tile_pool(name="p1nat", bufs=2) as natp, \
                     tc.tile_pool(name="p1pst", bufs=4, space="PSUM") as ptp:
                    # boundary token -> free position 0 of each k-chunk
                    nc.sync.dma_start(out=xkT[:, :, 0],
                                      in_=xkb[0].rearrange("(k p) -> p k", p=128))
                    nc.sync.dma_start(out=xvT[:, :, 0],
                                      in_=xvb[0].rearrange("(k p) -> p k", p=128))
                    transpose_in(xk_sh, xkT, 1, natp, ptp)
                    transpose_in(xv_sh, xvT, 1, natp, ptp)

                with tc.tile_pool(name="p1w", bufs=3) as wp, \
                     tc.tile_pool(name="p1ps", bufs=1, space="PSUM") as pskv:
                    psK = [pskv.tile([128, 512], f32, tag=f"pK{m}", name=f"pK{m}") for m in range(4)]
                    psV = [pskv.tile([128, 512], f32, tag=f"pV{m}", name=f"pV{m}") for m in range(4)]
                    for k2 in range(8):
                        wt = wp.tile([128, 2, 2048], bf16, tag="wkv")
                        nc.sync.dma_start(out=wt[:], in_=Wkv_t[k2])
                        for j in range(2):
                            k = 2 * k2 + j
                            w = wt[:, j, :]
                            for m in range(4):
                                n0, n1 = 1 + 128 * m, 129 + 128 * m   # normal
                                s0, s1 = 128 * m, 128 * m + 128       # shifted
                                nc.tensor.matmul(psK[m][:], xkT[:, k, n0:n1],
                                                 w[:, 0:512], start=(k == 0),
                                                 stop=False)
                                nc.tensor.matmul(psK[m][:], xkT[:, k, s0:s1],
                                                 w[:, 512:1024], start=False,
                                                 stop=(k == 15))
                                nc.tensor.matmul(psV[m][:], xvT[:, k, n0:n1],
                                                 w[:, 1024:1536], start=(k == 0),
                                                 stop=False)
                                nc.tensor.matmul(psV[m][:], xvT[:, k, s0:s1],
                                                 w[:, 1536:2048], start=False,
                                                 stop=(k == 15))
                    for m in range(4):
                        evac(ev(m), knat[m][:], psK[m][:])
                        evac(ev(m + 1), vnat[m][:], psV[m][:])

            # xq transposes here: tensor fills the p1-tail gap while the
            # vector engine does the K/V rms/rope below (evacs on scalar)
            with tc.tile_pool(name="p2nat", bufs=2) as natp, \
                 tc.tile_pool(name="p2pst", bufs=4, space="PSUM") as ptp:
                transpose_in(xq_sh, xqT, 0, natp, ptp, eng=nc.scalar)

            # prefetch the first Wq tiles ahead of the staging DMAs so the
            # Q matmuls are fed while the allgather occupies the queue
            for i in range(4):
                nc.sync.dma_start(out=wq0_t[i][:], in_=Wqg_t[0, i])

            with tc.tile_pool(name="p1pst2", bufs=2, space="PSUM") as ptp2, \
                 tc.tile_pool(name="p1sm", bufs=2) as smp, \
                 tc.tile_pool(name="p1st", bufs=3) as stp, \
                 tc.tile_pool(name="p1kt", bufs=1) as ktp:
                kT_full = ktp.tile([128, KVH, KVTOK], bf16, name="kT_full")
                for m in range(4):
                    # V: rms -> bf16, stage [t, kv, hd] (one fat DMA per m)
                    vout = stp.tile([128, KVH * HD], bf16, tag="vout")
                    rms_batch(vnat[m][:], KVH, smp, out_bf=vout[:])
                    d = nc.sync.dma_start(
                        out=v_loc_v[128 * m:128 * m + 128, :, :],
                        in_=vout[:].rearrange("p (kv hd) -> p kv hd", kv=KVH))
                    stage_dmas.append(d)
                    # K: rope raw (gpsimd) in parallel with rms factors
                    # (vector/scalar); per-head rms scaling commutes with rope
                    ri = rms_factors(knat[m][:], KVH, smp)
                    kror = stp.tile([128, KVH * HD], f32, tag="kror")
                    rope_batch(kror[:], knat[m][:], KVH, cosk_sb, sink_sb, m, smp,
                               eng=nc.gpsimd)
                    krot = stp.tile([128, KVH * HD], bf16, tag="krot")
                    nc.vector.tensor_tensor(
                        out=krot[:].rearrange("p (h d) -> p h d", h=KVH),
                        in0=kror[:].rearrange("p (h d) -> p h d", h=KVH),
                        in1=bc_free(ri[:], HD, 2), op=ALU.mult)
                    for kv in range(KVH):
                        pst = ptp2.tile([128, 128], bf16, tag="pst")
                        nc.tensor.transpose(pst[:], krot[:, 128 * kv:128 * kv + 128],
                                            ident[:])
                        evac(ev(kv), kT_full[:, kv, 128 * m:128 * m + 128], pst[:])
                for kv in range(KVH):
                    d = nc.sync.dma_start(out=k_loc_v[kv], in_=kT_full[:, kv, :])
                    stage_dmas.append(d)

        ag_k = nc.gpsimd.collective_compute(
            "AllGather", ALU.bypass,
            replica_groups=[[0, 1, 2, 3], [4, 5, 6, 7]],
            ins=[kv_loc[:]], outs=[kv_gath[:]])
        for d in stage_dmas:
            tile.add_dep_helper(ag_k.ins, d.ins, reason="stage before allgather")

        # ===================================================== phase 2: Q / G
        p_qT = es.enter_context(tc.tile_pool(name="ppqT", bufs=1))
        qT_sb = p_qT.tile([128, H, QTOK], bf16, name="qT_sb")
        p_gT = es.enter_context(tc.tile_pool(name="ppgT", bufs=1))
        gT_sb = p_gT.tile([128, H, QTOK], bf16, name="gT_sb")
        p_gn = es.enter_context(tc.tile_pool(name="ppgn", bufs=1))
        g_sb = [p_gn.tile([128, H * HD], bf16, name=f"g{m}") for m in range(4)]
        with tc.tile_pool(name="p2qn", bufs=1) as qnat, \
             tc.tile_pool(name="p2qr", bufs=1) as qrp:
            q_sb = [qnat.tile([128, H * HD], bf16, name=f"q{m}") for m in range(4)]
            qrot = [qrp.tile([128, H * HD], bf16, name=f"qr{m}") for m in range(4)]
            with tc.tile_pool(name="p2w", bufs=3) as wp, \
                 tc.tile_pool(name="p2ps", bufs=1, space="PSUM") as psqg, \
                 tc.tile_pool(name="p2sm", bufs=2) as smp:
                def qg_pass(n2, dsts):
                    ps = [psqg.tile([128, 512], f32, tag=f"pqg{i}",
                                    name=f"pqg{i}") for i in range(8)]
                    for k2 in range(8):
                        if n2 == 0 and k2 < 4:
                            wt = wq0_t[k2]
                        else:
                            wt = wp.tile([128, 2, 1024], bf16, tag="wqg")
                            nc.sync.dma_start(out=wt[:], in_=Wqg_t[n2, k2])
                        for j in range(2):
                            k = 2 * k2 + j
                            for m in range(4):
                                nc.tensor.matmul(ps[2 * m][:],
                                                 xqT[:, k, 128 * m:128 * m + 128],
                                                 wt[:, j, 0:512], start=(k == 0),
                                                 stop=(k == 15))
                                nc.tensor.matmul(ps[2 * m + 1][:],
                                                 xqT[:, k, 128 * m:128 * m + 128],
                                                 wt[:, j, 512:1024], start=(k == 0),
                                                 stop=(k == 15))
                    for m in range(4):
                        c0 = 1024 * (n2 % 2)
                        t = dsts[m]
                        evac(ev(m), t[:, c0:c0 + 512], ps[2 * m][:])
                        evac(ev(m + 1), t[:, c0 + 512:c0 + 1024], ps[2 * m + 1][:])

                qg_pass(0, q_sb)
                qg_pass(1, q_sb)
                # q rms factors + raw rope + scaled mult; hidden under G matmuls
                for m in range(4):
                    ri = rms_factors(q_sb[m][:], H, smp)
                    qror = smp.tile([128, H * HD], f32, tag="qror")
                    rope_batch(qror[:], q_sb[m][:], H, cosq_sb, sinq_sb, m, smp)
                    nc.vector.tensor_tensor(
                        out=qrot[m][:].rearrange("p (h d) -> p h d", h=H),
                        in0=qror[:].rearrange("p (h d) -> p h d", h=H),
                        in1=bc_free(ri[:], HD, 2), op=ALU.mult)
                qg_pass(2, g_sb)
                qg_pass(3, g_sb)

            # transpose q and g (fills the tensor gap before attention)
            with tc.tile_pool(name="p2pst2", bufs=4, space="PSUM") as ptp2:
                for m in range(4):
                    for h in range(H):
                        pst = ptp2.tile([128, 128], bf16, tag="pst")
                        nc.tensor.transpose(pst[:],
                                            qrot[m][:, 128 * h:128 * h + 128],
                                            ident[:])
                        evac(ev(h), qT_sb[:, h, 128 * m:128 * m + 128], pst[:])
                for m in range(4):
                    for h in range(H):
                        pst = ptp2.tile([128, 128], bf16, tag="pst")
                        nc.tensor.transpose(pst[:],
                                            g_sb[m][:, 128 * h:128 * h + 128],
                                            ident[:])
                        evac(ev(h + 1), gT_sb[:, h, 128 * m:128 * m + 128], pst[:])

        # ==================================================== phase 3: attention
        p_y = es.enter_context(tc.tile_pool(name="ppy", bufs=1))
        y_sb = p_y.tile([128, H, QTOK], bf16, name="y_sb")
        p_n = es.enter_context(tc.tile_pool(name="ppn", bufs=1))
        rbf_all = p_n.tile([128, H, QTOK], bf16, name="rbf_all")

        kload = []
        with tc.tile_pool(name="p3kv", bufs=2) as kvp, \
             tc.tile_pool(name="p3pt", bufs=6) as ptq, \
             tc.tile_pool(name="p3ps", bufs=1, space="PSUM") as pss_p, \
             tc.tile_pool(name="p3py", bufs=1, space="PSUM") as psy_p, \
             tc.tile_pool(name="p3sm", bufs=4) as smp, \
             tc.tile_pool(name="p3nf", bufs=1) as nfp:
            norms_full = nfp.tile([128, H, QTOK], f32, name="norms_full")
            for kv in range(KVH):
                K_sb = kvp.tile([128, 4, 512], bf16, tag="K")
                V_sb = kvp.tile([128, NCH, 128], bf16, tag="V")
                for g in range(4):
                    kg = kv_gath[g, 0].rearrange("(kv hd t) -> kv hd t",
                                                 kv=KVH, hd=HD)
                    vg = kv_gath[g, 1].rearrange("(t kv hd) -> t kv hd",
                                                 kv=KVH, hd=HD)
                    d = nc.sync.dma_start(out=K_sb[:, g, :], in_=kg[kv])
                    kload.append(d)
                    d = nc.sync.dma_start(
                        out=V_sb[:, 4 * g:4 * g + 4, :],
                        in_=vg[:, kv, :].rearrange("(c p) hd -> p c hd", p=128))
                    kload.append(d)
                psy = [psy_p.tile([128, 512], f32, tag=f"psy{hi}",
                                   name=f"psy{hi}") for hi in range(4)]
                for c in range(NCH):
                    q0, n = (0, 512) if c < 8 else (256, 256)
                    Kc = K_sb[:, c // 4, 128 * (c % 4):128 * (c % 4) + 128]
                    pts = []
                    for hi in range(4):
                        h = 4 * kv + hi
                        pss = pss_p.tile([128, 512], f32, tag=f"pss{hi}")
                        nc.tensor.matmul(pss[:, q0:q0 + n], Kc,
                                         qT_sb[:, h, q0:q0 + n],
                                         start=True, stop=True)
                        pt = ptq.tile([128, 512], bf16, tag="pt")
                        nc.scalar.activation(out=pt[:, q0:q0 + n],
                                             in_=pss[:, q0:q0 + n],
                                             func=AF.Exp, scale=INV_SQRT_HD)
                        mcol = 0 if c < 8 else 256
                        eng = nc.vector if hi % 2 == 0 else nc.gpsimd
                        eng.tensor_tensor(out=pt[:, mcol:mcol + 256],
                                          in0=pt[:, mcol:mcol + 256],
                                          in1=masks_sb[:, c % 8, :],
                                          op=ALU.mult)
                        pts.append(pt)
                    for hi in range(4):
                        nc.tensor.matmul(psy[hi][:, q0:q0 + n], V_sb[:, c, :],
                                         pts[hi][:, q0:q0 + n],
                                         start=(c == 0), stop=(c == NCH - 1),
                                         skip_group_check=True)
                # evacuate y, collect squared norms
                for hi in range(4):
                    h = 4 * kv + hi
                    nc.vector.tensor_copy(out=y_sb[:, h, :], in_=psy[hi][:])
                    ysq = smp.tile([128, 512], bf16, tag="ysq")
                    nc.gpsimd.tensor_tensor(out=ysq[:], in0=y_sb[:, h, :],
                                            in1=y_sb[:, h, :], op=ALU.mult)
                    psn = pss_p.tile([128, 512], f32, tag=f"pss{hi}")
                    nc.tensor.matmul(psn[:], ones_mat[:], ysq[:],
                                     start=True, stop=True)
                    evac(ev(hi), norms_full[:, h, :], psn[:])
            # batched rsqrt of all norms (one Ln + one Exp, 128 lanes)
            nf_flat = norms_full[:].rearrange("p h q -> p (h q)")
            nc.scalar.activation(out=nf_flat, in_=nf_flat, func=AF.Ln)
            nc.scalar.activation(out=rbf_all[:].rearrange("p h q -> p (h q)"),
                                 in_=nf_flat, func=AF.Exp, scale=-0.5)

        # gating in place: y_sb = y * g * rsqrt(norm)  (bf16 out-proj lhsT)
        gTr_sb = y_sb
        with tc.tile_pool(name="p3gs", bufs=4) as gsp:
            for h in range(H):
                tmp = gsp.tile([128, 512], bf16, tag="gtmp")
                nc.vector.tensor_tensor(out=tmp[:], in0=y_sb[:, h, :],
                                        in1=gT_sb[:, h, :], op=ALU.mult)
                nc.gpsimd.tensor_tensor(out=gTr_sb[:, h, :], in0=tmp[:],
                                        in1=rbf_all[:, h, :], op=ALU.mult)
        for d in kload:
            tile.add_dep_helper(d.ins, ag_k.ins, reason="allgather before load")

        # ==================================================== phase 4: out proj
        with tc.tile_pool(name="p4w", bufs=5) as wp, \
             tc.tile_pool(name="p4o", bufs=1) as op_, \
             tc.tile_pool(name="p4ps", bufs=1, space="PSUM") as pso_p, \
             tc.tile_pool(name="p4sm", bufs=2) as smp:
            out_sb = [op_.tile([128, D], f32, name=f"o{m}") for m in range(4)]
            for n in range(4):
                pso = [pso_p.tile([128, 512], f32, tag=f"po{m}", name=f"po{m}") for m in range(4)]
                for k4 in range(4):
                    wot = wp.tile([128, 4, 512], bf16, tag="wo")
                    nc.sync.dma_start(out=wot[:], in_=Wo_t[n, k4])
                    for j in range(4):
                        k = 4 * k4 + j
                        for m in range(4):
                            nc.tensor.matmul(pso[m][:],
                                             gTr_sb[:, k, 128 * m:128 * m + 128],
                                             wot[:, j, :], start=(k == 0),
                                             stop=(k == 15))
                for m in range(4):
                    evac(ev(m + n), out_sb[m][:, 512 * n:512 * n + 512], pso[m][:])
            for m in range(4):
                sq2 = smp.tile([128, D], f32, tag="osq")
                nc.vector.tensor_tensor(out=sq2[:], in0=out_sb[m][:],
                                        in1=out_sb[m][:], op=ALU.mult)
                s2 = smp.tile([128, 1], f32, tag="os2")
                nc.vector.tensor_reduce(out=s2[:], in_=sq2[:],
                                        axis=mybir.AxisListType.X, op=ALU.add)
                l2 = smp.tile([128, 1], f32, tag="oln")
                nc.scalar.activation(out=l2[:], in_=s2[:], func=AF.Ln,
                                     bias=oeps_t[:],
                                     scale=float(OUT_SCALE) / D)
                r2 = smp.tile([128, 1], f32, tag="ori")
                nc.scalar.activation(out=r2[:], in_=l2[:], func=AF.Exp, scale=-0.5)
                nc.vector.tensor_scalar_mul(out_sb[m][:], out_sb[m][:], r2[:])
                nc.sync.dma_start(out=out_y[128 * m:128 * m + 128, :],
                                  in_=out_sb[m][:])

    nc.compile()
    _CACHE["nc"] = nc
    return nc


def _host_inputs(xq, xk, xv, Wq, Wk, Wv, Wg, Wo, mix_k, mix_v):
    """Build the 8 per-core input maps."""
    import ml_dtypes
    f = np.float32
    bf = ml_dtypes.bfloat16
    xq = np.asarray(xq, f)
    xk = np.asarray(xk, f)
    xv = np.asarray(xv, f)
    Wq = np.asarray(Wq, f)
    Wk = np.asarray(Wk, f)
    Wv = np.asarray(Wv, f)
    Wg = np.asarray(Wg, f)
    Wo = np.asarray(Wo, f)
    mix_k = np.asarray(mix_k, f)
    mix_v = np.asarray(mix_v, f)

    Wk1 = (1.0 - mix_k)[:, None] * Wk
    Wk2 = mix_k[:, None] * Wk
    Wv1 = (1.0 - mix_v)[:, None] * Wv
    Wv2 = mix_v[:, None] * Wv

    # Wkv_t[k2][p][j] = row 256*k2+128*j+p of [Wk1|Wk2|Wv1|Wv2]
    Wkv = np.concatenate([Wk1, Wk2, Wv1, Wv2], axis=1)  # [2048, 2048]
    Wkv_t = np.ascontiguousarray(
        Wkv.reshape(8, 2, 128, 2048).transpose(0, 2, 1, 3).astype(bf))

    Wqg = np.concatenate([Wq, Wg], axis=1)  # [2048, 4096]
    # Wqg_t[n2][k2][p][j] = row 256*k2+128*j+p, cols 1024*n2..
    Wqg_t = np.ascontiguousarray(
        Wqg.reshape(8, 2, 128, 4, 1024).transpose(3, 0, 2, 1, 4).astype(bf))

    # Wo_t[n][k4][p][j] = row 512*k4+128*j+p, cols 512*n..
    Wo_t = np.ascontiguousarray(
        Wo.reshape(4, 4, 128, 4, 512).transpose(3, 0, 2, 1, 4).astype(bf))

    half = HD // 2
    inv_freq = 1.0 / (10000.0 ** (np.arange(half, dtype=np.float64) / half))
    ang = np.arange(T, dtype=np.float64)[:, None] * inv_freq[None, :]
    cos_t = np.concatenate([np.cos(ang), np.cos(ang)], axis=-1).astype(f)
    sin_t = np.concatenate([np.sin(ang), np.sin(ang)], axis=-1).astype(f)

    in_maps = []
    for c in range(NCORE):
        b, p = divmod(c, 4)
        rows_q = np.concatenate([np.arange(256 * p, 256 * p + 256),
                                 np.arange(1024 + 256 * p, 1024 + 256 * p + 256)])
        t0 = KVTOK * p
        rows_kv = np.arange(t0, t0 + KVTOK)

        xq_s = np.ascontiguousarray(xq[b, rows_q, :].astype(bf))
        xk_s = np.ascontiguousarray(xk[b, t0:t0 + KVTOK, :].astype(bf))
        xv_s = np.ascontiguousarray(xv[b, t0:t0 + KVTOK, :].astype(bf))
        xkb = np.zeros((1, D), f)
        xvb = np.zeros((1, D), f)
        if p > 0:
            xkb[0] = xk[b, t0 - 1, :]
            xvb[0] = xv[b, t0 - 1, :]

        # maskS[i][cc][j] = 1 iff kv token 128cc+i <= in-slot q token 256p+j
        ii = np.arange(128)[:, None]
        jj = np.arange(256)[None, :]
        mask = np.empty((8, 128, 256), f)
        for cc in range(8):
            mask[cc] = (128 * cc + ii <= 256 * p + jj).astype(f)
        mask = mask.transpose(1, 0, 2)  # partition-major [128, 8, 256]

        def pm(tab, rows):  # partition-major rope table [128, 4, HD]
            return np.ascontiguousarray(
                tab[rows].reshape(4, 128, HD).transpose(1, 0, 2))

        in_maps.append({
            "xq_sh": xq_s, "xk_sh": xk_s, "xv_sh": xv_s,
            "xkb": xkb.astype(bf), "xvb": xvb.astype(bf),
            "Wkv_t": Wkv_t, "Wqg_t": Wqg_t, "Wo_t": Wo_t,
            "cos_q": pm(cos_t, rows_q), "sin_q": pm(sin_t, rows_q),
            "cos_k": pm(cos_t, rows_kv), "sin_k": pm(sin_t, rows_kv),
            "maskS": np.ascontiguousarray(mask.astype(bf)),
        })
    return in_maps


def _run(in_maps, trace=False, tmpdir=None):
    _install_ntff_hook()
    from concourse.bass_utils import run_bass_kernel_spmd
    nc = _build()
    return run_bass_kernel_spmd(nc, in_maps, list(range(NCORE)),
                                trace=trace, tmpdir=tmpdir)


def kernel(xq, xk, xv, Wq, Wk, Wv, Wg, Wo, mix_k, mix_v,
           _trace=False, _tmpdir=None):
    in_maps = _host_inputs(xq, xk, xv, Wq, Wk, Wv, Wg, Wo, mix_k, mix_v)
    res = _run(in_maps, trace=_trace, tmpdir=_tmpdir)
    out = np.empty((B, T, D), np.float32)
    for c in range(NCORE):
        b, p = divmod(c, 4)
        y = res.results[c]["out_y"]
        out[b, 256 * p:256 * p + 256, :] = y[:256]
        out[b, 1024 + 256 * p:1024 + 256 * p + 256, :] = y[256:]
    kernel._last_exec_ns = res.exec_time_ns
    return out


# revision 48
# speedup vs baseline: 1.5186x; 1.0536x over previous
"""Trainium2 Bass kernel for nn_AttentionSubLayer (dense transformer attention
sublayer with time-lerp K/V mixing, QK-norm, RoPE, GQA, per-head l2 output
norm, gating, out-proj + final RMS norm).

Sharding: 8 cores = 2 batch groups x 4-way sequence parallel.  Core c
handles batch c//4; within the group (p = c%4) it owns q slots
slot0 = tokens [256p, 256p+256) and slot1 = [1024+256p, 1024+256p+256),
so slot0 only ever attends to kv tokens < 1024 and slot1 to all 2048.
K/V projections are computed on the owning quarter [512p, 512p+512) and
AllGathered (bf16) within each 4-core batch group.  Out-proj and final
RMS norm are local.

Numerics: bf16 matmul operands everywhere (fp32 PSUM), fp32 vector math
for the norms/rope.  Weights are pre-tiled on the host into contiguous
[128, n] k-chunk blocks so every weight DMA is one fat transfer.
Softmax skips max-subtraction (scores bounded by sqrt(HD) after QK
rms-norm) and the denominator (cancelled by the per-head l2 norm).
Causal masking is a 0/1 bf16 multiply on the exp output; the scalar
engine runs Exp only in attention (the l2-norm rsqrt is one batched
Ln+Exp at the end).
"""

import math
import sys
import types
from contextlib import ExitStack

sys.path.insert(0, "/opt/trn_rl_repo")

import numpy as np

# ---------------------------------------------------------------- problem dims
B, T, D, H, KVH, HD = 2, 2048, 2048, 16, 4, 128
N_LAYER = 24
EPS = 1e-8
NCORE = 8
QTOK = 512        # q tokens per core (2 slots x 256)
KVTOK = 512       # kv tokens per core (contiguous quarter)
NCH = 16          # kv chunks of 128 tokens (full 2048)
INV_SQRT_HD = 1.0 / math.sqrt(HD)
OUT_SCALE = 2 * N_LAYER  # final rms divided by sqrt(2*N_LAYER)


def _install_ntff_hook():
    try:
        import antenv
        if "antenv.axon_hooks" in sys.modules:
            return
        from trn_agent_boot.trn_boot import _ntff_profile_via_ctypes
        hook = _ntff_profile_via_ctypes("/opt/axon/libaxon_pjrt.so")
        mod = types.ModuleType("antenv.axon_hooks")
        mod.get_axon_ntff_profile_hook = lambda: hook
        antenv.axon_hooks = mod
        sys.modules["antenv.axon_hooks"] = mod
    except Exception:
        pass


_CACHE = {}


def _build():
    if "nc" in _CACHE:
        return _CACHE["nc"]

    import concourse.bass as bass
    import concourse.mybir as mybir
    import concourse.tile as tile
    from concourse import bacc
    from concourse.masks import make_identity

    f32 = mybir.dt.float32
    bf16 = mybir.dt.bfloat16
    AF = mybir.ActivationFunctionType
    ALU = mybir.AluOpType

    def bc_free(ap, n, at):
        """Insert a broadcast (stride-0) free dim of size n at position `at`
        of the AP's dim list (position counted incl. partition dim 0)."""
        new = list(list(d) for d in ap.ap)
        new.insert(at, [0, n])
        return bass.AP(tensor=ap.tensor, offset=ap.offset, ap=new)

    nc = bacc.Bacc("TRN2", target_bir_lowering=False, debug=False,
                   num_devices=NCORE)

    # ------------------------------------------------------------- I/O tensors
    xq_sh = nc.dram_tensor("xq_sh", [QTOK, D], bf16, kind="ExternalInput")
    xk_sh = nc.dram_tensor("xk_sh", [KVTOK, D], bf16, kind="ExternalInput")
    xv_sh = nc.dram_tensor("xv_sh", [KVTOK, D], bf16, kind="ExternalInput")
    xkb = nc.dram_tensor("xkb", [1, D], bf16, kind="ExternalInput")
    xvb = nc.dram_tensor("xvb", [1, D], bf16, kind="ExternalInput")
    # pre-tiled weights (host layout, all bf16, >=4KB per partition per DMA):
    #  Wkv_t[k2][p][j] = row 256*k2+128*j+p of [Wk1 | Wk2 | Wv1 | Wv2]
    Wkv_t = nc.dram_tensor("Wkv_t", [8, 128, 2, 2048], bf16, kind="ExternalInput")
    #  Wqg_t[n2][k2][p][j] = row 256*k2+128*j+p, cols 1024*n2.. of [Wq | Wg]
    Wqg_t = nc.dram_tensor("Wqg_t", [4, 8, 128, 2, 1024], bf16,
                           kind="ExternalInput")
    #  Wo_t[n][k4][p][j] = row 512*k4+128*j+p, cols 512*n.. of Wo
    Wo_t = nc.dram_tensor("Wo_t", [4, 4, 128, 4, 512], bf16,
                          kind="ExternalInput")
    # partition-major rope tables: [p][m][hd] = table[rows[128*m+p]][hd]
    cos_q = nc.dram_tensor("cos_q", [128, 4, HD], f32, kind="ExternalInput")
    sin_q = nc.dram_tensor("sin_q", [128, 4, HD], f32, kind="ExternalInput")
    cos_k = nc.dram_tensor("cos_k", [128, 4, HD], f32, kind="ExternalInput")
    sin_k = nc.dram_tensor("sin_k", [128, 4, HD], f32, kind="ExternalInput")
    # maskS[p][c] = 0/1 validity row p of kv chunk c vs 256 in-slot q cols
    maskS = nc.dram_tensor("maskS", [128, 8, 256], bf16, kind="ExternalInput")
    out_y = nc.dram_tensor("out_y", [QTOK, D], f32, kind="ExternalOutput")

    # staging for K/V allgather (within 4-core batch group)
    SHARD = KVH * HD * KVTOK
    kv_loc = nc.dram_tensor("kv_loc", [2, SHARD], bf16)
    kv_gath = nc.dram_tensor("kv_gath", [4, 2, SHARD], bf16)
    k_loc_v = kv_loc[0].rearrange("(kv hd t) -> kv hd t", kv=KVH, hd=HD)
    v_loc_v = kv_loc[1].rearrange("(t kv hd) -> t kv hd", kv=KVH, hd=HD)

    with tile.TileContext(nc) as tc, ExitStack() as es:
        # ------------------------------------------------------------ constants
        cpool = es.enter_context(tc.tile_pool(name="consts", bufs=1))
        ident = cpool.tile([128, 128], bf16)
        make_identity(nc, ident[:])
        ones_mat = cpool.tile([128, 128], bf16)
        nc.vector.memset(ones_mat[:], 1.0)
        eps_t = cpool.tile([128, 1], f32)
        nc.vector.memset(eps_t[:], EPS)
        oeps_t = cpool.tile([128, 1], f32)
        nc.vector.memset(oeps_t[:], float(OUT_SCALE) * EPS)
        cosq_sb = cpool.tile([128, 4, HD], f32)
        sinq_sb = cpool.tile([128, 4, HD], f32)
        cosk_sb = cpool.tile([128, 4, HD], f32)
        sink_sb = cpool.tile([128, 4, HD], f32)
        nc.sync.dma_start(out=cosq_sb[:], in_=cos_q[:, :, :])
        nc.sync.dma_start(out=sinq_sb[:], in_=sin_q[:, :, :])
        nc.sync.dma_start(out=cosk_sb[:], in_=cos_k[:, :, :])
        nc.sync.dma_start(out=sink_sb[:], in_=sin_k[:, :, :])
        masks_sb = cpool.tile([128, 8, 256], bf16, name="masks_sb")
        nc.sync.dma_start(out=masks_sb[:], in_=maskS[:, :, :])

        # ============================================================ helpers
        def ev(i):
            return nc.scalar if i % 2 == 0 else nc.vector

        def evac(engine, out, in_):
            if engine is nc.scalar:
                engine.copy(out=out, in_=in_)
            else:
                engine.tensor_copy(out=out, in_=in_)

        def transpose_in(x_dram, xT, col0, natp, ptp, eng=None):
            """Load natural [512, D] bf16 DRAM -> xT[:, k, col0+...] transposed."""
            for m in range(4):
                nat = natp.tile([128, D], bf16, tag="nat")
                nc.sync.dma_start(out=nat[:], in_=x_dram[128 * m:128 * m + 128, :])
                for k in range(16):
                    pst = ptp.tile([128, 128], bf16, tag="pst")
                    nc.tensor.transpose(pst[:], nat[:, 128 * k:128 * k + 128], ident[:])
                    evac(eng or ev(k), xT[:, k, col0 + 128 * m:col0 + 128 * m + 128],
                         pst[:])

        def rms_factors(x_ap, nh, smp):
            """Per-head rsqrt(mean(x^2 over HD) + EPS); returns ri [128, nh]."""
            sq = smp.tile([128, nh * HD], f32, tag="rsq")
            nc.vector.tensor_tensor(out=sq[:], in0=x_ap, in1=x_ap, op=ALU.mult)
            s2 = smp.tile([128, nh], f32, tag="rs2")
            nc.vector.tensor_reduce(out=s2[:],
                                    in_=sq[:].rearrange("p (h d) -> p h d", h=nh),
                                    axis=mybir.AxisListType.X, op=ALU.add)
            ln = smp.tile([128, nh], f32, tag="rln")
            nc.scalar.activation(out=ln[:], in_=s2[:], func=AF.Ln,
                                 bias=eps_t[:], scale=1.0 / HD)
            ri = smp.tile([128, nh], f32, tag="rri")
            nc.scalar.activation(out=ri[:], in_=ln[:], func=AF.Exp, scale=-0.5)
            return ri

        def rms_batch(x_ap, nh, smp, out_bf=None):
            """x *= rsqrt(mean(x^2 over HD) + EPS), batched over nh heads.
            x_ap [128, nh*HD] f32 AP; optionally write result to out_bf (bf16)."""
            x3 = x_ap.rearrange("p (h d) -> p h d", h=nh)
            ri = rms_factors(x_ap, nh, smp)
            dst = (out_bf.rearrange("p (h d) -> p h d", h=nh)
                   if out_bf is not None else x3)
            nc.vector.tensor_tensor(out=dst, in0=x3, in1=bc_free(ri[:], HD, 2),
                                    op=ALU.mult)

        def rope_batch(dst_bf, src, nh, cos_sb, sin_sb, m, smp, eng=None):
            """dst = rope(src) for nh heads at once; dst bf16 AP, src f32 AP."""
            eng = eng or nc.vector
            half = HD // 2
            cos_bc = bc_free(cos_sb[:, m, :], nh, 1)          # [128, nh, HD]
            sinlo_bc = bc_free(sin_sb[:, m, 0:half], nh, 1)   # [128, nh, half]
            sinhi_bc = bc_free(sin_sb[:, m, half:HD], nh, 1)
            s3 = src.rearrange("p (h d) -> p h d", h=nh)
            d3 = dst_bf.rearrange("p (h d) -> p h d", h=nh)
            t0 = smp.tile([128, nh * HD], f32, tag="ro0")
            t03 = t0[:].rearrange("p (h d) -> p h d", h=nh)
            eng.tensor_tensor(out=t03, in0=s3, in1=cos_bc, op=ALU.mult)
            t1 = smp.tile([128, nh * half], f32, tag="ro1")
            t13 = t1[:].rearrange("p (h d) -> p h d", h=nh)
            eng.tensor_tensor(out=t13, in0=s3[:, :, half:HD], in1=sinlo_bc,
                              op=ALU.mult)
            eng.tensor_tensor(out=d3[:, :, 0:half], in0=t03[:, :, 0:half],
                              in1=t13, op=ALU.subtract)
            eng.tensor_tensor(out=t13, in0=s3[:, :, 0:half], in1=sinhi_bc,
                              op=ALU.mult)
            eng.tensor_tensor(out=d3[:, :, half:HD], in0=t03[:, :, half:HD],
                              in1=t13, op=ALU.add)

        # ===================================================== phase 1: K / V
        stage_dmas = []
        p2x = es.enter_context(tc.tile_pool(name="p2x", bufs=1))
        xqT = p2x.tile([128, 16, QTOK], bf16, name="xqT")
        wq0p = es.enter_context(tc.tile_pool(name="wq0", bufs=1))
        wq0_t = [wq0p.tile([128, 2, 1024], bf16, name=f"wq0_{i}")
                 for i in range(4)]
        with tc.tile_pool(name="p1kvn", bufs=1) as kvnat:
            knat = [kvnat.tile([128, KVH * HD], f32, name=f"kn{m}")
                     for m in range(4)]
            vnat = [kvnat.tile([128, KVH * HD], f32, name=f"vn{m}") for m in range(4)]
            with tc.tile_pool(name="p1x", bufs=1) as p1x:
                xkT = p1x.tile([128, 16, KVTOK + 1], bf16, name="xkT")
                xvT = p1x.tile([128, 16, KVTOK + 1], bf16, name="xvT")
                with tc.tile_pool(name="p1nat", bufs=2) as natp, \
                     tc.tile_pool(name="p1pst", bufs=4, space="PSUM") as ptp:
                    # boundary token -> free position 0 of each k-chunk
                    nc.sync.dma_start(out=xkT[:, :, 0],
                                      in_=xkb[0].rearrange("(k p) -> p k", p=128))
                    nc.sync.dma_start(out=xvT[:, :, 0],
                                      in_=xvb[0].rearrange("(k p) -> p k", p=128))
                    transpose_in(xk_sh, xkT, 1, natp, ptp)
                    transpose_in(xv_sh, xvT, 1, natp, ptp)

                with tc.tile_pool(name="p1w", bufs=3) as wp, \
                     tc.tile_pool(name="p1ps", bufs=1, space="PSUM") as pskv:
                    psK = [pskv.tile([128, 512], f32, tag=f"pK{m}", name=f"pK{m}") for m in range(4)]
                    psV = [pskv.tile([128, 512], f32, tag=f"pV{m}", name=f"pV{m}") for m in range(4)]
                    for k2 in range(8):
                        wt = wp.tile([128, 2, 2048], bf16, tag="wkv")
                        nc.sync.dma_start(out=wt[:], in_=Wkv_t[k2])
                        for j in range(2):
                            k = 2 * k2 + j
                            w = wt[:, j, :]
                            for m in range(4):
                                n0, n1 = 1 + 128 * m, 129 + 128 * m   # normal
                                s0, s1 = 128 * m, 128 * m + 128       # shifted
                                nc.tensor.matmul(psK[m][:], xkT[:, k, n0:n1],
                                                 w[:, 0:512], start=(k == 0),
                                                 stop=False)
                                nc.tensor.matmul(psK[m][:], xkT[:, k, s0:s1],
                                                 w[:, 512:1024], start=False,
                                                 stop=(k == 15))
                                nc.tensor.matmul(psV[m][:], xvT[:, k, n0:n1],
                                                 w[:, 1024:1536], start=(k == 0),
                                                 stop=False)
                                nc.tensor.matmul(psV[m][:], xvT[:, k, s0:s1],
                                                 w[:, 1536:2048], start=False,
                                                 stop=(k == 15))
                    for m in range(4):
                        evac(ev(m), knat[m][:], psK[m][:])
                        evac(ev(m + 1), vnat[m][:], psV[m][:])

            # xq transposes here: tensor fills the p1-tail gap while the
            # vector engine does the K/V rms/rope below (evacs on scalar)
            with tc.tile_pool(name="p2nat", bufs=2) as natp, \
                 tc.tile_pool(name="p2pst", bufs=4, space="PSUM") as ptp:
                transpose_in(xq_sh, xqT, 0, natp, ptp, eng=nc.scalar)

            # prefetch the first Wq tiles ahead of the staging DMAs so the
            # Q matmuls are fed while the allgather occupies the queue
            for i in range(4):
                nc.sync.dma_start(out=wq0_t[i][:], in_=Wqg_t[0, i])

            with tc.tile_pool(name="p1pst2", bufs=2, space="PSUM") as ptp2, \
                 tc.tile_pool(name="p1sm", bufs=2) as smp, \
                 tc.tile_pool(name="p1st", bufs=3) as stp, \
                 tc.tile_pool(name="p1kt", bufs=1) as ktp:
                kT_full = ktp.tile([128, KVH, KVTOK], bf16, name="kT_full")
                for m in range(4):
                    # V: rms -> bf16, stage [t, kv, hd] (one fat DMA per m)
                    vout = stp.tile([128, KVH * HD], bf16, tag="vout")
                    rms_batch(vnat[m][:], KVH, smp, out_bf=vout[:])
                    d = nc.sync.dma_start(
                        out=v_loc_v[128 * m:128 * m + 128, :, :],
                        in_=vout[:].rearrange("p (kv hd) -> p kv hd", kv=KVH))
                    stage_dmas.append(d)
                    # K: rope raw (gpsimd) in parallel with rms factors
                    # (vector/scalar); per-head rms scaling commutes with rope
                    ri = rms_factors(knat[m][:], KVH, smp)
                    kror = stp.tile([128, KVH * HD], f32, tag="kror")
                    rope_batch(kror[:], knat[m][:], KVH, cosk_sb, sink_sb, m, smp,
                               eng=nc.gpsimd)
                    krot = stp.tile([128, KVH * HD], bf16, tag="krot")
                    nc.vector.tensor_tensor(
                        out=krot[:].rearrange("p (h d) -> p h d", h=KVH),
                        in0=kror[:].rearrange("p (h d) -> p h d", h=KVH),
                        in1=bc_free(ri[:], HD, 2), op=ALU.mult)
                    for kv in range(KVH):
                        pst = ptp2.tile([128, 128], bf16, tag="pst")
                        nc.tensor.transpose(pst[:], krot[:, 128 * kv:128 * kv + 128],
                                            ident[:])
                        evac(ev(kv), kT_full[:, kv, 128 * m:128 * m + 128], pst[:])
                for kv in range(KVH):
                    d = nc.sync.dma_start(out=k_loc_v[kv], in_=kT_full[:, kv, :])
                    stage_dmas.append(d)

        ag_k = nc.gpsimd.collective_compute(
            "AllGather", ALU.bypass,
            replica_groups=[[0, 1, 2, 3], [4, 5, 6, 7]],
            ins=[kv_loc[:]], outs=[kv_gath[:]])
        for d in stage_dmas:
            tile.add_dep_helper(ag_k.ins, d.ins, reason="stage before allgather")

        # ===================================================== phase 2: Q / G
        p_qT = es.enter_context(tc.tile_pool(name="ppqT", bufs=1))
        qT_sb = p_qT.tile([128, H, QTOK], bf16, name="qT_sb")
        p_gT = es.enter_context(tc.tile_pool(name="ppgT", bufs=1))
        gT_sb = p_gT.tile([128, H, QTOK], bf16, name="gT_sb")
        p_gn = es.enter_context(tc.tile_pool(name="ppgn", bufs=1))
        g_sb = [p_gn.tile([128, H * HD], bf16, name=f"g{m}") for m in range(4)]
        p_y = es.enter_context(tc.tile_pool(name="ppy", bufs=1))
        y_sb = p_y.tile([128, H, QTOK], bf16, name="y_sb")
        p_n = es.enter_context(tc.tile_pool(name="ppn", bufs=1))
        rbf_all = p_n.tile([128, H, QTOK], bf16, name="rbf_all")

        kload = []
        kvp = es.enter_context(tc.tile_pool(name="p3kv", bufs=2))

        def load_kv(kv):
            K_sb = kvp.tile([128, 4, 512], bf16, tag="K", name=f"K{kv}")
            V_sb = kvp.tile([128, NCH, 128], bf16, tag="V", name=f"V{kv}")
            for g in range(4):
                kg = kv_gath[g, 0].rearrange("(kv hd t) -> kv hd t",
                                             kv=KVH, hd=HD)
                vg = kv_gath[g, 1].rearrange("(t kv hd) -> t kv hd",
                                             kv=KVH, hd=HD)
                d = nc.sync.dma_start(out=K_sb[:, g, :], in_=kg[kv])
                kload.append(d)
                d = nc.sync.dma_start(
                    out=V_sb[:, 4 * g:4 * g + 4, :],
                    in_=vg[:, kv, :].rearrange("(c p) hd -> p c hd", p=128))
                kload.append(d)
            return K_sb, V_sb

        with tc.tile_pool(name="p2qn", bufs=1) as qnat, \
             tc.tile_pool(name="p2qr", bufs=1) as qrp:
            q_sb = [qnat.tile([128, H * HD], bf16, name=f"q{m}") for m in range(4)]
            qrot = [qrp.tile([128, H * HD], bf16, name=f"qr{m}") for m in range(4)]
            with tc.tile_pool(name="p2w", bufs=3) as wp, \
                 tc.tile_pool(name="p2ps", bufs=1, space="PSUM") as psqg, \
                 tc.tile_pool(name="p2sm", bufs=1) as smp:
                def qg_pass(n2, dsts):
                    ps = [psqg.tile([128, 512], f32, tag=f"pqg{i}",
                                    name=f"pqg{i}") for i in range(8)]
                    for k2 in range(8):
                        if n2 == 0 and k2 < 4:
                            wt = wq0_t[k2]
                        else:
                            wt = wp.tile([128, 2, 1024], bf16, tag="wqg")
                            nc.sync.dma_start(out=wt[:], in_=Wqg_t[n2, k2])
                        for j in range(2):
                            k = 2 * k2 + j
                            for m in range(4):
                                nc.tensor.matmul(ps[2 * m][:],
                                                 xqT[:, k, 128 * m:128 * m + 128],
                                                 wt[:, j, 0:512], start=(k == 0),
                                                 stop=(k == 15))
                                nc.tensor.matmul(ps[2 * m + 1][:],
                                                 xqT[:, k, 128 * m:128 * m + 128],
                                                 wt[:, j, 512:1024], start=(k == 0),
                                                 stop=(k == 15))
                    for m in range(4):
                        c0 = 1024 * (n2 % 2)
                        t = dsts[m]
                        evac(ev(m), t[:, c0:c0 + 512], ps[2 * m][:])
                        evac(ev(m + 1), t[:, c0 + 512:c0 + 1024], ps[2 * m + 1][:])

                qg_pass(0, q_sb)
                qg_pass(1, q_sb)
                # prefetch kv-head 0's K/V (DMAs land before the G weight
                # stream in queue order; gated on the allgather)
                kv_pre = load_kv(0)
                # q rms factors + raw rope + scaled mult; hidden under G matmuls
                for m in range(4):
                    for hf in range(2):
                        sl = slice(1024 * hf, 1024 * hf + 1024)
                        ri = rms_factors(q_sb[m][:, sl], 8, smp)
                        qror = smp.tile([128, 8 * HD], f32, tag="qror")
                        rope_batch(qror[:], q_sb[m][:, sl], 8, cosq_sb, sinq_sb,
                                   m, smp)
                        nc.vector.tensor_tensor(
                            out=qrot[m][:, sl].rearrange("p (h d) -> p h d", h=8),
                            in0=qror[:].rearrange("p (h d) -> p h d", h=8),
                            in1=bc_free(ri[:], HD, 2), op=ALU.mult)
                qg_pass(2, g_sb)
                qg_pass(3, g_sb)

            # transpose q and g (fills the tensor gap before attention)
            with tc.tile_pool(name="p2pst2", bufs=4, space="PSUM") as ptp2:
                for m in range(4):
                    for h in range(H):
                        pst = ptp2.tile([128, 128], bf16, tag="pst")
                        nc.tensor.transpose(pst[:],
                                            qrot[m][:, 128 * h:128 * h + 128],
                                            ident[:])
                        evac(ev(h), qT_sb[:, h, 128 * m:128 * m + 128], pst[:])
                for m in range(4):
                    for h in range(H):
                        pst = ptp2.tile([128, 128], bf16, tag="pst")
                        nc.tensor.transpose(pst[:],
                                            g_sb[m][:, 128 * h:128 * h + 128],
                                            ident[:])
                        evac(ev(h + 1), gT_sb[:, h, 128 * m:128 * m + 128], pst[:])

        # ==================================================== phase 3: attention
        with tc.tile_pool(name="p3pt", bufs=3) as ptq, \
             tc.tile_pool(name="p3ps", bufs=1, space="PSUM") as pss_p, \
             tc.tile_pool(name="p3py", bufs=1, space="PSUM") as psy_p, \
             tc.tile_pool(name="p3sm", bufs=4) as smp, \
             tc.tile_pool(name="p3nf", bufs=1) as nfp:
            norms_full = nfp.tile([128, H, QTOK], f32, name="norms_full")
            for kv in range(KVH):
                if kv == 0:
                    K_sb, V_sb = kv_pre
                else:
                    K_sb, V_sb = load_kv(kv)
                psy = [psy_p.tile([128, 512], f32, tag=f"psy{hi}",
                                   name=f"psy{hi}") for hi in range(4)]
                for c in range(NCH):
                    q0, n = (0, 512) if c < 8 else (256, 256)
                    Kc = K_sb[:, c // 4, 128 * (c % 4):128 * (c % 4) + 128]
                    pss = pss_p.tile([128, 4, 512], f32, tag="pss")
                    pt = ptq.tile([128, 4, 512], bf16, tag="pt")
                    for hi in range(4):
                        h = 4 * kv + hi
                        nc.tensor.matmul(pss[:, hi, q0:q0 + n], Kc,
                                         qT_sb[:, h, q0:q0 + n],
                                         start=True, stop=True)
                    # one batched exp + one batched mask multiply for all 4 h
                    nc.scalar.activation(out=pt[:, :, q0:q0 + n],
                                         in_=pss[:, :, q0:q0 + n],
                                         func=AF.Exp, scale=INV_SQRT_HD)
                    mcol = 0 if c < 8 else 256
                    nc.vector.tensor_tensor(
                        out=pt[:, :, mcol:mcol + 256],
                        in0=pt[:, :, mcol:mcol + 256],
                        in1=bc_free(masks_sb[:, c % 8, :], 4, 1),
                        op=ALU.mult)
                    for hi in range(4):
                        nc.tensor.matmul(psy[hi][:, q0:q0 + n], V_sb[:, c, :],
                                         pt[:, hi, q0:q0 + n],
                                         start=(c == 0), stop=(c == NCH - 1),
                                         skip_group_check=True)
                # evacuate y, collect squared norms
                psn = pss_p.tile([128, 4, 512], f32, tag="pss")
                for hi in range(4):
                    h = 4 * kv + hi
                    nc.vector.tensor_copy(out=y_sb[:, h, :], in_=psy[hi][:])
                    ysq = smp.tile([128, 512], bf16, tag="ysq")
                    nc.gpsimd.tensor_tensor(out=ysq[:], in0=y_sb[:, h, :],
                                            in1=y_sb[:, h, :], op=ALU.mult)
                    nc.tensor.matmul(psn[:, hi, :], ones_mat[:], ysq[:],
                                     start=True, stop=True)
                    evac(ev(hi), norms_full[:, h, :], psn[:, hi, :])
            # batched rsqrt of all norms (one Ln + one Exp, 128 lanes)
            nf_flat = norms_full[:].rearrange("p h q -> p (h q)")
            nc.scalar.activation(out=nf_flat, in_=nf_flat, func=AF.Ln)
            nc.scalar.activation(out=rbf_all[:].rearrange("p h q -> p (h q)"),
                                 in_=nf_flat, func=AF.Exp, scale=-0.5)

        # gating in place: y_sb = y * g * rsqrt(norm)  (bf16 out-proj lhsT)
        gTr_sb = y_sb
        with tc.tile_pool(name="p3gs", bufs=4) as gsp:
            for h in range(H):
                tmp = gsp.tile([128, 512], bf16, tag="gtmp")
                nc.vector.tensor_tensor(out=tmp[:], in0=y_sb[:, h, :],
                                        in1=gT_sb[:, h, :], op=ALU.mult)
                nc.gpsimd.tensor_tensor(out=gTr_sb[:, h, :], in0=tmp[:],
                                        in1=rbf_all[:, h, :], op=ALU.mult)
        for d in kload:
            tile.add_dep_helper(d.ins, ag_k.ins, reason="allgather before load")

        # ==================================================== phase 4: out proj
        with tc.tile_pool(name="p4w", bufs=5) as wp, \
             tc.tile_pool(name="p4o", bufs=1) as op_, \
             tc.tile_pool(name="p4ps", bufs=1, space="PSUM") as pso_p, \
             tc.tile_pool(name="p4sm", bufs=2) as smp:
            out_sb = [op_.tile([128, D], f32, name=f"o{m}") for m in range(4)]
            s2all = op_.tile([128, 4, 4], f32, name="s2all")
            for n in range(4):
                pso = [pso_p.tile([128, 512], f32, tag=f"po{m}", name=f"po{m}") for m in range(4)]
                for k4 in range(4):
                    wot = wp.tile([128, 4, 512], bf16, tag="wo")
                    nc.sync.dma_start(out=wot[:], in_=Wo_t[n, k4])
                    for j in range(4):
                        k = 4 * k4 + j
                        for m in range(4):
                            nc.tensor.matmul(pso[m][:],
                                             gTr_sb[:, k, 128 * m:128 * m + 128],
                                             wot[:, j, :], start=(k == 0),
                                             stop=(k == 15))
                for m in range(4):
                    seg = out_sb[m][:, 512 * n:512 * n + 512]
                    evac(ev(m + n), seg, pso[m][:])
                    # partial sum of squares (keeps the final rms off the tail)
                    sq2 = smp.tile([128, 512], f32, tag="osq")
                    nc.vector.tensor_tensor(out=sq2[:], in0=seg, in1=seg,
                                            op=ALU.mult)
                    nc.vector.tensor_reduce(out=s2all[:, m, n:n + 1], in_=sq2[:],
                                            axis=mybir.AxisListType.X, op=ALU.add)
            for m in range(4):
                s2 = smp.tile([128, 1], f32, tag="os2")
                nc.vector.tensor_reduce(out=s2[:], in_=s2all[:, m, :],
                                        axis=mybir.AxisListType.X, op=ALU.add)
                l2 = smp.tile([128, 1], f32, tag="oln")
                nc.scalar.activation(out=l2[:], in_=s2[:], func=AF.Ln,
                                     bias=oeps_t[:],
                                     scale=float(OUT_SCALE) / D)
                r2 = smp.tile([128, 1], f32, tag="ori")
                nc.scalar.activation(out=r2[:], in_=l2[:], func=AF.Exp, scale=-0.5)
                nc.vector.tensor_scalar_mul(out_sb[m][:], out_sb[m][:], r2[:])
                nc.sync.dma_start(out=out_y[128 * m:128 * m + 128, :],
                                  in_=out_sb[m][:])

    nc.compile()
    _CACHE["nc"] = nc
    return nc


def _host_inputs(xq, xk, xv, Wq, Wk, Wv, Wg, Wo, mix_k, mix_v):
    """Build the 8 per-core input maps."""
    import ml_dtypes
    f = np.float32
    bf = ml_dtypes.bfloat16
    xq = np.asarray(xq, f)
    xk = np.asarray(xk, f)
    xv = np.asarray(xv, f)
    Wq = np.asarray(Wq, f)
    Wk = np.asarray(Wk, f)
    Wv = np.asarray(Wv, f)
    Wg = np.asarray(Wg, f)
    Wo = np.asarray(Wo, f)
    mix_k = np.asarray(mix_k, f)
    mix_v = np.asarray(mix_v, f)

    Wk1 = (1.0 - mix_k)[:, None] * Wk
    Wk2 = mix_k[:, None] * Wk
    Wv1 = (1.0 - mix_v)[:, None] * Wv
    Wv2 = mix_v[:, None] * Wv

    # Wkv_t[k2][p][j] = row 256*k2+128*j+p of [Wk1|Wk2|Wv1|Wv2]
    Wkv = np.concatenate([Wk1, Wk2, Wv1, Wv2], axis=1)  # [2048, 2048]
    Wkv_t = np.ascontiguousarray(
        Wkv.reshape(8, 2, 128, 2048).transpose(0, 2, 1, 3).astype(bf))

    Wqg = np.concatenate([Wq, Wg], axis=1)  # [2048, 4096]
    # Wqg_t[n2][k2][p][j] = row 256*k2+128*j+p, cols 1024*n2..
    Wqg_t = np.ascontiguousarray(
        Wqg.reshape(8, 2, 128, 4, 1024).transpose(3, 0, 2, 1, 4).astype(bf))

    # Wo_t[n][k4][p][j] = row 512*k4+128*j+p, cols 512*n..
    Wo_t = np.ascontiguousarray(
        Wo.reshape(4, 4, 128, 4, 512).transpose(3, 0, 2, 1, 4).astype(bf))

    half = HD // 2
    inv_freq = 1.0 / (10000.0 ** (np.arange(half, dtype=np.float64) / half))
    ang = np.arange(T, dtype=np.float64)[:, None] * inv_freq[None, :]
    cos_t = np.concatenate([np.cos(ang), np.cos(ang)], axis=-1).astype(f)
    sin_t = np.concatenate([np.sin(ang), np.sin(ang)], axis=-1).astype(f)

    in_maps = []
    for c in range(NCORE):
        b, p = divmod(c, 4)
        rows_q = np.concatenate([np.arange(256 * p, 256 * p + 256),
                                 np.arange(1024 + 256 * p, 1024 + 256 * p + 256)])
        t0 = KVTOK * p
        rows_kv = np.arange(t0, t0 + KVTOK)

        xq_s = np.ascontiguousarray(xq[b, rows_q, :].astype(bf))
        xk_s = np.ascontiguousarray(xk[b, t0:t0 + KVTOK, :].astype(bf))
        xv_s = np.ascontiguousarray(xv[b, t0:t0 + KVTOK, :].astype(bf))
        xkb = np.zeros((1, D), f)
        xvb = np.zeros((1, D), f)
        if p > 0:
            xkb[0] = xk[b, t0 - 1, :]
            xvb[0] = xv[b, t0 - 1, :]

        # maskS[i][cc][j] = 1 iff kv token 128cc+i <= in-slot q token 256p+j
        ii = np.arange(128)[:, None]
        jj = np.arange(256)[None, :]
        mask = np.empty((8, 128, 256), f)
        for cc in range(8):
            mask[cc] = (128 * cc + ii <= 256 * p + jj).astype(f)
        mask = mask.transpose(1, 0, 2)  # partition-major [128, 8, 256]

        def pm(tab, rows):  # partition-major rope table [128, 4, HD]
            return np.ascontiguousarray(
                tab[rows].reshape(4, 128, HD).transpose(1, 0, 2))

        in_maps.append({
            "xq_sh": xq_s, "xk_sh": xk_s, "xv_sh": xv_s,
            "xkb": xkb.astype(bf), "xvb": xvb.astype(bf),
            "Wkv_t": Wkv_t, "Wqg_t": Wqg_t, "Wo_t": Wo_t,
            "cos_q": pm(cos_t, rows_q), "sin_q": pm(sin_t, rows_q),
            "cos_k": pm(cos_t, rows_kv), "sin_k": pm(sin_t, rows_kv),
            "maskS": np.ascontiguousarray(mask.astype(bf)),
        })
    return in_maps


def _run(in_maps, trace=False, tmpdir=None):
    _install_ntff_hook()
    from concourse.bass_utils import run_bass_kernel_spmd
    nc = _build()
    return run_bass_kernel_spmd(nc, in_maps, list(range(NCORE)),
                                trace=trace, tmpdir=tmpdir)


def kernel(xq, xk, xv, Wq, Wk, Wv, Wg, Wo, mix_k, mix_v,
           _trace=False, _tmpdir=None):
    in_maps = _host_inputs(xq, xk, xv, Wq, Wk, Wv, Wg, Wo, mix_k, mix_v)
    res = _run(in_maps, trace=_trace, tmpdir=_tmpdir)
    out = np.empty((B, T, D), np.float32)
    for c in range(NCORE):
        b, p = divmod(c, 4)
        y = res.results[c]["out_y"]
        out[b, 256 * p:256 * p + 256, :] = y[:256]
        out[b, 1024 + 256 * p:1024 + 256 * p + 256, :] = y[256:]
    kernel._last_exec_ns = res.exec_time_ns
    return out


# revision 52
# speedup vs baseline: 1.6452x; 1.0833x over previous
"""Trainium2 Bass kernel for nn_AttentionSubLayer (dense transformer attention
sublayer with time-lerp K/V mixing, QK-norm, RoPE, GQA, per-head l2 output
norm, gating, out-proj + final RMS norm).

Sharding: 8 cores = 2 batch groups x 4-way sequence parallel.  Core c
handles batch c//4; within the group (p = c%4) it owns q slots
slot0 = tokens [256p, 256p+256) and slot1 = [1024+256p, 1024+256p+256),
so slot0 only ever attends to kv tokens < 1024 and slot1 to all 2048.
K/V projections are computed on the owning quarter [512p, 512p+512) and
AllGathered (bf16) within each 4-core batch group.  Out-proj and final
RMS norm are local.

Numerics: bf16 matmul operands everywhere (fp32 PSUM), fp32 vector math
for the norms/rope.  Weights are pre-tiled on the host into contiguous
[128, n] k-chunk blocks so every weight DMA is one fat transfer.
Softmax skips max-subtraction (scores bounded by sqrt(HD) after QK
rms-norm) and the denominator (cancelled by the per-head l2 norm).
Causal masking is a 0/1 bf16 multiply on the exp output; the scalar
engine runs Exp only in attention (the l2-norm rsqrt is one batched
Ln+Exp at the end).
"""

import math
import sys
import types
from contextlib import ExitStack

sys.path.insert(0, "/opt/trn_rl_repo")

import numpy as np

# ---------------------------------------------------------------- problem dims
B, T, D, H, KVH, HD = 2, 2048, 2048, 16, 4, 128
N_LAYER = 24
EPS = 1e-8
NCORE = 8
QTOK = 512        # q tokens per core (2 slots x 256)
KVTOK = 512       # kv tokens per core (contiguous quarter)
NCH = 16          # kv chunks of 128 tokens (full 2048)
INV_SQRT_HD = 1.0 / math.sqrt(HD)
OUT_SCALE = 2 * N_LAYER  # final rms divided by sqrt(2*N_LAYER)


def _install_ntff_hook():
    try:
        import antenv
        if "antenv.axon_hooks" in sys.modules:
            return
        from trn_agent_boot.trn_boot import _ntff_profile_via_ctypes
        hook = _ntff_profile_via_ctypes("/opt/axon/libaxon_pjrt.so")
        mod = types.ModuleType("antenv.axon_hooks")
        mod.get_axon_ntff_profile_hook = lambda: hook
        antenv.axon_hooks = mod
        sys.modules["antenv.axon_hooks"] = mod
    except Exception:
        pass


_CACHE = {}


def _build():
    if "nc" in _CACHE:
        return _CACHE["nc"]

    import concourse.bass as bass
    import concourse.mybir as mybir
    import concourse.tile as tile
    from concourse import bacc
    from concourse.masks import make_identity

    f32 = mybir.dt.float32
    bf16 = mybir.dt.bfloat16
    AF = mybir.ActivationFunctionType
    ALU = mybir.AluOpType

    def bc_free(ap, n, at):
        """Insert a broadcast (stride-0) free dim of size n at position `at`
        of the AP's dim list (position counted incl. partition dim 0)."""
        new = list(list(d) for d in ap.ap)
        new.insert(at, [0, n])
        return bass.AP(tensor=ap.tensor, offset=ap.offset, ap=new)

    nc = bacc.Bacc("TRN2", target_bir_lowering=False, debug=False,
                   num_devices=NCORE)

    # ------------------------------------------------------------- I/O tensors
    xq_sh = nc.dram_tensor("xq_sh", [QTOK, D], bf16, kind="ExternalInput")
    xk_sh = nc.dram_tensor("xk_sh", [KVTOK, D], bf16, kind="ExternalInput")
    xv_sh = nc.dram_tensor("xv_sh", [KVTOK, D], bf16, kind="ExternalInput")
    xkb = nc.dram_tensor("xkb", [1, D], bf16, kind="ExternalInput")
    xvb = nc.dram_tensor("xvb", [1, D], bf16, kind="ExternalInput")
    # pre-tiled weights (host layout, all bf16, >=4KB per partition per DMA):
    #  Wkv_t[k2][p][j] = row 256*k2+128*j+p of [Wk1 | Wk2 | Wv1 | Wv2]
    Wkv_t = nc.dram_tensor("Wkv_t", [8, 128, 2, 2048], bf16, kind="ExternalInput")
    #  Wqg_t[n2][k2][p][j] = row 256*k2+128*j+p, cols 1024*n2.. of [Wq | Wg]
    Wqg_t = nc.dram_tensor("Wqg_t", [4, 8, 128, 2, 1024], bf16,
                           kind="ExternalInput")
    #  Wo_t[n][k4][p][j] = row 512*k4+128*j+p, cols 512*n.. of Wo
    Wo_t = nc.dram_tensor("Wo_t", [4, 4, 128, 4, 512], bf16,
                          kind="ExternalInput")
    # partition-major rope tables: [p][m][hd] = table[rows[128*m+p]][hd]
    cos_q = nc.dram_tensor("cos_q", [128, 4, HD], f32, kind="ExternalInput")
    sin_q = nc.dram_tensor("sin_q", [128, 4, HD], f32, kind="ExternalInput")
    cos_k = nc.dram_tensor("cos_k", [128, 4, HD], f32, kind="ExternalInput")
    sin_k = nc.dram_tensor("sin_k", [128, 4, HD], f32, kind="ExternalInput")
    # maskS[p][c] = 0/1 validity row p of kv chunk c vs 256 in-slot q cols
    maskS = nc.dram_tensor("maskS", [128, 8, 256], bf16, kind="ExternalInput")
    out_y = nc.dram_tensor("out_y", [QTOK, D], f32, kind="ExternalOutput")

    # staging for K/V allgather (within 4-core batch group)
    SHARD = KVH * HD * KVTOK
    kv_loc = nc.dram_tensor("kv_loc", [2, SHARD], bf16)
    kv_gath = nc.dram_tensor("kv_gath", [4, 2, SHARD], bf16)
    k_loc_v = kv_loc[0].rearrange("(kv hd t) -> kv hd t", kv=KVH, hd=HD)
    v_loc_v = kv_loc[1].rearrange("(t kv hd) -> t kv hd", kv=KVH, hd=HD)

    with tile.TileContext(nc) as tc, ExitStack() as es:
        # ------------------------------------------------------------ constants
        cpool = es.enter_context(tc.tile_pool(name="consts", bufs=1))
        ident = cpool.tile([128, 128], bf16)
        make_identity(nc, ident[:])
        ones_mat = cpool.tile([128, 128], bf16)
        nc.vector.memset(ones_mat[:], 1.0)
        eps_t = cpool.tile([128, 1], f32)
        nc.vector.memset(eps_t[:], EPS)
        oeps_t = cpool.tile([128, 1], f32)
        nc.vector.memset(oeps_t[:], float(OUT_SCALE) * EPS)
        cosq_sb = cpool.tile([128, 4, HD], f32)
        sinq_sb = cpool.tile([128, 4, HD], f32)
        cosk_sb = cpool.tile([128, 4, HD], f32)
        sink_sb = cpool.tile([128, 4, HD], f32)
        nc.sync.dma_start(out=cosq_sb[:], in_=cos_q[:, :, :])
        nc.sync.dma_start(out=sinq_sb[:], in_=sin_q[:, :, :])
        nc.sync.dma_start(out=cosk_sb[:], in_=cos_k[:, :, :])
        nc.sync.dma_start(out=sink_sb[:], in_=sin_k[:, :, :])
        masks_sb = cpool.tile([128, 8, 256], bf16, name="masks_sb")
        nc.sync.dma_start(out=masks_sb[:], in_=maskS[:, :, :])

        # ============================================================ helpers
        def ev(i):
            return nc.scalar if i % 2 == 0 else nc.vector

        def evac(engine, out, in_):
            if engine is nc.scalar:
                engine.copy(out=out, in_=in_)
            else:
                engine.tensor_copy(out=out, in_=in_)

        def transpose_in(x_dram, xT, col0, natp, ptp, eng=None):
            """Load natural [512, D] bf16 DRAM -> xT[:, k, col0+...] transposed."""
            for m in range(4):
                nat = natp.tile([128, D], bf16, tag="nat")
                nc.sync.dma_start(out=nat[:], in_=x_dram[128 * m:128 * m + 128, :])
                for k in range(16):
                    pst = ptp.tile([128, 128], bf16, tag="pst")
                    nc.tensor.transpose(pst[:], nat[:, 128 * k:128 * k + 128], ident[:])
                    evac(eng or ev(k), xT[:, k, col0 + 128 * m:col0 + 128 * m + 128],
                         pst[:])

        def rms_factors(x_ap, nh, smp):
            """Per-head rsqrt(mean(x^2 over HD) + EPS); returns ri [128, nh]."""
            sq = smp.tile([128, nh * HD], f32, tag="rsq")
            nc.vector.tensor_tensor(out=sq[:], in0=x_ap, in1=x_ap, op=ALU.mult)
            s2 = smp.tile([128, nh], f32, tag="rs2")
            nc.vector.tensor_reduce(out=s2[:],
                                    in_=sq[:].rearrange("p (h d) -> p h d", h=nh),
                                    axis=mybir.AxisListType.X, op=ALU.add)
            ln = smp.tile([128, nh], f32, tag="rln")
            nc.scalar.activation(out=ln[:], in_=s2[:], func=AF.Ln,
                                 bias=eps_t[:], scale=1.0 / HD)
            ri = smp.tile([128, nh], f32, tag="rri")
            nc.scalar.activation(out=ri[:], in_=ln[:], func=AF.Exp, scale=-0.5)
            return ri

        def rms_batch(x_ap, nh, smp, out_bf=None):
            """x *= rsqrt(mean(x^2 over HD) + EPS), batched over nh heads.
            x_ap [128, nh*HD] f32 AP; optionally write result to out_bf (bf16)."""
            x3 = x_ap.rearrange("p (h d) -> p h d", h=nh)
            ri = rms_factors(x_ap, nh, smp)
            dst = (out_bf.rearrange("p (h d) -> p h d", h=nh)
                   if out_bf is not None else x3)
            nc.vector.tensor_tensor(out=dst, in0=x3, in1=bc_free(ri[:], HD, 2),
                                    op=ALU.mult)

        def rope_batch(dst_bf, src, nh, cos_sb, sin_sb, m, smp, eng=None):
            """dst = rope(src) for nh heads at once; dst bf16 AP, src f32 AP."""
            eng = eng or nc.vector
            half = HD // 2
            cos_bc = bc_free(cos_sb[:, m, :], nh, 1)          # [128, nh, HD]
            sinlo_bc = bc_free(sin_sb[:, m, 0:half], nh, 1)   # [128, nh, half]
            sinhi_bc = bc_free(sin_sb[:, m, half:HD], nh, 1)
            s3 = src.rearrange("p (h d) -> p h d", h=nh)
            d3 = dst_bf.rearrange("p (h d) -> p h d", h=nh)
            t0 = smp.tile([128, nh * HD], f32, tag="ro0")
            t03 = t0[:].rearrange("p (h d) -> p h d", h=nh)
            eng.tensor_tensor(out=t03, in0=s3, in1=cos_bc, op=ALU.mult)
            t1 = smp.tile([128, nh * half], f32, tag="ro1")
            t13 = t1[:].rearrange("p (h d) -> p h d", h=nh)
            eng.tensor_tensor(out=t13, in0=s3[:, :, half:HD], in1=sinlo_bc,
                              op=ALU.mult)
            eng.tensor_tensor(out=d3[:, :, 0:half], in0=t03[:, :, 0:half],
                              in1=t13, op=ALU.subtract)
            eng.tensor_tensor(out=t13, in0=s3[:, :, 0:half], in1=sinhi_bc,
                              op=ALU.mult)
            eng.tensor_tensor(out=d3[:, :, half:HD], in0=t03[:, :, half:HD],
                              in1=t13, op=ALU.add)

        # ===================================================== phase 1: K / V
        stage_dmas = []
        p2x = es.enter_context(tc.tile_pool(name="p2x", bufs=1))
        xqT = p2x.tile([128, 16, QTOK], bf16, name="xqT")
        wq0p = es.enter_context(tc.tile_pool(name="wq0", bufs=1))
        wq0_t = [wq0p.tile([128, 2, 1024], bf16, name=f"wq0_{i}")
                 for i in range(4)]
        with tc.tile_pool(name="p1kvn", bufs=1) as kvnat:
            knat = [kvnat.tile([128, KVH * HD], f32, name=f"kn{m}")
                     for m in range(4)]
            vnat = [kvnat.tile([128, KVH * HD], f32, name=f"vn{m}") for m in range(4)]
            with tc.tile_pool(name="p1x", bufs=1) as p1x:
                xkT = p1x.tile([128, 16, KVTOK + 1], bf16, name="xkT")
                xvT = p1x.tile([128, 16, KVTOK + 1], bf16, name="xvT")
                with tc.tile_pool(name="p1nat", bufs=2) as natp, \
                     tc.tile_pool(name="p1pst", bufs=4, space="PSUM") as ptp:
                    # boundary token -> free position 0 of each k-chunk
                    nc.sync.dma_start(out=xkT[:, :, 0],
                                      in_=xkb[0].rearrange("(k p) -> p k", p=128))
                    nc.sync.dma_start(out=xvT[:, :, 0],
                                      in_=xvb[0].rearrange("(k p) -> p k", p=128))
                    transpose_in(xk_sh, xkT, 1, natp, ptp)
                    transpose_in(xv_sh, xvT, 1, natp, ptp)

                with tc.tile_pool(name="p1w", bufs=3) as wp, \
                     tc.tile_pool(name="p1ps", bufs=1, space="PSUM") as pskv:
                    psK = [pskv.tile([128, 512], f32, tag=f"pK{m}", name=f"pK{m}") for m in range(4)]
                    psV = [pskv.tile([128, 512], f32, tag=f"pV{m}", name=f"pV{m}") for m in range(4)]
                    for k2 in range(8):
                        wt = wp.tile([128, 2, 2048], bf16, tag="wkv")
                        nc.sync.dma_start(out=wt[:], in_=Wkv_t[k2])
                        for j in range(2):
                            k = 2 * k2 + j
                            w = wt[:, j, :]
                            for m in range(4):
                                n0, n1 = 1 + 128 * m, 129 + 128 * m   # normal
                                s0, s1 = 128 * m, 128 * m + 128       # shifted
                                nc.tensor.matmul(psK[m][:], xkT[:, k, n0:n1],
                                                 w[:, 0:512], start=(k == 0),
                                                 stop=False)
                                nc.tensor.matmul(psK[m][:], xkT[:, k, s0:s1],
                                                 w[:, 512:1024], start=False,
                                                 stop=(k == 15))
                                nc.tensor.matmul(psV[m][:], xvT[:, k, n0:n1],
                                                 w[:, 1024:1536], start=(k == 0),
                                                 stop=False)
                                nc.tensor.matmul(psV[m][:], xvT[:, k, s0:s1],
                                                 w[:, 1536:2048], start=False,
                                                 stop=(k == 15))
                    for m in range(4):
                        evac(ev(m), knat[m][:], psK[m][:])
                        evac(ev(m + 1), vnat[m][:], psV[m][:])

            # xq transposes here: tensor fills the p1-tail gap while the
            # vector engine does the K/V rms/rope below (evacs on scalar)
            with tc.tile_pool(name="p2nat", bufs=2) as natp, \
                 tc.tile_pool(name="p2pst", bufs=4, space="PSUM") as ptp:
                transpose_in(xq_sh, xqT, 0, natp, ptp, eng=nc.scalar)

            # prefetch the first Wq tiles ahead of the staging DMAs so the
            # Q matmuls are fed while the allgather occupies the queue
            for i in range(4):
                nc.sync.dma_start(out=wq0_t[i][:], in_=Wqg_t[0, i])

            with tc.tile_pool(name="p1pst2", bufs=2, space="PSUM") as ptp2, \
                 tc.tile_pool(name="p1sm", bufs=2) as smp, \
                 tc.tile_pool(name="p1st", bufs=3) as stp, \
                 tc.tile_pool(name="p1kt", bufs=1) as ktp:
                kT_full = ktp.tile([128, KVH, KVTOK], bf16, name="kT_full")
                for m in range(4):
                    # V: rms -> bf16, stage [t, kv, hd] (one fat DMA per m)
                    vout = stp.tile([128, KVH * HD], bf16, tag="vout")
                    rms_batch(vnat[m][:], KVH, smp, out_bf=vout[:])
                    d = nc.sync.dma_start(
                        out=v_loc_v[128 * m:128 * m + 128, :, :],
                        in_=vout[:].rearrange("p (kv hd) -> p kv hd", kv=KVH))
                    stage_dmas.append(d)
                    # K: rope raw (gpsimd) in parallel with rms factors
                    # (vector/scalar); per-head rms scaling commutes with rope
                    ri = rms_factors(knat[m][:], KVH, smp)
                    kror = stp.tile([128, KVH * HD], f32, tag="kror")
                    rope_batch(kror[:], knat[m][:], KVH, cosk_sb, sink_sb, m, smp,
                               eng=nc.gpsimd)
                    krot = stp.tile([128, KVH * HD], bf16, tag="krot")
                    nc.vector.tensor_tensor(
                        out=krot[:].rearrange("p (h d) -> p h d", h=KVH),
                        in0=kror[:].rearrange("p (h d) -> p h d", h=KVH),
                        in1=bc_free(ri[:], HD, 2), op=ALU.mult)
                    for kv in range(KVH):
                        pst = ptp2.tile([128, 128], bf16, tag="pst")
                        nc.tensor.transpose(pst[:], krot[:, 128 * kv:128 * kv + 128],
                                            ident[:])
                        evac(ev(kv), kT_full[:, kv, 128 * m:128 * m + 128], pst[:])
                for kv in range(KVH):
                    d = nc.sync.dma_start(out=k_loc_v[kv], in_=kT_full[:, kv, :])
                    stage_dmas.append(d)

        ag_k = nc.gpsimd.collective_compute(
            "AllGather", ALU.bypass,
            replica_groups=[[0, 1, 2, 3], [4, 5, 6, 7]],
            ins=[kv_loc[:]], outs=[kv_gath[:]])
        for d in stage_dmas:
            tile.add_dep_helper(ag_k.ins, d.ins, reason="stage before allgather")

        # ===================================================== phase 2: Q / G
        p_qT = es.enter_context(tc.tile_pool(name="ppqT", bufs=1))
        qT_sb = p_qT.tile([128, H, QTOK], bf16, name="qT_sb")
        p_gT = es.enter_context(tc.tile_pool(name="ppgT", bufs=1))
        gT_sb = p_gT.tile([128, H, QTOK], bf16, name="gT_sb")
        p_gn = es.enter_context(tc.tile_pool(name="ppgn", bufs=1))
        g_sb = [p_gn.tile([128, H * HD], bf16, name=f"g{m}") for m in range(4)]
        p_y = es.enter_context(tc.tile_pool(name="ppy", bufs=1))
        y_sb = p_y.tile([128, H, QTOK], bf16, name="y_sb")
        p_n = es.enter_context(tc.tile_pool(name="ppn", bufs=1))
        rbf_all = p_n.tile([128, H, QTOK], bf16, name="rbf_all")

        kload = []
        kvp = es.enter_context(tc.tile_pool(name="p3kv", bufs=2))

        def load_kv(kv):
            K_sb = kvp.tile([128, 4, 512], bf16, tag="K", name=f"K{kv}")
            V_sb = kvp.tile([128, NCH, 128], bf16, tag="V", name=f"V{kv}")
            for g in range(4):
                kg = kv_gath[g, 0].rearrange("(kv hd t) -> kv hd t",
                                             kv=KVH, hd=HD)
                vg = kv_gath[g, 1].rearrange("(t kv hd) -> t kv hd",
                                             kv=KVH, hd=HD)
                d = nc.sync.dma_start(out=K_sb[:, g, :], in_=kg[kv])
                kload.append(d)
                d = nc.sync.dma_start(
                    out=V_sb[:, 4 * g:4 * g + 4, :],
                    in_=vg[:, kv, :].rearrange("(c p) hd -> p c hd", p=128))
                kload.append(d)
            return K_sb, V_sb

        with tc.tile_pool(name="p2qn", bufs=1) as qnat, \
             tc.tile_pool(name="p2qr", bufs=1) as qrp:
            q_sb = [qnat.tile([128, H * HD], bf16, name=f"q{m}") for m in range(4)]
            qrot = [qrp.tile([128, H * HD], bf16, name=f"qr{m}") for m in range(4)]
            with tc.tile_pool(name="p2w", bufs=3) as wp, \
                 tc.tile_pool(name="p2ps", bufs=1, space="PSUM") as psqg, \
                 tc.tile_pool(name="p2sm", bufs=1) as smp:
                def qg_pass(n2, dsts):
                    ps = [psqg.tile([128, 512], f32, tag=f"pqg{i}",
                                    name=f"pqg{i}") for i in range(8)]
                    for k2 in range(8):
                        if n2 == 0 and k2 < 4:
                            wt = wq0_t[k2]
                        else:
                            wt = wp.tile([128, 2, 1024], bf16, tag="wqg")
                            nc.sync.dma_start(out=wt[:], in_=Wqg_t[n2, k2])
                        for j in range(2):
                            k = 2 * k2 + j
                            for m in range(4):
                                nc.tensor.matmul(ps[2 * m][:],
                                                 xqT[:, k, 128 * m:128 * m + 128],
                                                 wt[:, j, 0:512], start=(k == 0),
                                                 stop=(k == 15))
                                nc.tensor.matmul(ps[2 * m + 1][:],
                                                 xqT[:, k, 128 * m:128 * m + 128],
                                                 wt[:, j, 512:1024], start=(k == 0),
                                                 stop=(k == 15))
                    for m in range(4):
                        c0 = 1024 * (n2 % 2)
                        t = dsts[m]
                        evac(ev(m), t[:, c0:c0 + 512], ps[2 * m][:])
                        evac(ev(m + 1), t[:, c0 + 512:c0 + 1024], ps[2 * m + 1][:])

                qg_pass(0, q_sb)
                qg_pass(1, q_sb)
                # prefetch kv-head 0's K/V (DMAs land before the G weight
                # stream in queue order; gated on the allgather)
                kv_pre = load_kv(0)
                # q rms factors + raw rope + scaled mult; hidden under G matmuls
                for m in range(4):
                    for hf in range(2):
                        sl = slice(1024 * hf, 1024 * hf + 1024)
                        ri = rms_factors(q_sb[m][:, sl], 8, smp)
                        qror = smp.tile([128, 8 * HD], f32, tag="qror")
                        rope_batch(qror[:], q_sb[m][:, sl], 8, cosq_sb, sinq_sb,
                                   m, smp)
                        nc.vector.tensor_tensor(
                            out=qrot[m][:, sl].rearrange("p (h d) -> p h d", h=8),
                            in0=qror[:].rearrange("p (h d) -> p h d", h=8),
                            in1=bc_free(ri[:], HD, 2), op=ALU.mult)
                qg_pass(2, g_sb)
                qg_pass(3, g_sb)

            # transpose q and g (fills the tensor gap before attention)
            with tc.tile_pool(name="p2pst2", bufs=4, space="PSUM") as ptp2:
                for m in range(4):
                    for h in range(H):
                        pst = ptp2.tile([128, 128], bf16, tag="pst")
                        nc.tensor.transpose(pst[:],
                                            qrot[m][:, 128 * h:128 * h + 128],
                                            ident[:])
                        evac(ev(h), qT_sb[:, h, 128 * m:128 * m + 128], pst[:])
                for m in range(4):
                    for h in range(H):
                        pst = ptp2.tile([128, 128], bf16, tag="pst")
                        nc.tensor.transpose(pst[:],
                                            g_sb[m][:, 128 * h:128 * h + 128],
                                            ident[:])
                        evac(ev(h + 1), gT_sb[:, h, 128 * m:128 * m + 128], pst[:])

        # ==================================================== phase 3: attention
        gTr_sb = y_sb   # gating writes in place

        with tc.tile_pool(name="p3pt", bufs=3) as ptq, \
             tc.tile_pool(name="p3ps", bufs=2, space="PSUM") as pss_p, \
             tc.tile_pool(name="p3py", bufs=1, space="PSUM") as psy_p, \
             tc.tile_pool(name="p3sm", bufs=4) as smp, \
             tc.tile_pool(name="p3nf", bufs=1) as nfp:
            norms_full = nfp.tile([128, H, QTOK], f32, name="norms_full")

            def gate(h, smp):
                # y_sb[h] *= g * rsqrt(norm)   (bf16 out-proj lhsT)
                tmp = smp.tile([128, 512], bf16, tag="gtmp")
                nc.vector.tensor_tensor(out=tmp[:], in0=y_sb[:, h, :],
                                        in1=gT_sb[:, h, :], op=ALU.mult)
                nc.gpsimd.tensor_tensor(out=gTr_sb[:, h, :], in0=tmp[:],
                                        in1=rbf_all[:, h, :], op=ALU.mult)
            for kv in range(KVH):
                if kv == 0:
                    K_sb, V_sb = kv_pre
                else:
                    K_sb, V_sb = load_kv(kv)
                for hp in range(2):      # head pairs: double-buffered scores
                    h0 = 4 * kv + 2 * hp
                    psy = psy_p.tile([128, 2, 512], f32, tag=f"psy{hp}",
                                     name=f"psy{kv}_{hp}")
                    for c in range(NCH):
                        q0, n = (0, 512) if c < 8 else (256, 256)
                        Kc = K_sb[:, c // 4, 128 * (c % 4):128 * (c % 4) + 128]
                        pss = pss_p.tile([128, 2, 512], f32, tag="pss")
                        pt = ptq.tile([128, 2, 512], bf16, tag="pt")
                        for hi in range(2):
                            nc.tensor.matmul(pss[:, hi, q0:q0 + n], Kc,
                                             qT_sb[:, h0 + hi, q0:q0 + n],
                                             start=True, stop=True)
                        nc.scalar.activation(out=pt[:, :, q0:q0 + n],
                                             in_=pss[:, :, q0:q0 + n],
                                             func=AF.Exp, scale=INV_SQRT_HD)
                        mcol = 0 if c < 8 else 256
                        nc.vector.tensor_tensor(
                            out=pt[:, :, mcol:mcol + 256],
                            in0=pt[:, :, mcol:mcol + 256],
                            in1=bc_free(masks_sb[:, c % 8, :], 2, 1),
                            op=ALU.mult)
                        for hi in range(2):
                            nc.tensor.matmul(psy[:, hi, q0:q0 + n], V_sb[:, c, :],
                                             pt[:, hi, q0:q0 + n],
                                             start=(c == 0), stop=(c == NCH - 1),
                                             skip_group_check=True)
                    # evacuate y, collect squared norms
                    psn = pss_p.tile([128, 2, 512], f32, tag="pss")
                    for hi in range(2):
                        h = h0 + hi
                        nc.vector.tensor_copy(out=y_sb[:, h, :], in_=psy[:, hi, :])
                        ysq = smp.tile([128, 512], bf16, tag="ysq")
                        nc.gpsimd.tensor_tensor(out=ysq[:], in0=y_sb[:, h, :],
                                                in1=y_sb[:, h, :], op=ALU.mult)
                        nc.tensor.matmul(psn[:, hi, :], ones_mat[:], ysq[:],
                                         start=True, stop=True)
                        evac(ev(hi), norms_full[:, h, :], psn[:, hi, :])
                if kv == 1:
                    # rsqrt + gating for heads 0..7 while kv 2/3 still compute
                    nf0 = norms_full[:, 0:8, :].rearrange("p h q -> p (h q)")
                    nc.scalar.activation(out=nf0, in_=nf0, func=AF.Ln)
                    nc.scalar.activation(
                        out=rbf_all[:, 0:8, :].rearrange("p h q -> p (h q)"),
                        in_=nf0, func=AF.Exp, scale=-0.5)
                    for h in range(8):
                        gate(h, smp)
            nf1 = norms_full[:, 8:16, :].rearrange("p h q -> p (h q)")
            nc.scalar.activation(out=nf1, in_=nf1, func=AF.Ln)
            nc.scalar.activation(
                out=rbf_all[:, 8:16, :].rearrange("p h q -> p (h q)"),
                in_=nf1, func=AF.Exp, scale=-0.5)
            for h in range(8, 16):
                gate(h, smp)

        for d in kload:
            tile.add_dep_helper(d.ins, ag_k.ins, reason="allgather before load")

        # ==================================================== phase 4: out proj
        with tc.tile_pool(name="p4w", bufs=5) as wp, \
             tc.tile_pool(name="p4o", bufs=1) as op_, \
             tc.tile_pool(name="p4ps", bufs=1, space="PSUM") as pso_p, \
             tc.tile_pool(name="p4sm", bufs=2) as smp:
            out_sb = [op_.tile([128, D], f32, name=f"o{m}") for m in range(4)]
            s2all = op_.tile([128, 4, 4], f32, name="s2all")
            for n in range(4):
                pso = [pso_p.tile([128, 512], f32, tag=f"po{m}", name=f"po{m}") for m in range(4)]
                for k4 in range(4):
                    wot = wp.tile([128, 4, 512], bf16, tag="wo")
                    nc.sync.dma_start(out=wot[:], in_=Wo_t[n, k4])
                    for j in range(4):
                        k = 4 * k4 + j
                        for m in range(4):
                            nc.tensor.matmul(pso[m][:],
                                             gTr_sb[:, k, 128 * m:128 * m + 128],
                                             wot[:, j, :], start=(k == 0),
                                             stop=(k == 15))
                for m in range(4):
                    seg = out_sb[m][:, 512 * n:512 * n + 512]
                    evac(ev(m + n), seg, pso[m][:])
                    # partial sum of squares (keeps the final rms off the tail)
                    sq2 = smp.tile([128, 512], f32, tag="osq")
                    nc.vector.tensor_tensor(out=sq2[:], in0=seg, in1=seg,
                                            op=ALU.mult)
                    nc.vector.tensor_reduce(out=s2all[:, m, n:n + 1], in_=sq2[:],
                                            axis=mybir.AxisListType.X, op=ALU.add)
            for m in range(4):
                s2 = smp.tile([128, 1], f32, tag="os2")
                nc.vector.tensor_reduce(out=s2[:], in_=s2all[:, m, :],
                                        axis=mybir.AxisListType.X, op=ALU.add)
                l2 = smp.tile([128, 1], f32, tag="oln")
                nc.scalar.activation(out=l2[:], in_=s2[:], func=AF.Ln,
                                     bias=oeps_t[:],
                                     scale=float(OUT_SCALE) / D)
                r2 = smp.tile([128, 1], f32, tag="ori")
                nc.scalar.activation(out=r2[:], in_=l2[:], func=AF.Exp, scale=-0.5)
                nc.vector.tensor_scalar_mul(out_sb[m][:], out_sb[m][:], r2[:])
                nc.sync.dma_start(out=out_y[128 * m:128 * m + 128, :],
                                  in_=out_sb[m][:])

    nc.compile()
    _CACHE["nc"] = nc
    return nc


def _host_inputs(xq, xk, xv, Wq, Wk, Wv, Wg, Wo, mix_k, mix_v):
    """Build the 8 per-core input maps."""
    import ml_dtypes
    f = np.float32
    bf = ml_dtypes.bfloat16
    xq = np.asarray(xq, f)
    xk = np.asarray(xk, f)
    xv = np.asarray(xv, f)
    Wq = np.asarray(Wq, f)
    Wk = np.asarray(Wk, f)
    Wv = np.asarray(Wv, f)
    Wg = np.asarray(Wg, f)
    Wo = np.asarray(Wo, f)
    mix_k = np.asarray(mix_k, f)
    mix_v = np.asarray(mix_v, f)

    Wk1 = (1.0 - mix_k)[:, None] * Wk
    Wk2 = mix_k[:, None] * Wk
    Wv1 = (1.0 - mix_v)[:, None] * Wv
    Wv2 = mix_v[:, None] * Wv

    # Wkv_t[k2][p][j] = row 256*k2+128*j+p of [Wk1|Wk2|Wv1|Wv2]
    Wkv = np.concatenate([Wk1, Wk2, Wv1, Wv2], axis=1)  # [2048, 2048]
    Wkv_t = np.ascontiguousarray(
        Wkv.reshape(8, 2, 128, 2048).transpose(0, 2, 1, 3).astype(bf))

    Wqg = np.concatenate([Wq, Wg], axis=1)  # [2048, 4096]
    # Wqg_t[n2][k2][p][j] = row 256*k2+128*j+p, cols 1024*n2..
    Wqg_t = np.ascontiguousarray(
        Wqg.reshape(8, 2, 128, 4, 1024).transpose(3, 0, 2, 1, 4).astype(bf))

    # Wo_t[n][k4][p][j] = row 512*k4+128*j+p, cols 512*n..
    Wo_t = np.ascontiguousarray(
        Wo.reshape(4, 4, 128, 4, 512).transpose(3, 0, 2, 1, 4).astype(bf))

    half = HD // 2
    inv_freq = 1.0 / (10000.0 ** (np.arange(half, dtype=np.float64) / half))
    ang = np.arange(T, dtype=np.float64)[:, None] * inv_freq[None, :]
    cos_t = np.concatenate([np.cos(ang), np.cos(ang)], axis=-1).astype(f)
    sin_t = np.concatenate([np.sin(ang), np.sin(ang)], axis=-1).astype(f)

    in_maps = []
    for c in range(NCORE):
        b, p = divmod(c, 4)
        rows_q = np.concatenate([np.arange(256 * p, 256 * p + 256),
                                 np.arange(1024 + 256 * p, 1024 + 256 * p + 256)])
        t0 = KVTOK * p
        rows_kv = np.arange(t0, t0 + KVTOK)

        xq_s = np.ascontiguousarray(xq[b, rows_q, :].astype(bf))
        xk_s = np.ascontiguousarray(xk[b, t0:t0 + KVTOK, :].astype(bf))
        xv_s = np.ascontiguousarray(xv[b, t0:t0 + KVTOK, :].astype(bf))
        xkb = np.zeros((1, D), f)
        xvb = np.zeros((1, D), f)
        if p > 0:
            xkb[0] = xk[b, t0 - 1, :]
            xvb[0] = xv[b, t0 - 1, :]

        # maskS[i][cc][j] = 1 iff kv token 128cc+i <= in-slot q token 256p+j
        ii = np.arange(128)[:, None]
        jj = np.arange(256)[None, :]
        mask = np.empty((8, 128, 256), f)
        for cc in range(8):
            mask[cc] = (128 * cc + ii <= 256 * p + jj).astype(f)
        mask = mask.transpose(1, 0, 2)  # partition-major [128, 8, 256]

        def pm(tab, rows):  # partition-major rope table [128, 4, HD]
            return np.ascontiguousarray(
                tab[rows].reshape(4, 128, HD).transpose(1, 0, 2))

        in_maps.append({
            "xq_sh": xq_s, "xk_sh": xk_s, "xv_sh": xv_s,
            "xkb": xkb.astype(bf), "xvb": xvb.astype(bf),
            "Wkv_t": Wkv_t, "Wqg_t": Wqg_t, "Wo_t": Wo_t,
            "cos_q": pm(cos_t, rows_q), "sin_q": pm(sin_t, rows_q),
            "cos_k": pm(cos_t, rows_kv), "sin_k": pm(sin_t, rows_kv),
            "maskS": np.ascontiguousarray(mask.astype(bf)),
        })
    return in_maps


def _run(in_maps, trace=False, tmpdir=None):
    _install_ntff_hook()
    from concourse.bass_utils import run_bass_kernel_spmd
    nc = _build()
    return run_bass_kernel_spmd(nc, in_maps, list(range(NCORE)),
                                trace=trace, tmpdir=tmpdir)


def kernel(xq, xk, xv, Wq, Wk, Wv, Wg, Wo, mix_k, mix_v,
           _trace=False, _tmpdir=None):
    in_maps = _host_inputs(xq, xk, xv, Wq, Wk, Wv, Wg, Wo, mix_k, mix_v)
    res = _run(in_maps, trace=_trace, tmpdir=_tmpdir)
    out = np.empty((B, T, D), np.float32)
    for c in range(NCORE):
        b, p = divmod(c, 4)
        y = res.results[c]["out_y"]
        out[b, 256 * p:256 * p + 256, :] = y[:256]
        out[b, 1024 + 256 * p:1024 + 256 * p + 256, :] = y[256:]
    kernel._last_exec_ns = res.exec_time_ns
    return out
